# revision 1
# baseline (speedup 1.0000x reference)
"""Causal MHA on 8 trn2 cores — v2b: single-phase interleaved schedule.

Sharding: 8 cores = 4 batches x 2 head-groups (8 heads each).

Schedule: proj(st=0) runs first; attention for q-tile qt is interleaved
with projection chains for s-tile st=qt+1 and the output projection of
qt-1, so the PE never waits on the ACT exp pipeline. All data bf16;
psum f32. Causal mask folded into the scores matmul group (PE). Z
reciprocal via fast DVE approx, broadcast via K=1 selector matmuls.

PSUM budget (8 banks): ss 2bufs x 2 + poA/poB 2 + chain pool 2.
"""

import sys

if "/opt/trn_rl_repo" not in sys.path:
    sys.path.insert(0, "/opt/trn_rl_repo")

import numpy as np

import concourse.bass as bass
import concourse.mybir as mybir
from concourse import bacc, tile
from concourse.bass_utils import run_bass_kernel_spmd

P = 128
D_MODEL = 1024
NUM_HEADS = 16
DK = 64
B, S = 4, 2048
HG = NUM_HEADS // 2
MG = HG * DK
N_CORES = 8

QT = S // 512
JT = S // P
KT = D_MODEL // P
MSUB = MG // P
NT = D_MODEL // P

F32 = mybir.dt.float32
F32R = mybir.dt.float32r
BF16 = mybir.dt.bfloat16
EXP = mybir.ActivationFunctionType.Exp

_CACHED_NC = None


def build_nc() -> bass.Bass:
    nc = bacc.Bacc("TRN2", target_bir_lowering=False, debug=False)

    # inputs pre-tiled host-side to partition-major layout so every DMA
    # partition-row is 8KB contiguous (8x fewer DMA packets than the
    # natural [d_model, seq] layout)
    xt4 = nc.dram_tensor("xt4", [QT, P, KT, 512], BF16, kind="ExternalInput")
    wq4 = nc.dram_tensor("wq4", [P, KT, MG], BF16, kind="ExternalInput")
    wk4 = nc.dram_tensor("wk4", [P, KT, MG], BF16, kind="ExternalInput")
    wv4 = nc.dram_tensor("wv4", [P, KT, MG], BF16, kind="ExternalInput")
    wo4 = nc.dram_tensor("wo4", [P, MSUB, D_MODEL], BF16, kind="ExternalInput")
    tmask = nc.dram_tensor("tmask", [P, P], BF16, kind="ExternalInput")
    ident2 = nc.dram_tensor("ident2", [P, 2, P], BF16, kind="ExternalInput")
    # output y^T tiled [qt, p, nt, 512]: 4 nt-tiles per DMA -> 4KB rows
    yT4 = nc.dram_tensor("yT4", [QT, P, NT, 512], BF16, kind="ExternalOutput")

    with tile.TileContext(nc) as tc:
        with (
            tc.tile_pool(name="wpool", bufs=1) as wpool,
            tc.tile_pool(name="qkv", bufs=1) as qkv,
            tc.tile_pool(name="xs", bufs=2) as xs,
            tc.tile_pool(name="oh", bufs=2) as ohp,
            tc.tile_pool(name="ys", bufs=4) as ysp,
            tc.tile_pool(name="attn", bufs=3) as attn,
            tc.tile_pool(name="attnc", bufs=1) as attnc,
            tc.tile_pool(name="ps_s", bufs=2, space="PSUM") as ps_s,
            tc.tile_pool(name="ps_o", bufs=1, space="PSUM") as ps_o,
            tc.tile_pool(name="ps_c", bufs=2, space="PSUM") as ps_c,
        ):
            # ---- persistent sbuf ----
            w_sb = {}
            for name in ("q", "k", "v"):
                w_sb[name] = wpool.tile(
                    [P, KT, MG], BF16, tag=f"w{name}", name=f"w{name}"
                )
            wo_sb = wpool.tile([P, MSUB, D_MODEL], BF16, tag="wo")
            qT_sb = qkv.tile([P, MSUB, S], BF16, tag="qT")
            kT_sb = qkv.tile([P, MSUB, S], BF16, tag="kT")
            v_sb = qkv.tile([P, JT, HG, DK + 1], BF16, tag="v")
            nc.vector.memset(v_sb[:, :, :, DK : DK + 1], 1.0)

            tm_sb = attnc.tile([P, P], BF16, tag="tm")
            id2_sb = attnc.tile([P, 2, P], BF16, tag="id2")
            # [33,128] selector: row 0 lights partitions 0:64, row 32 lights
            # 64:128 (partition bases must be 32-aligned, so Z lives in
            # partitions 0 and 32 of z2)
            sel2 = attnc.tile([33, P], F32R, tag="sel2")
            nc.vector.memset(sel2[:].bitcast(F32), 0.0)
            nc.vector.memset(sel2[0:1, 0:DK].bitcast(F32), 1.0)
            nc.vector.memset(sel2[32:33, DK:P].bitcast(F32), 1.0)

            # ---- input DMA: split across queues, first-needed-first ----
            warm_src = attnc.tile([P, 256], BF16, tag="warm_src")
            nc.vector.memset(warm_src[:], 0.5)
            nc.sync.dma_start(tm_sb[:], tmask[:])
            nc.sync.dma_start(id2_sb[:], ident2[:])
            x_tiles = [None] * QT

            def issue_x_dma(st):
                x_tiles[st] = xs.tile([P, KT, 512], BF16, tag="x", name=f"x{st}")
                for kp in range(4):
                    nc.sync.dma_start(
                        x_tiles[st][:, 2 * kp : 2 * kp + 2],
                        xt4[st, :, 2 * kp : 2 * kp + 2],
                    )

            issue_x_dma(0)
            for name, wsrc in (("q", wq4), ("k", wk4), ("v", wv4)):
                nc.sync.dma_start(w_sb[name][:, 0:4], wsrc[:, 0:4])
                nc.sync.dma_start(w_sb[name][:, 4:8], wsrc[:, 4:8])
            nc.sync.dma_start(wo_sb[:], wo4[:])

            # warm the PE while the x/w DMAs land (memset source, no DMA
            # dependency): p-state ramps before the first projection chain
            warm = ps_c.tile([P, 512], F32, tag="pp", name="warm")
            for _ in range(52):
                nc.tensor.matmul(
                    warm[:, 0:256], warm_src[:, 0:P], warm_src[:],
                    start=True, stop=True, skip_group_check=True,
                )

            # ---- filler-step factories (each step = ~4 matmuls on PE) ----
            def proj_qk_steps(name, dst, st):
                ssl = slice(st * 512, (st + 1) * 512)
                w = w_sb[name]
                x_t = x_tiles[st]
                steps = []
                for mt in range(MSUB):
                    msl = slice(mt * P, (mt + 1) * P)
                    holder = {}

                    def s1(mt=mt, msl=msl, holder=holder):
                        pt = ps_c.tile([P, 512], F32, tag="pp", name="prq")
                        holder["pt"] = pt
                        for kt in range(4):
                            nc.tensor.matmul(
                                pt[:], w[:, kt, msl], x_t[:, kt],
                                start=(kt == 0), stop=False,
                            )

                    def s2(mt=mt, msl=msl, holder=holder):
                        pt = holder["pt"]
                        for kt in range(4, KT):
                            nc.tensor.matmul(
                                pt[:], w[:, kt, msl], x_t[:, kt],
                                start=False, stop=(kt == KT - 1),
                            )
                        nc.vector.tensor_copy(dst[:, mt, ssl], pt[:])

                    steps += [s1, s2]
                return steps

            def proj_v_steps(st):
                x_t = x_tiles[st]
                steps = []
                for ssub in range(4):
                    jt = st * 4 + ssub
                    s0 = ssub * P
                    holder = {}

                    def s1(jt=jt, s0=s0, holder=holder):
                        pt = ps_c.tile([P, 512], F32, tag="pp", name="prv")
                        holder["pt"] = pt
                        for kt in range(4):
                            nc.tensor.matmul(
                                pt[:], x_t[:, kt, s0 : s0 + P], w_sb["v"][:, kt],
                                start=(kt == 0), stop=False,
                            )

                    def s2(jt=jt, s0=s0, holder=holder):
                        pt = holder["pt"]
                        for kt in range(4, KT):
                            nc.tensor.matmul(
                                pt[:], x_t[:, kt, s0 : s0 + P], w_sb["v"][:, kt],
                                start=False, stop=(kt == KT - 1),
                            )
                        nc.vector.tensor_copy(
                            v_sb[:, jt, :, 0:DK],
                            pt.rearrange("p (h d) -> p h d", h=HG),
                        )

                    steps += [s1, s2]
                return steps

            def outproj_steps(ohT_prev, qt_prev):
                steps = []
                holder = {}
                for nt in range(NT):
                    def s1(nt=nt):
                        py = ps_c.tile([P, 512], F32, tag="pp", name="py")
                        for mt in range(MSUB):
                            nc.tensor.matmul(
                                py[:],
                                wo_sb[:, mt, nt * P : (nt + 1) * P],
                                ohT_prev[:, mt, :],
                                start=(mt == 0), stop=(mt == MSUB - 1),
                            )
                        if nt % 4 == 0:
                            holder["y4"] = ysp.tile(
                                [P, 4, 512], BF16, tag="y", name="y4"
                            )
                        nc.vector.tensor_copy(holder["y4"][:, nt % 4, :], py[:])
                        if nt % 4 == 3:  # 4 tiles buffered -> one 4KB-row DMA
                            nc.sync.dma_start(
                                yT4[qt_prev, :, nt - 3 : nt + 1], holder["y4"][:]
                            )

                    steps.append(s1)
                return steps

            # ---- attention primitives ----
            def emit_scores(qt, hp, jt):
                jsl = slice(jt * P, (jt + 1) * P)
                di = jt - qt * 4
                delta = 128 * di if di >= 0 else 0
                qsl_d = slice(qt * 512 + delta, (qt + 1) * 512)
                ss = ps_s.tile([P, 2, 512], F32, tag="ss")
                nc.tensor.matmul(
                    ss[:, 0, delta:],
                    kT_sb[0:DK, hp, jsl],
                    qT_sb[0:DK, hp, qsl_d],
                    start=True, stop=False, skip_group_check=True,
                )
                nc.tensor.matmul(
                    ss[:, 1, delta:],
                    kT_sb[DK:P, hp, jsl],
                    qT_sb[DK:P, hp, qsl_d],
                    start=True, stop=(di < 0), skip_group_check=True,
                )
                if di >= 0:
                    nc.tensor.matmul(
                        ss[:, :, delta : delta + P],
                        tm_sb[:], id2_sb[:],
                        start=False, stop=True, skip_group_check=True,
                    )
                return ss, delta

            # ---- main interleaved schedule ----
            # x(st=1) streams while proj(st=0) runs standalone (attention
            # qt=0 depends on proj(st=0))
            issue_x_dma(1)
            for step in proj_qk_steps("q", qT_sb, 0):
                step()
            for step in proj_qk_steps("k", kT_sb, 0):
                step()
            for step in proj_v_steps(0):
                step()

            prev = None  # (ohT, qt) with outproj pending
            deferred_kv3 = None
            fin = {}
            for qt in range(QT):
                # x DMA two q-tiles ahead (xs bufs=2: the tile waits for the
                # previous generation's readers automatically)
                if qt + 2 < QT:
                    issue_x_dma(qt + 2)
                fillers = []
                nfront = 0
                if qt + 1 < QT:
                    st = qt + 1
                    fillers += proj_qk_steps("q", qT_sb, st)
                    if st < QT - 1:
                        fillers += proj_qk_steps("k", kT_sb, st)
                        fillers += proj_v_steps(st)
                    else:
                        # defer k/v(st=3) into qt=3's early blocks: qt3 is
                        # ACT(exp)-bound, so this PE work fills its bubbles
                        deferred_kv3 = (
                            proj_qk_steps("k", kT_sb, st) + proj_v_steps(st)
                        )
                if qt == QT - 1 and deferred_kv3 is not None:
                    fillers = deferred_kv3 + fillers
                    # k/v(st3) feeds this qt's jt>=12 blocks: hp0 hits jt=12
                    # at block 12, so these steps must front-load
                    nfront = len(deferred_kv3)
                if prev is not None:
                    fillers += outproj_steps(prev[0], prev[1])

                njt = 4 * (qt + 1)
                ohT = ohp.tile([P, MSUB, 512], BF16, tag="ohT")
                work = [(hp, jt) for hp in range(MSUB) for jt in range(njt)]
                nw = len(work)
                nf = len(fillers)
                fi = 0
                po = {}
                pend = emit_scores(qt, 0, 0)
                for wi, (hp, jt) in enumerate(work):
                    if jt == 0:
                        po[hp] = (
                            ps_o.tile([DK + 1, 512], F32, tag="poA", name="po_a"),
                            ps_o.tile([DK + 1, 512], F32, tag="poB", name="po_b"),
                        )
                    ss, delta = pend
                    pp = attn.tile([P, 2, 512], BF16, tag="pp")
                    nc.scalar.activation(
                        pp[:, :, delta:], ss[:, :, delta:], EXP, scale=0.125
                    )
                    # next block's scores ahead of this block's PV
                    if wi + 1 < nw:
                        pend = emit_scores(qt, *work[wi + 1])
                    # filler quota for this block (front-loaded steps must
                    # finish within the first 11 blocks)
                    want = (wi + 1) * nf // nw
                    if nfront and wi < 12:
                        want = max(want, min(nfront, (wi + 1) * nfront // 11))
                    while fi < want:
                        fillers[fi]()
                        fi += 1
                    po_a, po_b = po[hp]
                    nc.tensor.matmul(
                        po_a[:, delta:],
                        v_sb[:, jt, 2 * hp, :],
                        pp[:, 0, delta:],
                        start=(jt == 0), stop=(jt == njt - 1),
                    )
                    nc.tensor.matmul(
                        po_b[:, delta:],
                        v_sb[:, jt, 2 * hp + 1, :],
                        pp[:, 1, delta:],
                        start=(jt == 0), stop=(jt == njt - 1),
                    )
                    if jt == njt - 1:
                        z2 = attnc.tile([33, 512], F32R, tag="z2")
                        if hp == 0 and qt == 0:
                            # rows 1..31 are never written; zero once so the
                            # K=33 broadcast matmul multiplies 0 * 0
                            nc.vector.memset(z2[:].bitcast(F32), 0.0)
                        dst = ohT[:, hp, :]
                        nc.vector.tensor_copy(z2[0:1, :], po_a[DK : DK + 1, :])
                        nc.vector.tensor_copy(z2[32:33, :], po_b[DK : DK + 1, :])
                        nc.vector.tensor_copy(dst[0:DK], po_a[0:DK, :])
                        nc.vector.tensor_copy(dst[DK:P], po_b[0:DK, :])
                        last_ep = qt == QT - 1 and hp == MSUB - 1
                        if last_ep:
                            # very last epilogue: pre-open the first final
                            # outproj chain (mt 0..2 only need already-scaled
                            # heads) so the PE covers the z2->bcz->mult chain
                            fin["py0"] = fin_py0 = ps_c.tile(
                                [P, 512], F32, tag="pp", name="py0f"
                            )
                            for mt in range(MSUB - 1):
                                nc.tensor.matmul(
                                    fin_py0[:],
                                    wo_sb[:, mt, 0:P],
                                    ohT[:, mt, :],
                                    start=(mt == 0), stop=False,
                                )
                        # keep the PE busy while the DVE drains z2 (the bcz
                        # matmul below would otherwise head-of-line block)
                        for _ in range(2):
                            if fi < nf:
                                fillers[fi]()
                                fi += 1
                        bcz = ps_c.tile([P, 512], F32, tag="pp", name="bcz")
                        nc.tensor.matmul(
                            bcz[:], sel2[:], z2[:], start=True, stop=True
                        )
                        bcr = attnc.tile([P, 512], F32, tag="bcr")
                        nc.vector.reciprocal_approx_fast(bcr[:], bcz[:])
                        nc.gpsimd.tensor_tensor(
                            dst, dst, bcr[:], mybir.AluOpType.mult
                        )
                        if last_ep:
                            nc.tensor.matmul(
                                fin_py0[:],
                                wo_sb[:, MSUB - 1, 0:P],
                                ohT[:, MSUB - 1, :],
                                start=False, stop=True,
                            )
                while fi < len(fillers):
                    fillers[fi]()
                    fi += 1
                prev = (ohT, qt)
            # final output projection: nt=0 was computed inside the last
            # epilogue; evict it and run nt 1..7
            ohT3, qt3 = prev
            y2f = None
            for nt in range(NT):
                if nt % 2 == 0:
                    y2f = ysp.tile([P, 2, 512], BF16, tag="y2", name="y2f")
                if nt == 0:
                    py = fin["py0"]
                else:
                    py = ps_c.tile([P, 512], F32, tag="pp", name="pyf")
                    for mt in range(MSUB):
                        nc.tensor.matmul(
                            py[:],
                            wo_sb[:, mt, nt * P : (nt + 1) * P],
                            ohT3[:, mt, :],
                            start=(mt == 0), stop=(mt == MSUB - 1),
                        )
                # alternate eviction engines so chains never wait on one
                if nt % 2 == 0:
                    nc.vector.tensor_copy(y2f[:, 0, :], py[:])
                else:
                    nc.scalar.copy(y2f[:, 1, :], py[:])
                    nc.sync.dma_start(yT4[qt3, :, nt - 1 : nt + 1], y2f[:])

    nc.finalize()
    return nc


def _get_nc() -> bass.Bass:
    global _CACHED_NC
    if _CACHED_NC is None:
        _CACHED_NC = build_nc()
    return _CACHED_NC


def _make_masks() -> np.ndarray:
    import ml_dtypes

    k = np.arange(P)[:, None]
    j = np.arange(P)[None, :]
    return np.where(j > k, -30000.0, 0.0).astype(ml_dtypes.bfloat16)


def _make_ident2() -> np.ndarray:
    import ml_dtypes

    e = np.eye(P, dtype=np.float32)
    return np.stack([e, e], axis=1).astype(ml_dtypes.bfloat16)


def make_in_maps(inputs):
    import ml_dtypes

    bf = ml_dtypes.bfloat16
    x = np.asarray(inputs["x"], np.float32)
    q_heads = np.asarray(inputs["q_heads"], np.float32)
    k_heads = np.asarray(inputs["k_heads"], np.float32)
    v_heads = np.asarray(inputs["v_heads"], np.float32)
    output_proj = np.asarray(inputs["output_proj"], np.float32)

    tm = _make_masks()
    id2 = _make_ident2()

    def tile_w(wT):  # [1024, 512] -> [p, kt, m]
        return np.ascontiguousarray(
            wT.reshape(KT, P, MG).transpose(1, 0, 2)
        ).astype(bf)

    in_maps = []
    for core in range(N_CORES):
        b, g = divmod(core, 2)
        gsl = slice(g * MG, (g + 1) * MG)
        xT = x[b].T  # [1024, 2048]
        xt4 = np.ascontiguousarray(
            xT.reshape(KT, P, QT, 512).transpose(2, 1, 0, 3)
        ).astype(bf)  # [st, p, kt, 512]
        wo = output_proj[:, gsl].T  # [512, 1024]
        wo4 = np.ascontiguousarray(
            wo.reshape(MSUB, P, D_MODEL).transpose(1, 0, 2)
        ).astype(bf)
        in_maps.append(
            {
                "xt4": xt4,
                "wq4": tile_w(q_heads[gsl].T),
                "wk4": tile_w(k_heads[gsl].T),
                "wv4": tile_w(v_heads[gsl].T),
                "wo4": wo4,
                "tmask": tm,
                "ident2": id2,
            }
        )
    return in_maps


def kernel(x, q_heads, k_heads, v_heads, output_proj):
    inputs = {
        "x": x,
        "q_heads": q_heads,
        "k_heads": k_heads,
        "v_heads": v_heads,
        "output_proj": output_proj,
    }
    in_maps = make_in_maps(inputs)
    nc = _get_nc()
    res = run_bass_kernel_spmd(nc, in_maps, list(range(N_CORES)))
    y = np.empty((B, S, D_MODEL), np.float32)
    for b in range(B):
        # yT4 [qt, p, nt, 512] -> yT [nt*128+p, qt*512+c]
        acc = res.results[2 * b]["yT4"].astype(np.float32) + res.results[
            2 * b + 1
        ]["yT4"].astype(np.float32)
        yT = acc.transpose(2, 1, 0, 3).reshape(D_MODEL, S)
        y[b] = yT.T
    return y



# revision 16
# speedup vs baseline: 11638.3483x; 11638.3483x over previous
"""Causal MHA on 8 trn2 cores — v3: transposed-PV schedule.

Sharding: 8 cores = 4 batches x 2 head-groups (8 heads each).

v3 changes vs v2:
- PV computed transposed: out[q, dk] = pp[keys, q].T @ v[keys, dk] —
  stationary = probs tile, moving = v (N=64) — halves PV matmul time
  (cost scales with moving free size; old orientation paid N=512 for
  M=65 useful rows).
- Z (softmax denom) via N=1 ones-column matmuls chained like PV.
- causal mask as a post-exp DVE 0/1 multiply (off the PE).
- per-(head,qsub) epilogue: reciprocal + per-partition tensor_scalar
  scale, then a PE transpose places oh back in [dk, q] for the output
  projection (tile_position lands head 1 in partitions 64:128).
- per-head scores/exp (ss [P,512] f32, 3 PSUM banks) with 2-deep
  pend-ahead so ACT stays fed through the exp-bound late q-tiles.

PSUM budget (8 banks): ss 3 + pv 2 + chain 2 + (z, tp slivers) 1.
"""

import sys

if "/opt/trn_rl_repo" not in sys.path:
    sys.path.insert(0, "/opt/trn_rl_repo")

import numpy as np

import concourse.bass as bass
import concourse.mybir as mybir
from concourse import bacc, tile
from concourse.bass_utils import run_bass_kernel_spmd

P = 128
D_MODEL = 1024
NUM_HEADS = 16
DK = 64
B, S = 4, 2048
HG = NUM_HEADS // 2
MG = HG * DK
N_CORES = 8

QT = S // 512
JT = S // P
KT = D_MODEL // P
MSUB = MG // P
NT = D_MODEL // P

F32 = mybir.dt.float32
BF16 = mybir.dt.bfloat16
EXP = mybir.ActivationFunctionType.Exp
MULT = mybir.AluOpType.mult


def build_nc() -> bass.Bass:
    nc = bacc.Bacc("TRN2", target_bir_lowering=False, debug=False)

    # inputs pre-tiled host-side to partition-major layout so every DMA
    # partition-row is 8KB contiguous
    xt4 = nc.dram_tensor("xt4", [QT, P, KT, 512], BF16, kind="ExternalInput")
    wq4 = nc.dram_tensor("wq4", [P, KT, MG], BF16, kind="ExternalInput")
    wk4 = nc.dram_tensor("wk4", [P, KT, MG], BF16, kind="ExternalInput")
    wv4 = nc.dram_tensor("wv4", [P, KT, MG], BF16, kind="ExternalInput")
    wo4 = nc.dram_tensor("wo4", [P, MSUB, D_MODEL], BF16, kind="ExternalInput")
    tm2 = nc.dram_tensor("tm2", [P, 2, P], BF16, kind="ExternalInput")
    ident = nc.dram_tensor("ident", [P, P], BF16, kind="ExternalInput")
    # output y^T tiled [qt, p, nt, 512]
    yT4 = nc.dram_tensor("yT4", [QT, P, NT, 512], BF16, kind="ExternalOutput")

    with tile.TileContext(nc) as tc:
        with (
            tc.tile_pool(name="wpool", bufs=1) as wpool,
            tc.tile_pool(name="qkv", bufs=1) as qkv,
            tc.tile_pool(name="xs", bufs=2) as xs,
            tc.tile_pool(name="oh", bufs=2) as ohp,
            tc.tile_pool(name="ys", bufs=4) as ysp,
            tc.tile_pool(name="attn", bufs=4) as attn,
            tc.tile_pool(name="attnc", bufs=1) as attnc,
            tc.tile_pool(name="ohq", bufs=6) as ohqp,
            tc.tile_pool(name="zr", bufs=6) as zrp,
            # PSUM pools — creation order fixes bank packing:
            # ss 3 banks, pv 2, chains 2, z+tp slivers in bank 8
            tc.tile_pool(name="ps_s", bufs=2, space="PSUM") as ps_s,
            tc.tile_pool(name="ps_v", bufs=2, space="PSUM") as ps_v,
            tc.tile_pool(name="ps_c", bufs=2, space="PSUM") as ps_c,
        ):
            # ---- persistent sbuf ----
            w_sb = {}
            for name in ("q", "k", "v"):
                w_sb[name] = wpool.tile(
                    [P, KT, MG], BF16, tag=f"w{name}", name=f"w{name}"
                )
            wo_sb = wpool.tile([P, MSUB, D_MODEL], BF16, tag="wo")
            qT_sb = qkv.tile([P, MSUB, S], BF16, tag="qT")
            kT_sb = qkv.tile([P, MSUB, S], BF16, tag="kT")
            v_sb = qkv.tile([P, JT, HG, DK + 1], BF16, tag="v")
            nc.vector.memset(v_sb[:, :, :, DK : DK + 1], 1.0)

            tm_sb = attnc.tile([P, 2, P], BF16, tag="tm")
            id_sb = attnc.tile([P, P], BF16, tag="id")

            # ---- input DMA ----
            warm_src = attnc.tile([P, 256], BF16, tag="warm_src")
            nc.vector.memset(warm_src[:], 0.5)
            nc.sync.dma_start(tm_sb[:], tm2[:])
            nc.sync.dma_start(id_sb[:], ident[:])
            x_tiles = [None] * QT

            def issue_x_dma(st):
                x_tiles[st] = xs.tile([P, KT, 512], BF16, tag="x", name=f"x{st}")
                for kp in range(4):
                    nc.sync.dma_start(
                        x_tiles[st][:, 2 * kp : 2 * kp + 2],
                        xt4[st, :, 2 * kp : 2 * kp + 2],
                    )

            issue_x_dma(0)
            for name, wsrc in (("q", wq4), ("k", wk4), ("v", wv4)):
                nc.sync.dma_start(w_sb[name][:, 0:4], wsrc[:, 0:4])
                nc.sync.dma_start(w_sb[name][:, 4:8], wsrc[:, 4:8])
            nc.sync.dma_start(wo_sb[:], wo4[:])

            # warm the PE while the x/w DMAs land
            warm = ps_c.tile([P, 512], F32, tag="pp", name="warm")
            for _ in range(18):
                nc.tensor.matmul(
                    warm[:, 0:256], warm_src[:, 0:P], warm_src[:],
                    start=True, stop=True, skip_group_check=True,
                )

            # ---- filler-step factories (each step = ~4 matmuls on PE) ----
            def proj_qk_steps(name, dst, st):
                ssl = slice(st * 512, (st + 1) * 512)
                w = w_sb[name]
                x_t = x_tiles[st]
                steps = []
                for mt in range(MSUB):
                    msl = slice(mt * P, (mt + 1) * P)
                    holder = {}

                    def s1(mt=mt, msl=msl, holder=holder):
                        pt = ps_c.tile([P, 512], F32, tag="pp", name="prq")
                        holder["pt"] = pt
                        for kt in range(4):
                            nc.tensor.matmul(
                                pt[:], w[:, kt, msl], x_t[:, kt],
                                start=(kt == 0), stop=False,
                            )

                    def s2(mt=mt, msl=msl, holder=holder):
                        pt = holder["pt"]
                        for kt in range(4, KT):
                            nc.tensor.matmul(
                                pt[:], w[:, kt, msl], x_t[:, kt],
                                start=False, stop=(kt == KT - 1),
                            )
                        nc.vector.tensor_copy(dst[:, mt, ssl], pt[:])

                    steps += [s1, s2]
                return steps

            def proj_v_steps(st):
                x_t = x_tiles[st]
                steps = []
                for ssub in range(4):
                    jt = st * 4 + ssub
                    s0 = ssub * P
                    holder = {}

                    def s1(jt=jt, s0=s0, holder=holder):
                        pt = ps_c.tile([P, 512], F32, tag="pp", name="prv")
                        holder["pt"] = pt
                        for kt in range(4):
                            nc.tensor.matmul(
                                pt[:], x_t[:, kt, s0 : s0 + P], w_sb["v"][:, kt],
                                start=(kt == 0), stop=False,
                            )

                    def s2(jt=jt, s0=s0, holder=holder):
                        pt = holder["pt"]
                        for kt in range(4, KT):
                            nc.tensor.matmul(
                                pt[:], x_t[:, kt, s0 : s0 + P], w_sb["v"][:, kt],
                                start=False, stop=(kt == KT - 1),
                            )
                        nc.vector.tensor_copy(
                            v_sb[:, jt, :, 0:DK],
                            pt.rearrange("p (h d) -> p h d", h=HG),
                        )

                    steps += [s1, s2]
                return steps

            def outproj_steps(ohT_prev, qt_prev):
                steps = []
                holder = {}
                for nt in range(NT):
                    def s1(nt=nt):
                        py = ps_c.tile([P, 512], F32, tag="pp", name="py")
                        for mt in range(MSUB):
                            nc.tensor.matmul(
                                py[:],
                                wo_sb[:, mt, nt * P : (nt + 1) * P],
                                ohT_prev[:, mt, :],
                                start=(mt == 0), stop=(mt == MSUB - 1),
                            )
                        if nt % 4 == 0:
                            holder["y4"] = ysp.tile(
                                [P, 4, 512], BF16, tag="y", name="y4"
                            )
                        nc.vector.tensor_copy(holder["y4"][:, nt % 4, :], py[:])
                        if nt % 4 == 3:
                            nc.sync.dma_start(
                                yT4[qt_prev, :, nt - 3 : nt + 1], holder["y4"][:]
                            )

                    steps.append(s1)
                return steps

            # ---- attention primitives ----
            def emit_scores(qt, hp, jt, ctx):
                """Two K=64 matmuls: ss[keys, h, q] for the pair's heads."""
                jsl = slice(jt * P, (jt + 1) * P)
                di = jt - qt * 4
                delta = P * di if di >= 0 else 0
                qsl_d = slice(qt * 512 + delta, (qt + 1) * 512)
                ss = ps_s.tile([P, 2, 512], F32, tag="ss")
                for h in range(2):
                    hd = slice(h * DK, (h + 1) * DK)
                    nc.tensor.matmul(
                        ss[:, h, delta:],
                        kT_sb[hd, hp, jsl],
                        qT_sb[hd, hp, qsl_d],
                        start=True, stop=True, skip_group_check=True,
                    )
                ctx["ss"] = ss
                ctx["delta"] = delta
                ctx["di"] = di

            def emit_exp(ctx):
                ss, delta, di = ctx["ss"], ctx["delta"], ctx["di"]
                pp = attn.tile([P, 2, 512], BF16, tag="pp")
                nc.scalar.activation(
                    pp[:, :, delta:], ss[:, :, delta:], EXP, scale=0.125
                )
                if di >= 0:
                    # zero the upper-triangle of the diagonal 128-block
                    nc.vector.tensor_tensor(
                        pp[:, :, delta : delta + P],
                        pp[:, :, delta : delta + P],
                        tm_sb[:],
                        MULT,
                    )
                ctx["pp"] = pp

            # ---- main interleaved schedule ----
            issue_x_dma(1)
            for step in proj_qk_steps("q", qT_sb, 0):
                step()
            for step in proj_qk_steps("k", kT_sb, 0):
                step()
            for step in proj_v_steps(0):
                step()

            prev = None  # (ohT, qt) with outproj pending
            deferred_kv3 = None
            fin = {}
            for qt in range(QT):
                if qt + 2 < QT:
                    issue_x_dma(qt + 2)
                fillers = []
                nfront = 0
                if qt + 1 < QT:
                    st = qt + 1
                    fillers += proj_qk_steps("q", qT_sb, st)
                    if st < QT - 1:
                        fillers += proj_qk_steps("k", kT_sb, st)
                        fillers += proj_v_steps(st)
                    else:
                        # defer k/v(st=3) into qt=3 (exp-bound there)
                        deferred_kv3 = (
                            proj_qk_steps("k", kT_sb, st) + proj_v_steps(st)
                        )
                if qt == QT - 1 and deferred_kv3 is not None:
                    fillers = deferred_kv3 + fillers
                    # k/v(st3) feeds qt3's jt>=12 blocks (unit ~24 of hp0)
                    nfront = len(deferred_kv3)
                if prev is not None:
                    fillers += outproj_steps(prev[0], prev[1])

                njt = 4 * (qt + 1)
                ohT = ohp.tile([P, MSUB, 512], BF16, tag="ohT")
                # blocks: pair-granular (hp, jt); pv accumulators per head
                units = [
                    (hp, jt) for hp in range(MSUB) for jt in range(njt)
                ]
                nu = len(units)
                nf = len(fillers)
                fi = 0
                sweep_pv = {}  # (hp, h) -> pv_tile
                pend_tp = []  # (ohq_t, hp, h, qs, wi_pushed)
                pend_cp = []  # (tpt, hp, h, qs, wi_emitted)
                ctxs = {}

                # oh transpose via regular matmul against the identity:
                # out[d, j] = sum_q ohq[q, d] * I[q, j] = ohq.T.  Each
                # transpose gets its own chain-pool generation — PSUM
                # start=True zeroes lazily at bank granularity, so an
                # accumulator bank must never host two live groups.
                def do_transpose(ent, wi):
                    ohq_t, ehp, eh, qs = ent
                    tpt = ps_c.tile([P, 512], F32, tag="pp", name="tp")
                    nc.tensor.matmul(
                        tpt[eh * DK : (eh + 1) * DK, 0:P],
                        ohq_t[:],
                        id_sb[:],
                        start=True, stop=True,
                        skip_group_check=True,
                    )
                    pend_cp.append((tpt, ehp, eh, qs, wi))

                def do_copy(ent):
                    tpt, ehp, eh, qs, _ = ent
                    nc.vector.tensor_copy(
                        ohT[eh * DK : (eh + 1) * DK, ehp, qs * P : (qs + 1) * P],
                        tpt[eh * DK : (eh + 1) * DK, 0:P],
                    )

                # prologue: emit scores for blocks 0,1 and exp for block 0
                ctxs[0] = {}
                emit_scores(qt, *units[0], ctxs[0])
                if nu > 1:
                    ctxs[1] = {}
                    emit_scores(qt, *units[1], ctxs[1])
                emit_exp(ctxs[0])

                for wi, (hp, jt) in enumerate(units):
                    ctx = ctxs.pop(wi)
                    # issue exp for the NEXT block early (ACT queue ahead)
                    if wi + 1 < nu:
                        emit_exp(ctxs[wi + 1])
                    # epilogue pipeline: transpose ~2 blocks after the DVE
                    # scale was issued, ohT copy ~1 block after the transpose
                    while pend_tp and pend_tp[0][4] <= wi - 2:
                        ent = pend_tp.pop(0)
                        do_transpose(ent[:4], wi)
                    while pend_cp and pend_cp[0][4] <= wi - 1:
                        do_copy(pend_cp.pop(0))
                    # PV for current block (both heads)
                    if jt == 0:
                        for h in range(2):
                            sweep_pv[hp, h] = ps_v.tile(
                                [P, 4, DK + 1], F32, tag="pv", name="pv"
                            )
                    pp = ctx["pp"]
                    di = ctx["di"]
                    q0 = di if di > 0 else 0
                    for h in range(2):
                        pv_t = sweep_pv[hp, h]
                        hh = hp * 2 + h
                        for qs in range(q0, 4):
                            last = jt == qt * 4 + qs
                            # start only on the bank's first group touch:
                            # PSUM start zeroes the whole bank lazily, so
                            # sibling slices rely on that single mark
                            nc.tensor.matmul(
                                pv_t[:, qs, :],
                                pp[:, h, qs * P : (qs + 1) * P],
                                v_sb[:, jt, hh, :],
                                start=(jt == 0 and qs == q0), stop=last,
                                skip_group_check=True,
                            )
                            if last:
                                # epilogue DVE: 1/z then scale into sbuf
                                zr = zrp.tile([P, 1], F32, tag="zr")
                                nc.vector.reciprocal(
                                    zr[:], pv_t[:, qs, DK : DK + 1]
                                )
                                ohq_t = ohqp.tile([P, DK], BF16, tag="ohq")
                                nc.vector.tensor_scalar_mul(
                                    ohq_t[:], pv_t[:, qs, 0:DK], zr[:]
                                )
                                pend_tp.append((ohq_t, hp, h, qs, wi))
                    # emit scores for block wi+2 (pend-ahead depth 2)
                    if wi + 2 < nu:
                        ctxs[wi + 2] = {}
                        emit_scores(qt, *units[wi + 2], ctxs[wi + 2])
                    # filler quota
                    want = (wi + 1) * nf // nu
                    if nfront and wi < 12:
                        want = max(want, min(nfront, (wi + 1) * nfront // 11))
                    while fi < want:
                        fillers[fi]()
                        fi += 1

                # drain pipeline at qt end, fillers between PE stages
                nrem = len(pend_tp)
                for ri in range(nrem):
                    ent = pend_tp.pop(0)
                    if fi < nf:
                        fillers[fi]()
                        fi += 1
                    do_transpose(ent[:4], nu + ri)
                    while pend_cp and pend_cp[0][4] <= nu + ri - 1:
                        do_copy(pend_cp.pop(0))
                while fi < nf:
                    fillers[fi]()
                    fi += 1
                while pend_cp:
                    do_copy(pend_cp.pop(0))
                prev = (ohT, qt)

            # final output projection for qt=3
            ohT3, qt3 = prev
            y2f = None
            for nt in range(NT):
                if nt % 2 == 0:
                    y2f = ysp.tile([P, 2, 512], BF16, tag="y2", name="y2f")
                py = ps_c.tile([P, 512], F32, tag="pp", name="pyf")
                for mt in range(MSUB):
                    nc.tensor.matmul(
                        py[:],
                        wo_sb[:, mt, nt * P : (nt + 1) * P],
                        ohT3[:, mt, :],
                        start=(mt == 0), stop=(mt == MSUB - 1),
                    )
                if nt % 2 == 0:
                    nc.vector.tensor_copy(y2f[:, 0, :], py[:])
                else:
                    nc.scalar.copy(y2f[:, 1, :], py[:])
                    nc.sync.dma_start(yT4[qt3, :, nt - 1 : nt + 1], y2f[:])

    nc.finalize()
    return nc


_CACHED_NC = None


def _get_nc() -> bass.Bass:
    global _CACHED_NC
    if _CACHED_NC is None:
        _CACHED_NC = build_nc()
    return _CACHED_NC


def _make_tm2() -> np.ndarray:
    import ml_dtypes

    k = np.arange(P)[:, None]
    j = np.arange(P)[None, :]
    tm = np.where(j >= k, 1.0, 0.0).astype(np.float32)
    return np.stack([tm, tm], axis=1).astype(ml_dtypes.bfloat16)


def _make_ident() -> np.ndarray:
    import ml_dtypes

    return np.eye(P, dtype=np.float32).astype(ml_dtypes.bfloat16)


def make_in_maps(inputs):
    import ml_dtypes

    bf = ml_dtypes.bfloat16
    x = np.asarray(inputs["x"], np.float32)
    q_heads = np.asarray(inputs["q_heads"], np.float32)
    k_heads = np.asarray(inputs["k_heads"], np.float32)
    v_heads = np.asarray(inputs["v_heads"], np.float32)
    output_proj = np.asarray(inputs["output_proj"], np.float32)

    tm = _make_tm2()
    idn = _make_ident()

    def tile_w(wT):  # [1024, 512] -> [p, kt, m]
        return np.ascontiguousarray(
            wT.reshape(KT, P, MG).transpose(1, 0, 2)
        ).astype(bf)

    in_maps = []
    for core in range(N_CORES):
        b, g = divmod(core, 2)
        gsl = slice(g * MG, (g + 1) * MG)
        xT = x[b].T  # [1024, 2048]
        xt4 = np.ascontiguousarray(
            xT.reshape(KT, P, QT, 512).transpose(2, 1, 0, 3)
        ).astype(bf)  # [st, p, kt, 512]
        wo = output_proj[:, gsl].T  # [512, 1024]
        wo4 = np.ascontiguousarray(
            wo.reshape(MSUB, P, D_MODEL).transpose(1, 0, 2)
        ).astype(bf)
        in_maps.append(
            {
                "xt4": xt4,
                "wq4": tile_w(q_heads[gsl].T),
                "wk4": tile_w(k_heads[gsl].T),
                "wv4": tile_w(v_heads[gsl].T),
                "wo4": wo4,
                "tm2": tm,
                "ident": idn,
            }
        )
    return in_maps


def kernel(x, q_heads, k_heads, v_heads, output_proj):
    inputs = {
        "x": x,
        "q_heads": q_heads,
        "k_heads": k_heads,
        "v_heads": v_heads,
        "output_proj": output_proj,
    }
    in_maps = make_in_maps(inputs)
    nc = _get_nc()
    res = run_bass_kernel_spmd(nc, in_maps, list(range(N_CORES)))
    y = np.empty((B, S, D_MODEL), np.float32)
    for b in range(B):
        acc = res.results[2 * b]["yT4"].astype(np.float32) + res.results[
            2 * b + 1
        ]["yT4"].astype(np.float32)
        yT = acc.transpose(2, 1, 0, 3).reshape(D_MODEL, S)
        y[b] = yT.T
    return y


# revision 34
# speedup vs baseline: 11837.0847x; 1.0171x over previous
"""Causal MHA on 8 trn2 cores — v3: transposed-PV schedule.

Sharding: 8 cores = 4 batches x 2 head-groups (8 heads each).

v3 changes vs v2:
- PV computed transposed: out[q, dk] = pp[keys, q].T @ v[keys, dk] —
  stationary = probs tile, moving = v (N=64) — halves PV matmul time
  (cost scales with moving free size; old orientation paid N=512 for
  M=65 useful rows).
- Z (softmax denom) via N=1 ones-column matmuls chained like PV.
- causal mask as a post-exp DVE 0/1 multiply (off the PE).
- per-(head,qsub) epilogue: reciprocal + per-partition tensor_scalar
  scale, then a PE transpose places oh back in [dk, q] for the output
  projection (tile_position lands head 1 in partitions 64:128).
- per-head scores/exp (ss [P,512] f32, 3 PSUM banks) with 2-deep
  pend-ahead so ACT stays fed through the exp-bound late q-tiles.

PSUM budget (8 banks): ss 3 + pv 2 + chain 2 + (z, tp slivers) 1.
"""

import sys

if "/opt/trn_rl_repo" not in sys.path:
    sys.path.insert(0, "/opt/trn_rl_repo")

import numpy as np

import concourse.bass as bass
import concourse.mybir as mybir
from concourse import bacc, tile
from concourse.bass_utils import run_bass_kernel_spmd

P = 128
D_MODEL = 1024
NUM_HEADS = 16
DK = 64
B, S = 4, 2048
HG = NUM_HEADS // 2
MG = HG * DK
N_CORES = 8

QT = S // 512
JT = S // P
KT = D_MODEL // P
MSUB = MG // P
NT = D_MODEL // P

F32 = mybir.dt.float32
BF16 = mybir.dt.bfloat16
EXP = mybir.ActivationFunctionType.Exp
MULT = mybir.AluOpType.mult


def build_nc() -> bass.Bass:
    nc = bacc.Bacc("TRN2", target_bir_lowering=False, debug=False)

    # inputs pre-tiled host-side to partition-major layout so every DMA
    # partition-row is 8KB contiguous
    xt4 = nc.dram_tensor("xt4", [QT, P, KT, 512], BF16, kind="ExternalInput")
    wq4 = nc.dram_tensor("wq4", [P, KT, MG], BF16, kind="ExternalInput")
    wk4 = nc.dram_tensor("wk4", [P, KT, MG], BF16, kind="ExternalInput")
    wv4 = nc.dram_tensor("wv4", [P, KT, MG], BF16, kind="ExternalInput")
    wo4 = nc.dram_tensor("wo4", [P, MSUB, D_MODEL], BF16, kind="ExternalInput")
    tm2 = nc.dram_tensor("tm2", [P, 2, P], BF16, kind="ExternalInput")
    ident = nc.dram_tensor("ident", [P, P], BF16, kind="ExternalInput")
    # output y^T tiled [qt, p, nt, 512]
    yT4 = nc.dram_tensor("yT4", [QT, P, NT, 512], BF16, kind="ExternalOutput")

    with tile.TileContext(nc) as tc:
        with (
            tc.tile_pool(name="wpool", bufs=1) as wpool,
            tc.tile_pool(name="qkv", bufs=1) as qkv,
            tc.tile_pool(name="xs", bufs=2) as xs,
            tc.tile_pool(name="oh", bufs=3) as ohp,
            tc.tile_pool(name="ys", bufs=4) as ysp,
            tc.tile_pool(name="attn", bufs=4) as attn,
            tc.tile_pool(name="attnc", bufs=1) as attnc,
            tc.tile_pool(name="ohq", bufs=6) as ohqp,
            tc.tile_pool(name="zr", bufs=6) as zrp,
            # PSUM pools — creation order fixes bank packing:
            # ss 3 banks, pv 2, chains 2, z+tp slivers in bank 8
            tc.tile_pool(name="ps_s", bufs=2, space="PSUM") as ps_s,
            tc.tile_pool(name="ps_v", bufs=2, space="PSUM") as ps_v,
            tc.tile_pool(name="ps_c", bufs=2, space="PSUM") as ps_c,
        ):
            # ---- persistent sbuf ----
            w_sb = {}
            for name in ("q", "k", "v"):
                w_sb[name] = wpool.tile(
                    [P, KT, MG], BF16, tag=f"w{name}", name=f"w{name}"
                )
            wo_sb = wpool.tile([P, MSUB, D_MODEL], BF16, tag="wo")
            qT_sb = qkv.tile([P, MSUB, S], BF16, tag="qT")
            kT_sb = qkv.tile([P, MSUB, S], BF16, tag="kT")
            v_sb = qkv.tile([P, JT, HG, DK + 1], BF16, tag="v")

            tm_sb = attnc.tile([P, 2, P], BF16, tag="tm")
            id_sb = attnc.tile([P, P], BF16, tag="id")

            # ---- input DMA ----
            # issue cost is ~565-667ns per dma_start, serial per engine —
            # split the startup DMAs across the two HWDGE engines (SP +
            # Activation; ACT is idle until the first exp) so the first
            # projection's dependencies land by ~2.5us
            warm_src = attnc.tile([P, 256], BF16, tag="warm_src")
            nc.vector.memset(warm_src[:], 0.5)
            nc.gpsimd.memset(v_sb[:, :, :, DK : DK + 1], 1.0)
            x_tiles = [None] * QT

            def issue_x_dma(st):
                x_tiles[st] = xs.tile([P, KT, 512], BF16, tag="x", name=f"x{st}")
                for kp in range(4):
                    nc.sync.dma_start(
                        x_tiles[st][:, 2 * kp : 2 * kp + 2],
                        xt4[st, :, 2 * kp : 2 * kp + 2],
                    )

            # SP: q/k/v weight halves then wo; ACT: x0 + mask/identity
            nc.sync.dma_start(w_sb["q"][:, 0:4], wq4[:, 0:4])
            x_tiles[0] = xs.tile([P, KT, 512], BF16, tag="x", name="x0")
            nc.scalar.dma_start(x_tiles[0][:, 0:4], xt4[0, :, 0:4])
            nc.sync.dma_start(w_sb["q"][:, 4:8], wq4[:, 4:8])
            nc.scalar.dma_start(x_tiles[0][:, 4:8], xt4[0, :, 4:8])
            for name, wsrc in (("k", wk4), ("v", wv4)):
                nc.sync.dma_start(w_sb[name][:, 0:4], wsrc[:, 0:4])
                nc.sync.dma_start(w_sb[name][:, 4:8], wsrc[:, 4:8])
            nc.scalar.dma_start(tm_sb[:], tm2[:])
            nc.scalar.dma_start(id_sb[:], ident[:])
            nc.sync.dma_start(wo_sb[:], wo4[:])

            # warm the PE while the x/w DMAs land
            warm = ps_c.tile([P, 512], F32, tag="pp", name="warm")
            for _ in range(17):
                nc.tensor.matmul(
                    warm[:, 0:256], warm_src[:, 0:P], warm_src[:],
                    start=True, stop=True, skip_group_check=True,
                )

            # ---- filler-step factories (each step = ~4 matmuls on PE) ----
            def proj_qk_steps(name, dst, st):
                ssl = slice(st * 512, (st + 1) * 512)
                w = w_sb[name]
                x_t = x_tiles[st]
                steps = []
                for mt in range(MSUB):
                    msl = slice(mt * P, (mt + 1) * P)
                    holder = {}

                    def s1(mt=mt, msl=msl, holder=holder):
                        pt = ps_c.tile([P, 512], F32, tag="pp", name="prq")
                        holder["pt"] = pt
                        for kt in range(4):
                            nc.tensor.matmul(
                                pt[:], w[:, kt, msl], x_t[:, kt],
                                start=(kt == 0), stop=False,
                            )

                    def s2(mt=mt, msl=msl, holder=holder):
                        pt = holder["pt"]
                        for kt in range(4, KT):
                            nc.tensor.matmul(
                                pt[:], w[:, kt, msl], x_t[:, kt],
                                start=False, stop=(kt == KT - 1),
                            )
                        nc.vector.tensor_copy(dst[:, mt, ssl], pt[:])

                    steps += [s1, s2]
                return steps

            def proj_v_steps(st):
                x_t = x_tiles[st]
                steps = []
                for ssub in range(4):
                    jt = st * 4 + ssub
                    s0 = ssub * P
                    holder = {}

                    def s1(jt=jt, s0=s0, holder=holder):
                        pt = ps_c.tile([P, 512], F32, tag="pp", name="prv")
                        holder["pt"] = pt
                        for kt in range(4):
                            nc.tensor.matmul(
                                pt[:], x_t[:, kt, s0 : s0 + P], w_sb["v"][:, kt],
                                start=(kt == 0), stop=False,
                            )

                    def s2(jt=jt, s0=s0, holder=holder):
                        pt = holder["pt"]
                        for kt in range(4, KT):
                            nc.tensor.matmul(
                                pt[:], x_t[:, kt, s0 : s0 + P], w_sb["v"][:, kt],
                                start=False, stop=(kt == KT - 1),
                            )
                        nc.vector.tensor_copy(
                            v_sb[:, jt, :, 0:DK],
                            pt.rearrange("p (h d) -> p h d", h=HG),
                        )

                    steps += [s1, s2]
                return steps

            def outproj_steps(ohT_prev, qt_prev):
                steps = []
                holder = {}
                for nt in range(NT):
                    def s1(nt=nt):
                        py = ps_c.tile([P, 512], F32, tag="pp", name="py")
                        for mt in range(MSUB):
                            nc.tensor.matmul(
                                py[:],
                                wo_sb[:, mt, nt * P : (nt + 1) * P],
                                ohT_prev[:, mt, :],
                                start=(mt == 0), stop=(mt == MSUB - 1),
                            )
                        if nt % 4 == 0:
                            holder["y4"] = ysp.tile(
                                [P, 4, 512], BF16, tag="y", name="y4"
                            )
                        nc.vector.tensor_copy(holder["y4"][:, nt % 4, :], py[:])
                        if nt % 4 == 3:
                            nc.gpsimd.dma_start(
                                yT4[qt_prev, :, nt - 3 : nt + 1], holder["y4"][:]
                            )

                    steps.append(s1)
                return steps

            # ---- attention primitives ----
            def emit_scores(qt, hp, jt, ctx):
                """Two K=64 matmuls: ss[keys, h, q] for the pair's heads."""
                jsl = slice(jt * P, (jt + 1) * P)
                di = jt - qt * 4
                delta = P * di if di >= 0 else 0
                qsl_d = slice(qt * 512 + delta, (qt + 1) * 512)
                ss = ps_s.tile([P, 2, 512], F32, tag="ss")
                for h in range(2):
                    hd = slice(h * DK, (h + 1) * DK)
                    nc.tensor.matmul(
                        ss[:, h, delta:],
                        kT_sb[hd, hp, jsl],
                        qT_sb[hd, hp, qsl_d],
                        start=True, stop=True, skip_group_check=True,
                    )
                ctx["ss"] = ss
                ctx["delta"] = delta
                ctx["di"] = di

            def emit_exp(ctx):
                ss, delta, di = ctx["ss"], ctx["delta"], ctx["di"]
                pp = attn.tile([P, 2, 512], BF16, tag="pp")
                nc.scalar.activation(
                    pp[:, :, delta:], ss[:, :, delta:], EXP, scale=0.125
                )
                if di >= 0:
                    # zero the upper-triangle of the diagonal 128-block
                    nc.vector.tensor_tensor(
                        pp[:, :, delta : delta + P],
                        pp[:, :, delta : delta + P],
                        tm_sb[:],
                        MULT,
                    )
                ctx["pp"] = pp

            # ---- main interleaved schedule ----
            issue_x_dma(1)
            st0_steps = (
                proj_qk_steps("q", qT_sb, 0)
                + proj_qk_steps("k", kT_sb, 0)
                + proj_v_steps(0)
            )
            # reorder [s1,s2]x12 -> s1,s1,(s2,s1)...,s2,s2: two chains
            # stay open (psum bufs=2) and the first s2 (needs the second
            # half of x0/w) starts ~2 chains later, after its DMA lands
            s1s = st0_steps[0::2]
            s2s = st0_steps[1::2]
            order = [s1s[0], s1s[1]]
            for i in range(2, len(s1s)):
                order += [s2s[i - 2], s1s[i]]
            order += [s2s[-2], s2s[-1]]
            for step in order:
                step()

            # ---- flat block stream across all q-tiles ----
            # one continuous pend-ahead pipeline (scores 2 ahead, exp 1
            # ahead) so nothing resets at q-tile boundaries
            all_units = []
            for qt in range(QT):
                for hp in range(MSUB):
                    for jt in range(4 * (qt + 1)):
                        all_units.append((qt, hp, jt))
            NU = len(all_units)
            qt_base = {}
            for wi, (qt, hp, jt) in enumerate(all_units):
                if qt not in qt_base:
                    qt_base[qt] = wi

            ohTs = []
            sweep_pv = {}
            pend_tp = []  # (ohq_t, ohT, hp, h, qs, wi_pushed)
            pend_cp = []  # (tpt, ohT, hp, h, qs, wi_emitted)
            ctxs = {}
            filler_state = {"fillers": [], "fi": 0, "nf": 0, "nfront": 0}

            # oh transpose via regular matmul against the identity:
            # out[d, j] = sum_q ohq[q, d] * I[q, j] = ohq.T.  Each
            # transpose gets its own chain-pool generation — PSUM
            # start=True zeroes lazily at bank granularity, so an
            # accumulator bank must never host two live groups.
            def do_transpose(ent, wi):
                ohq_t, ohT_e, ehp, eh, qs = ent
                tpt = ps_c.tile([P, 512], F32, tag="pp", name="tp")
                nc.tensor.matmul(
                    tpt[eh * DK : (eh + 1) * DK, 0:P],
                    ohq_t[:],
                    id_sb[:],
                    start=True, stop=True,
                    skip_group_check=True,
                )
                pend_cp.append((tpt, ohT_e, ehp, eh, qs, wi))

            def do_copy(ent):
                tpt, ohT_e, ehp, eh, qs, _ = ent
                nc.vector.tensor_copy(
                    ohT_e[eh * DK : (eh + 1) * DK, ehp, qs * P : (qs + 1) * P],
                    tpt[eh * DK : (eh + 1) * DK, 0:P],
                )

            def qt_fillers(qt):
                """Filler steps to interleave into q-tile qt's blocks."""
                fillers = []
                nfront = 0
                if qt + 1 < QT:
                    st = qt + 1
                    fillers += proj_qk_steps("q", qT_sb, st)
                    if st < QT - 1:
                        fillers += proj_qk_steps("k", kT_sb, st)
                        fillers += proj_v_steps(st)
                if qt == QT - 1:
                    # k/v(st3) deferred into qt3 (exp-bound): feeds jt>=12,
                    # so front-load it within the first blocks
                    kv3 = proj_qk_steps("k", kT_sb, 3) + proj_v_steps(3)
                    fillers = kv3 + fillers
                    nfront = len(kv3)
                # outproj runs 2 q-tiles late: the late q-tiles are
                # exp-bound and need the extra PE filler
                if qt == 2:
                    fillers += outproj_steps(ohTs[0], 0)
                elif qt == 3:
                    fillers += outproj_steps(ohTs[1], 1)
                    fillers += outproj_steps(ohTs[2], 2)
                return fillers, nfront

            # prologue for block 0/1 of qt0
            ctxs[0] = {}
            emit_scores(all_units[0][0], *all_units[0][1:], ctxs[0])
            ctxs[1] = {}
            emit_scores(all_units[1][0], *all_units[1][1:], ctxs[1])
            emit_exp(ctxs[0])

            for wi, (qt, hp, jt) in enumerate(all_units):
                if wi == qt_base[qt]:
                    # q-tile entry: flush previous fillers, set up new ones
                    while filler_state["fi"] < filler_state["nf"]:
                        filler_state["fillers"][filler_state["fi"]]()
                        filler_state["fi"] += 1
                    fillers, nfront = qt_fillers(qt)
                    filler_state = {
                        "fillers": fillers,
                        "fi": 0,
                        "nf": len(fillers),
                        "nfront": nfront,
                    }
                    if qt + 2 < QT:
                        issue_x_dma(qt + 2)
                    ohT = ohp.tile([P, MSUB, 512], BF16, tag="ohT")
                    ohTs.append(ohT)
                    nu_qt = (
                        qt_base[qt + 1] - qt_base[qt]
                        if qt + 1 < QT
                        else NU - qt_base[qt]
                    )
                ctx = ctxs.pop(wi)
                # issue exp for the NEXT block early (ACT queue ahead)
                if wi + 1 < NU:
                    emit_exp(ctxs[wi + 1])
                # epilogue pipeline: transpose ~2 blocks after the DVE
                # scale was issued, ohT copy ~1 block after the transpose
                while pend_tp and pend_tp[0][5] <= wi - 2:
                    ent = pend_tp.pop(0)
                    do_transpose(ent[:5], wi)
                while pend_cp and pend_cp[0][5] <= wi - 1:
                    do_copy(pend_cp.pop(0))
                # filler quota (local block index within this q-tile)
                bi = wi - qt_base[qt]
                nf = filler_state["nf"]
                nfront = filler_state["nfront"]
                want = (bi + 1) * nf // nu_qt
                if nfront and bi < 12:
                    want = max(want, min(nfront, (bi + 1) * nfront // 11))
                # at sweep starts the pv-tile WAR wait (previous sweep's
                # epilogue reads on DVE) stalls the PE — run a filler
                # first so the DVE drains behind real PE work
                sc_emitted = False
                if jt <= 1 and filler_state["fi"] < want:
                    filler_state["fillers"][filler_state["fi"]]()
                    filler_state["fi"] += 1
                # PV for current block (both heads)
                if jt == 0:
                    for h in range(2):
                        sweep_pv[hp, h] = ps_v.tile(
                            [P, 4, DK + 1], F32, tag="pv", name="pv"
                        )
                pp = ctx["pp"]
                di = ctx["di"]
                q0 = di if di > 0 else 0
                for h in range(2):
                    pv_t = sweep_pv[hp, h]
                    hh = hp * 2 + h
                    for qs in range(q0, 4):
                        last = jt == qt * 4 + qs
                        # start only on the bank's first group touch:
                        # PSUM start zeroes the whole bank lazily, so
                        # sibling slices rely on that single mark
                        nc.tensor.matmul(
                            pv_t[:, qs, :],
                            pp[:, h, qs * P : (qs + 1) * P],
                            v_sb[:, jt, hh, :],
                            start=(jt == 0 and qs == q0), stop=last,
                            skip_group_check=True,
                        )
                        if last:
                            # epilogue DVE: 1/z then scale into sbuf
                            zr = zrp.tile([P, 1], F32, tag="zr")
                            nc.vector.reciprocal(
                                zr[:], pv_t[:, qs, DK : DK + 1]
                            )
                            ohq_t = ohqp.tile([P, DK], BF16, tag="ohq")
                            nc.vector.tensor_scalar_mul(
                                ohq_t[:], pv_t[:, qs, 0:DK], zr[:]
                            )
                            pend_tp.append((ohq_t, ohT, hp, h, qs, wi))
                # emit scores for block wi+2 (pend-ahead depth 2)
                if wi + 2 < NU and not sc_emitted:
                    ctxs[wi + 2] = {}
                    u = all_units[wi + 2]
                    emit_scores(u[0], *u[1:], ctxs[wi + 2])
                # filler quota
                while filler_state["fi"] < want:
                    filler_state["fillers"][filler_state["fi"]]()
                    filler_state["fi"] += 1

            # drain the tail pipeline, remaining fillers interleaved
            nrem = len(pend_tp)
            for ri in range(nrem):
                ent = pend_tp.pop(0)
                if filler_state["fi"] < filler_state["nf"]:
                    filler_state["fillers"][filler_state["fi"]]()
                    filler_state["fi"] += 1
                do_transpose(ent[:5], NU + ri)
                while pend_cp and pend_cp[0][5] <= NU + ri - 1:
                    do_copy(pend_cp.pop(0))
            while filler_state["fi"] < filler_state["nf"]:
                filler_state["fillers"][filler_state["fi"]]()
                filler_state["fi"] += 1
            while pend_cp:
                do_copy(pend_cp.pop(0))

            # final output projection for qt=3: per-nt eviction + DMA on
            # alternating engines so the tail after the last matmul is
            # one small copy + one 1KB-row DMA.  Chains come from the
            # (now idle) pv pool so they don't contend with the drain's
            # transpose generations in the chain pool.
            ohT3, qt3 = ohTs[3], 3
            for nt in range(NT):
                y1f = ysp.tile([P, 512], BF16, tag="y2", name="y1f")
                py = ps_v.tile([P, 512], F32, tag="pv", name="pyf")
                for mt in range(MSUB):
                    nc.tensor.matmul(
                        py[:],
                        wo_sb[:, mt, nt * P : (nt + 1) * P],
                        ohT3[:, mt, :],
                        start=(mt == 0), stop=(mt == MSUB - 1),
                    )
                if nt % 2 == 0:
                    nc.vector.tensor_copy(y1f[:], py[:])
                    nc.sync.dma_start(yT4[qt3, :, nt, :], y1f[:])
                else:
                    nc.scalar.copy(y1f[:], py[:])
                    nc.scalar.dma_start(yT4[qt3, :, nt, :], y1f[:])

    nc.finalize()
    return nc


_CACHED_NC = None


def _get_nc() -> bass.Bass:
    global _CACHED_NC
    if _CACHED_NC is None:
        _CACHED_NC = build_nc()
    return _CACHED_NC


def _make_tm2() -> np.ndarray:
    import ml_dtypes

    k = np.arange(P)[:, None]
    j = np.arange(P)[None, :]
    tm = np.where(j >= k, 1.0, 0.0).astype(np.float32)
    return np.stack([tm, tm], axis=1).astype(ml_dtypes.bfloat16)


def _make_ident() -> np.ndarray:
    import ml_dtypes

    return np.eye(P, dtype=np.float32).astype(ml_dtypes.bfloat16)


def make_in_maps(inputs):
    import ml_dtypes

    bf = ml_dtypes.bfloat16
    x = np.asarray(inputs["x"], np.float32)
    q_heads = np.asarray(inputs["q_heads"], np.float32)
    k_heads = np.asarray(inputs["k_heads"], np.float32)
    v_heads = np.asarray(inputs["v_heads"], np.float32)
    output_proj = np.asarray(inputs["output_proj"], np.float32)

    tm = _make_tm2()
    idn = _make_ident()

    def tile_w(wT):  # [1024, 512] -> [p, kt, m]
        return np.ascontiguousarray(
            wT.reshape(KT, P, MG).transpose(1, 0, 2)
        ).astype(bf)

    in_maps = []
    for core in range(N_CORES):
        b, g = divmod(core, 2)
        gsl = slice(g * MG, (g + 1) * MG)
        xT = x[b].T  # [1024, 2048]
        xt4 = np.ascontiguousarray(
            xT.reshape(KT, P, QT, 512).transpose(2, 1, 0, 3)
        ).astype(bf)  # [st, p, kt, 512]
        wo = output_proj[:, gsl].T  # [512, 1024]
        wo4 = np.ascontiguousarray(
            wo.reshape(MSUB, P, D_MODEL).transpose(1, 0, 2)
        ).astype(bf)
        in_maps.append(
            {
                "xt4": xt4,
                "wq4": tile_w(q_heads[gsl].T),
                "wk4": tile_w(k_heads[gsl].T),
                "wv4": tile_w(v_heads[gsl].T),
                "wo4": wo4,
                "tm2": tm,
                "ident": idn,
            }
        )
    return in_maps


def kernel(x, q_heads, k_heads, v_heads, output_proj):
    inputs = {
        "x": x,
        "q_heads": q_heads,
        "k_heads": k_heads,
        "v_heads": v_heads,
        "output_proj": output_proj,
    }
    in_maps = make_in_maps(inputs)
    nc = _get_nc()
    res = run_bass_kernel_spmd(nc, in_maps, list(range(N_CORES)))
    y = np.empty((B, S, D_MODEL), np.float32)
    for b in range(B):
        acc = res.results[2 * b]["yT4"].astype(np.float32) + res.results[
            2 * b + 1
        ]["yT4"].astype(np.float32)
        yT = acc.transpose(2, 1, 0, 3).reshape(D_MODEL, S)
        y[b] = yT.T
    return y


# revision 39
# speedup vs baseline: 12003.6940x; 1.0141x over previous
"""Causal MHA on 8 trn2 cores — v3: transposed-PV schedule.

Sharding: 8 cores = 4 batches x 2 head-groups (8 heads each).

v3 changes vs v2:
- PV computed transposed: out[q, dk] = pp[keys, q].T @ v[keys, dk] —
  stationary = probs tile, moving = v (N=64) — halves PV matmul time
  (cost scales with moving free size; old orientation paid N=512 for
  M=65 useful rows).
- Z (softmax denom) via N=1 ones-column matmuls chained like PV.
- causal mask as a post-exp DVE 0/1 multiply (off the PE).
- per-(head,qsub) epilogue: reciprocal + per-partition tensor_scalar
  scale, then a PE transpose places oh back in [dk, q] for the output
  projection (tile_position lands head 1 in partitions 64:128).
- per-head scores/exp (ss [P,512] f32, 3 PSUM banks) with 2-deep
  pend-ahead so ACT stays fed through the exp-bound late q-tiles.

PSUM budget (8 banks): ss 3 + pv 2 + chain 2 + (z, tp slivers) 1.
"""

import sys

if "/opt/trn_rl_repo" not in sys.path:
    sys.path.insert(0, "/opt/trn_rl_repo")

import numpy as np

import concourse.bass as bass
import concourse.mybir as mybir
from concourse import bacc, tile
from concourse.bass_utils import run_bass_kernel_spmd

P = 128
D_MODEL = 1024
NUM_HEADS = 16
DK = 64
B, S = 4, 2048
HG = NUM_HEADS // 2
MG = HG * DK
N_CORES = 8

QT = S // 512
JT = S // P
KT = D_MODEL // P
MSUB = MG // P
NT = D_MODEL // P

F32 = mybir.dt.float32
BF16 = mybir.dt.bfloat16
EXP = mybir.ActivationFunctionType.Exp
MULT = mybir.AluOpType.mult


def build_nc() -> bass.Bass:
    nc = bacc.Bacc("TRN2", target_bir_lowering=False, debug=False)

    # inputs pre-tiled host-side to partition-major layout so every DMA
    # partition-row is 8KB contiguous
    xt4 = nc.dram_tensor("xt4", [QT, P, KT, 512], BF16, kind="ExternalInput")
    wq4 = nc.dram_tensor("wq4", [P, KT, MG], BF16, kind="ExternalInput")
    wk4 = nc.dram_tensor("wk4", [P, KT, MG], BF16, kind="ExternalInput")
    wv4 = nc.dram_tensor("wv4", [P, KT, MG], BF16, kind="ExternalInput")
    wo4 = nc.dram_tensor("wo4", [P, MSUB, D_MODEL], BF16, kind="ExternalInput")
    tm2 = nc.dram_tensor("tm2", [P, 2, P], BF16, kind="ExternalInput")
    ident = nc.dram_tensor("ident", [P, P], BF16, kind="ExternalInput")
    # output y^T tiled [qt, p, nt, 512]
    yT4 = nc.dram_tensor("yT4", [QT, P, NT, 512], BF16, kind="ExternalOutput")

    with tile.TileContext(nc) as tc:
        with (
            tc.tile_pool(name="wpool", bufs=1) as wpool,
            tc.tile_pool(name="qkv", bufs=1) as qkv,
            tc.tile_pool(name="xs", bufs=2) as xs,
            tc.tile_pool(name="oh", bufs=3) as ohp,
            tc.tile_pool(name="ys", bufs=4) as ysp,
            tc.tile_pool(name="attn", bufs=4) as attn,
            tc.tile_pool(name="attnc", bufs=1) as attnc,
            tc.tile_pool(name="ohq", bufs=6) as ohqp,
            tc.tile_pool(name="zr", bufs=6) as zrp,
            # PSUM pools — creation order fixes bank packing:
            # ss 3 banks, pv 2, chains 2, z+tp slivers in bank 8
            tc.tile_pool(name="ps_s", bufs=2, space="PSUM") as ps_s,
            tc.tile_pool(name="ps_v", bufs=2, space="PSUM") as ps_v,
            tc.tile_pool(name="ps_c", bufs=2, space="PSUM") as ps_c,
        ):
            # ---- persistent sbuf ----
            w_sb = {}
            for name in ("q", "k", "v"):
                w_sb[name] = wpool.tile(
                    [P, KT, MG], BF16, tag=f"w{name}", name=f"w{name}"
                )
            wo_sb = wpool.tile([P, MSUB, D_MODEL], BF16, tag="wo")
            qT_sb = qkv.tile([P, MSUB, S], BF16, tag="qT")
            kT_sb = qkv.tile([P, MSUB, S], BF16, tag="kT")
            v_sb = qkv.tile([P, JT, HG, DK + 1], BF16, tag="v")

            tm_sb = attnc.tile([P, 2, P], BF16, tag="tm")
            id_sb = attnc.tile([P, P], BF16, tag="id")

            # ---- input DMA ----
            # issue cost is ~565-667ns per dma_start, serial per engine —
            # split the startup DMAs across the two HWDGE engines (SP +
            # Activation; ACT is idle until the first exp) so the first
            # projection's dependencies land by ~2.5us
            warm_src = attnc.tile([P, 256], BF16, tag="warm_src")
            nc.vector.memset(warm_src[:], 0.5)
            nc.gpsimd.memset(v_sb[:, :, :, DK : DK + 1], 1.0)
            x_tiles = [None] * QT

            def issue_x_dma(st):
                x_tiles[st] = xs.tile([P, KT, 512], BF16, tag="x", name=f"x{st}")
                for kp in range(4):
                    nc.sync.dma_start(
                        x_tiles[st][:, 2 * kp : 2 * kp + 2],
                        xt4[st, :, 2 * kp : 2 * kp + 2],
                    )

            # SP: q/k/v weight halves then wo; ACT: x0 + mask/identity
            nc.sync.dma_start(w_sb["q"][:, 0:4], wq4[:, 0:4])
            x_tiles[0] = xs.tile([P, KT, 512], BF16, tag="x", name="x0")
            nc.scalar.dma_start(x_tiles[0][:, 0:4], xt4[0, :, 0:4])
            nc.sync.dma_start(w_sb["q"][:, 4:8], wq4[:, 4:8])
            nc.scalar.dma_start(x_tiles[0][:, 4:8], xt4[0, :, 4:8])
            for name, wsrc in (("k", wk4), ("v", wv4)):
                nc.sync.dma_start(w_sb[name][:, 0:4], wsrc[:, 0:4])
                nc.sync.dma_start(w_sb[name][:, 4:8], wsrc[:, 4:8])
            nc.scalar.dma_start(tm_sb[:], tm2[:])
            nc.scalar.dma_start(id_sb[:], ident[:])
            nc.sync.dma_start(wo_sb[:], wo4[:])

            # warm the PE while the x/w DMAs land
            warm = ps_c.tile([P, 512], F32, tag="pp", name="warm")
            for _ in range(17):
                nc.tensor.matmul(
                    warm[:, 0:256], warm_src[:, 0:P], warm_src[:],
                    start=True, stop=True, skip_group_check=True,
                )

            # ---- filler-step factories (each step = ~4 matmuls on PE) ----
            def proj_qk_steps(name, dst, st):
                ssl = slice(st * 512, (st + 1) * 512)
                w = w_sb[name]
                x_t = x_tiles[st]
                steps = []
                for mt in range(MSUB):
                    msl = slice(mt * P, (mt + 1) * P)
                    holder = {}

                    def sk(k0, k1, mt=mt, msl=msl, holder=holder):
                        if k0 == 0:
                            holder["pt"] = ps_c.tile(
                                [P, 512], F32, tag="pp", name="prq"
                            )
                        pt = holder["pt"]
                        for kt in range(k0, k1):
                            nc.tensor.matmul(
                                pt[:], w[:, kt, msl], x_t[:, kt],
                                start=(kt == 0), stop=(kt == KT - 1),
                            )
                        if k1 == KT:
                            nc.vector.tensor_copy(dst[:, mt, ssl], pt[:])

                    # 2-matmul sub-steps: finer quota placement absorbs
                    # sub-500ns PE stalls
                    for k0 in range(0, KT, 2):
                        steps.append(
                            lambda k0=k0, sk=sk: sk(k0, k0 + 2)
                        )
                return steps

            def proj_v_steps(st):
                x_t = x_tiles[st]
                steps = []
                for ssub in range(4):
                    jt = st * 4 + ssub
                    s0 = ssub * P
                    holder = {}

                    def sk(k0, k1, jt=jt, s0=s0, holder=holder):
                        if k0 == 0:
                            holder["pt"] = ps_c.tile(
                                [P, 512], F32, tag="pp", name="prv"
                            )
                        pt = holder["pt"]
                        for kt in range(k0, k1):
                            nc.tensor.matmul(
                                pt[:], x_t[:, kt, s0 : s0 + P], w_sb["v"][:, kt],
                                start=(kt == 0), stop=(kt == KT - 1),
                            )
                        if k1 == KT:
                            nc.vector.tensor_copy(
                                v_sb[:, jt, :, 0:DK],
                                pt.rearrange("p (h d) -> p h d", h=HG),
                            )

                    for k0 in range(0, KT, 2):
                        steps.append(
                            lambda k0=k0, sk=sk: sk(k0, k0 + 2)
                        )
                return steps

            def outproj_steps(ohT_prev, qt_prev):
                steps = []
                holder = {}
                for nt in range(NT):
                    def sm(m0, m1, nt=nt):
                        if m0 == 0:
                            holder["py"] = ps_c.tile(
                                [P, 512], F32, tag="pp", name="py"
                            )
                        py = holder["py"]
                        for mt in range(m0, m1):
                            nc.tensor.matmul(
                                py[:],
                                wo_sb[:, mt, nt * P : (nt + 1) * P],
                                ohT_prev[:, mt, :],
                                start=(mt == 0), stop=(mt == MSUB - 1),
                            )
                        if m1 < MSUB:
                            return
                        if nt % 4 == 0:
                            holder["y4"] = ysp.tile(
                                [P, 4, 512], BF16, tag="y", name="y4"
                            )
                        nc.vector.tensor_copy(holder["y4"][:, nt % 4, :], py[:])
                        if nt % 4 == 3:
                            nc.gpsimd.dma_start(
                                yT4[qt_prev, :, nt - 3 : nt + 1], holder["y4"][:]
                            )

                    steps.append(lambda sm=sm: sm(0, 2))
                    steps.append(lambda sm=sm: sm(2, 4))
                return steps

            # ---- attention primitives ----
            def emit_scores(qt, hp, jt, ctx):
                """Two K=64 matmuls: ss[keys, h, q] for the pair's heads."""
                jsl = slice(jt * P, (jt + 1) * P)
                di = jt - qt * 4
                delta = P * di if di >= 0 else 0
                qsl_d = slice(qt * 512 + delta, (qt + 1) * 512)
                ss = ps_s.tile([P, 2, 512], F32, tag="ss")
                for h in range(2):
                    hd = slice(h * DK, (h + 1) * DK)
                    nc.tensor.matmul(
                        ss[:, h, delta:],
                        kT_sb[hd, hp, jsl],
                        qT_sb[hd, hp, qsl_d],
                        start=True, stop=True, skip_group_check=True,
                    )
                ctx["ss"] = ss
                ctx["delta"] = delta
                ctx["di"] = di

            def emit_exp(ctx):
                ss, delta, di = ctx["ss"], ctx["delta"], ctx["di"]
                pp = attn.tile([P, 2, 512], BF16, tag="pp")
                nc.scalar.activation(
                    pp[:, :, delta:], ss[:, :, delta:], EXP, scale=0.125
                )
                if di >= 0:
                    # zero the upper-triangle of the diagonal 128-block
                    nc.vector.tensor_tensor(
                        pp[:, :, delta : delta + P],
                        pp[:, :, delta : delta + P],
                        tm_sb[:],
                        MULT,
                    )
                ctx["pp"] = pp

            # ---- main interleaved schedule ----
            issue_x_dma(1)
            st0_steps = (
                proj_qk_steps("q", qT_sb, 0)
                + proj_qk_steps("k", kT_sb, 0)
                + proj_v_steps(0)
            )
            # reorder [s1,s2]x12 -> s1,s1,(s2,s1)...,s2,s2: two chains
            # stay open (psum bufs=2) and the first s2 (needs the second
            # half of x0/w) starts ~2 chains later, after its DMA lands
            s1s = st0_steps[0::2]
            s2s = st0_steps[1::2]
            order = [s1s[0], s1s[1]]
            for i in range(2, len(s1s)):
                order += [s2s[i - 2], s1s[i]]
            order += [s2s[-2], s2s[-1]]
            for step in order:
                step()

            # ---- flat block stream across all q-tiles ----
            # one continuous pend-ahead pipeline (scores 2 ahead, exp 1
            # ahead) so nothing resets at q-tile boundaries
            all_units = []
            for qt in range(QT):
                for hp in range(MSUB):
                    for jt in range(4 * (qt + 1)):
                        all_units.append((qt, hp, jt))
            NU = len(all_units)
            qt_base = {}
            for wi, (qt, hp, jt) in enumerate(all_units):
                if qt not in qt_base:
                    qt_base[qt] = wi

            ohTs = []
            sweep_pv = {}
            pend_tp = []  # (ohq_t, ohT, hp, h, qs, wi_pushed)
            pend_cp = []  # (tpt, ohT, hp, h, qs, wi_emitted)
            pend_pv = []  # sweep-start pv emissions deferred one block
            ctxs = {}
            filler_state = {"fillers": [], "fi": 0, "nf": 0, "nfront": 0}

            # oh transpose via regular matmul against the identity:
            # out[d, j] = sum_q ohq[q, d] * I[q, j] = ohq.T.  Each
            # transpose gets its own chain-pool generation — PSUM
            # start=True zeroes lazily at bank granularity, so an
            # accumulator bank must never host two live groups.
            def do_transpose(ent, wi):
                ohq_t, ohT_e, ehp, eh, qs = ent
                tpt = ps_c.tile([P, 512], F32, tag="pp", name="tp")
                nc.tensor.matmul(
                    tpt[eh * DK : (eh + 1) * DK, 0:P],
                    ohq_t[:],
                    id_sb[:],
                    start=True, stop=True,
                    skip_group_check=True,
                )
                pend_cp.append((tpt, ohT_e, ehp, eh, qs, wi))

            def do_copy(ent):
                tpt, ohT_e, ehp, eh, qs, _ = ent
                nc.vector.tensor_copy(
                    ohT_e[eh * DK : (eh + 1) * DK, ehp, qs * P : (qs + 1) * P],
                    tpt[eh * DK : (eh + 1) * DK, 0:P],
                )

            def qt_fillers(qt):
                """Filler steps to interleave into q-tile qt's blocks."""
                fillers = []
                nfront = 0
                if qt + 1 < QT:
                    st = qt + 1
                    fillers += proj_qk_steps("q", qT_sb, st)
                    if st < QT - 1:
                        fillers += proj_qk_steps("k", kT_sb, st)
                        fillers += proj_v_steps(st)
                if qt == QT - 1:
                    # k/v(st3) deferred into qt3 (exp-bound): feeds jt>=12,
                    # so front-load it within the first blocks
                    kv3 = proj_qk_steps("k", kT_sb, 3) + proj_v_steps(3)
                    fillers = kv3 + fillers
                    nfront = len(kv3)
                # outproj runs 2 q-tiles late: the late q-tiles are
                # exp-bound and need the extra PE filler
                if qt == 2:
                    fillers += outproj_steps(ohTs[0], 0)
                elif qt == 3:
                    fillers += outproj_steps(ohTs[1], 1)
                    fillers += outproj_steps(ohTs[2], 2)
                return fillers, nfront

            # prologue for block 0/1 of qt0
            ctxs[0] = {}
            emit_scores(all_units[0][0], *all_units[0][1:], ctxs[0])
            emit_exp(ctxs[0])
            ctxs[1] = {}
            emit_scores(all_units[1][0], *all_units[1][1:], ctxs[1])

            for wi, (qt, hp, jt) in enumerate(all_units):
                if wi == qt_base[qt]:
                    # q-tile entry: flush previous fillers, set up new ones
                    while filler_state["fi"] < filler_state["nf"]:
                        filler_state["fillers"][filler_state["fi"]]()
                        filler_state["fi"] += 1
                    fillers, nfront = qt_fillers(qt)
                    filler_state = {
                        "fillers": fillers,
                        "fi": 0,
                        "nf": len(fillers),
                        "nfront": nfront,
                    }
                    if qt + 2 < QT:
                        issue_x_dma(qt + 2)
                    ohT = ohp.tile([P, MSUB, 512], BF16, tag="ohT")
                    ohTs.append(ohT)
                    nu_qt = (
                        qt_base[qt + 1] - qt_base[qt]
                        if qt + 1 < QT
                        else NU - qt_base[qt]
                    )
                ctx = ctxs.pop(wi)
                # issue exp for the NEXT block early (ACT queue ahead)
                if wi + 1 < NU and "pp" not in ctxs.get(wi + 1, {"pp": 1}):
                    emit_exp(ctxs[wi + 1])
                # epilogue pipeline: transpose ~2 blocks after the DVE
                # scale was issued, ohT copy ~1 block after the transpose
                while pend_tp and pend_tp[0][5] <= wi - 2:
                    ent = pend_tp.pop(0)
                    do_transpose(ent[:5], wi)
                while pend_cp and pend_cp[0][5] <= wi - 1:
                    do_copy(pend_cp.pop(0))
                # filler quota (local block index within this q-tile)
                bi = wi - qt_base[qt]
                nf = filler_state["nf"]
                nfront = filler_state["nfront"]
                want = (bi + 1) * nf // nu_qt
                if nfront and bi < 12:
                    want = max(want, min(nfront, (bi + 1) * nfront // 11))
                # at sweep starts the pv-tile WAR wait (previous sweep's
                # epilogue reads on DVE) stalls the PE — run a filler
                # first so the DVE drains behind real PE work
                sc_emitted = False
                if jt <= 1 and filler_state["fi"] < want:
                    filler_state["fillers"][filler_state["fi"]]()
                    filler_state["fi"] += 1
                # PV for current block (both heads)
                def emit_pv(qt, hp, jt, pp, di, ohT_e, wi):
                    q0 = di if di > 0 else 0
                    for h in range(2):
                        pv_t = sweep_pv[hp, h]
                        hh = hp * 2 + h
                        for qs in range(q0, 4):
                            last = jt == qt * 4 + qs
                            # start only on the bank's first group touch:
                            # PSUM start zeroes the whole bank lazily, so
                            # sibling slices rely on that single mark
                            nc.tensor.matmul(
                                pv_t[:, qs, :],
                                pp[:, h, qs * P : (qs + 1) * P],
                                v_sb[:, jt, hh, :],
                                start=(jt == 0 and qs == q0), stop=last,
                                skip_group_check=True,
                            )
                            if last:
                                # epilogue DVE: 1/z then scale into sbuf
                                zr = zrp.tile([P, 1], F32, tag="zr")
                                nc.vector.reciprocal(
                                    zr[:], pv_t[:, qs, DK : DK + 1]
                                )
                                ohq_t = ohqp.tile([P, DK], BF16, tag="ohq")
                                nc.vector.tensor_scalar_mul(
                                    ohq_t[:], pv_t[:, qs, 0:DK], zr[:]
                                )
                                pend_tp.append((ohq_t, ohT_e, hp, h, qs, wi))

                pp = ctx["pp"]
                di = ctx["di"]
                if jt == 0:
                    # defer the new sweep's first pv matmuls one block:
                    # the old generation's epilogue reads (DVE) get a full
                    # block to clear the pv-slot WAR before the start
                    for h in range(2):
                        sweep_pv[hp, h] = ps_v.tile(
                            [P, 4, DK + 1], F32, tag="pv", name="pv"
                        )
                    pend_pv.append((qt, hp, jt, pp, di, ohT, wi))
                else:
                    while pend_pv:
                        emit_pv(*pend_pv.pop(0))
                    emit_pv(qt, hp, jt, pp, di, ohT, wi)
                # emit scores for block wi+2 (pend-ahead depth 2)
                if wi + 2 < NU and not sc_emitted:
                    ctxs[wi + 2] = {}
                    u = all_units[wi + 2]
                    emit_scores(u[0], *u[1:], ctxs[wi + 2])
                # filler quota
                while filler_state["fi"] < want:
                    filler_state["fillers"][filler_state["fi"]]()
                    filler_state["fi"] += 1

            # pre-open the first two final outproj chains (mt 0..2 only
            # — those ohT pieces are long done) so the drain's DVE/PE
            # latency hides behind them
            ohT3 = ohTs[3]
            fin_pys = []
            for nt in range(2):
                py = ps_v.tile([P, 512], F32, tag="pv", name="pyf0")
                for mt in range(MSUB - 1):
                    nc.tensor.matmul(
                        py[:],
                        wo_sb[:, mt, nt * P : (nt + 1) * P],
                        ohT3[:, mt, :],
                        start=(mt == 0), stop=False,
                    )
                fin_pys.append(py)

            # drain the tail pipeline, remaining fillers interleaved
            nrem = len(pend_tp)
            for ri in range(nrem):
                ent = pend_tp.pop(0)
                if filler_state["fi"] < filler_state["nf"]:
                    filler_state["fillers"][filler_state["fi"]]()
                    filler_state["fi"] += 1
                do_transpose(ent[:5], NU + ri)
                while pend_cp and pend_cp[0][5] <= NU + ri - 1:
                    do_copy(pend_cp.pop(0))
            while filler_state["fi"] < filler_state["nf"]:
                filler_state["fillers"][filler_state["fi"]]()
                filler_state["fi"] += 1
            while pend_cp:
                do_copy(pend_cp.pop(0))

            # final output projection for qt=3: per-nt eviction + DMA on
            # alternating engines so the tail after the last matmul is
            # one small copy + one 1KB-row DMA.  Chains come from the
            # (now idle) pv pool so they don't contend with the drain's
            # transpose generations in the chain pool.
            qt3 = 3
            for nt in range(NT):
                y1f = ysp.tile([P, 512], BF16, tag="y2", name="y1f")
                if nt < 2:
                    py = fin_pys[nt]
                    nc.tensor.matmul(
                        py[:],
                        wo_sb[:, MSUB - 1, nt * P : (nt + 1) * P],
                        ohT3[:, MSUB - 1, :],
                        start=False, stop=True,
                    )
                else:
                    py = ps_v.tile([P, 512], F32, tag="pv", name="pyf")
                    for mt in range(MSUB):
                        nc.tensor.matmul(
                            py[:],
                            wo_sb[:, mt, nt * P : (nt + 1) * P],
                            ohT3[:, mt, :],
                            start=(mt == 0), stop=(mt == MSUB - 1),
                        )
                if nt % 2 == 0:
                    nc.vector.tensor_copy(y1f[:], py[:])
                    nc.sync.dma_start(yT4[qt3, :, nt, :], y1f[:])
                else:
                    nc.scalar.copy(y1f[:], py[:])
                    nc.scalar.dma_start(yT4[qt3, :, nt, :], y1f[:])

    nc.finalize()
    return nc


_CACHED_NC = None


def _get_nc() -> bass.Bass:
    global _CACHED_NC
    if _CACHED_NC is None:
        _CACHED_NC = build_nc()
    return _CACHED_NC


def _make_tm2() -> np.ndarray:
    import ml_dtypes

    k = np.arange(P)[:, None]
    j = np.arange(P)[None, :]
    tm = np.where(j >= k, 1.0, 0.0).astype(np.float32)
    return np.stack([tm, tm], axis=1).astype(ml_dtypes.bfloat16)


def _make_ident() -> np.ndarray:
    import ml_dtypes

    return np.eye(P, dtype=np.float32).astype(ml_dtypes.bfloat16)


def make_in_maps(inputs):
    import ml_dtypes

    bf = ml_dtypes.bfloat16
    x = np.asarray(inputs["x"], np.float32)
    q_heads = np.asarray(inputs["q_heads"], np.float32)
    k_heads = np.asarray(inputs["k_heads"], np.float32)
    v_heads = np.asarray(inputs["v_heads"], np.float32)
    output_proj = np.asarray(inputs["output_proj"], np.float32)

    tm = _make_tm2()
    idn = _make_ident()

    def tile_w(wT):  # [1024, 512] -> [p, kt, m]
        return np.ascontiguousarray(
            wT.reshape(KT, P, MG).transpose(1, 0, 2)
        ).astype(bf)

    in_maps = []
    for core in range(N_CORES):
        b, g = divmod(core, 2)
        gsl = slice(g * MG, (g + 1) * MG)
        xT = x[b].T  # [1024, 2048]
        xt4 = np.ascontiguousarray(
            xT.reshape(KT, P, QT, 512).transpose(2, 1, 0, 3)
        ).astype(bf)  # [st, p, kt, 512]
        wo = output_proj[:, gsl].T  # [512, 1024]
        wo4 = np.ascontiguousarray(
            wo.reshape(MSUB, P, D_MODEL).transpose(1, 0, 2)
        ).astype(bf)
        in_maps.append(
            {
                "xt4": xt4,
                "wq4": tile_w(q_heads[gsl].T),
                "wk4": tile_w(k_heads[gsl].T),
                "wv4": tile_w(v_heads[gsl].T),
                "wo4": wo4,
                "tm2": tm,
                "ident": idn,
            }
        )
    return in_maps


def kernel(x, q_heads, k_heads, v_heads, output_proj):
    inputs = {
        "x": x,
        "q_heads": q_heads,
        "k_heads": k_heads,
        "v_heads": v_heads,
        "output_proj": output_proj,
    }
    in_maps = make_in_maps(inputs)
    nc = _get_nc()
    res = run_bass_kernel_spmd(nc, in_maps, list(range(N_CORES)))
    y = np.empty((B, S, D_MODEL), np.float32)
    for b in range(B):
        acc = res.results[2 * b]["yT4"].astype(np.float32) + res.results[
            2 * b + 1
        ]["yT4"].astype(np.float32)
        yT = acc.transpose(2, 1, 0, 3).reshape(D_MODEL, S)
        y[b] = yT.T
    return y


# revision 40
# speedup vs baseline: 12030.9152x; 1.0023x over previous
"""Causal MHA on 8 trn2 cores — v3: transposed-PV schedule.

Sharding: 8 cores = 4 batches x 2 head-groups (8 heads each).

v3 changes vs v2:
- PV computed transposed: out[q, dk] = pp[keys, q].T @ v[keys, dk] —
  stationary = probs tile, moving = v (N=64) — halves PV matmul time
  (cost scales with moving free size; old orientation paid N=512 for
  M=65 useful rows).
- Z (softmax denom) via N=1 ones-column matmuls chained like PV.
- causal mask as a post-exp DVE 0/1 multiply (off the PE).
- per-(head,qsub) epilogue: reciprocal + per-partition tensor_scalar
  scale, then a PE transpose places oh back in [dk, q] for the output
  projection (tile_position lands head 1 in partitions 64:128).
- per-head scores/exp (ss [P,512] f32, 3 PSUM banks) with 2-deep
  pend-ahead so ACT stays fed through the exp-bound late q-tiles.

PSUM budget (8 banks): ss 3 + pv 2 + chain 2 + (z, tp slivers) 1.
"""

import sys

if "/opt/trn_rl_repo" not in sys.path:
    sys.path.insert(0, "/opt/trn_rl_repo")

import numpy as np

import concourse.bass as bass
import concourse.mybir as mybir
from concourse import bacc, tile
from concourse.bass_utils import run_bass_kernel_spmd

P = 128
D_MODEL = 1024
NUM_HEADS = 16
DK = 64
B, S = 4, 2048
HG = NUM_HEADS // 2
MG = HG * DK
N_CORES = 8

QT = S // 512
JT = S // P
KT = D_MODEL // P
MSUB = MG // P
NT = D_MODEL // P

F32 = mybir.dt.float32
BF16 = mybir.dt.bfloat16
EXP = mybir.ActivationFunctionType.Exp
MULT = mybir.AluOpType.mult


def build_nc() -> bass.Bass:
    nc = bacc.Bacc("TRN2", target_bir_lowering=False, debug=False)

    # inputs pre-tiled host-side to partition-major layout so every DMA
    # partition-row is 8KB contiguous
    xt4 = nc.dram_tensor("xt4", [QT, P, KT, 512], BF16, kind="ExternalInput")
    wq4 = nc.dram_tensor("wq4", [P, KT, MG], BF16, kind="ExternalInput")
    wk4 = nc.dram_tensor("wk4", [P, KT, MG], BF16, kind="ExternalInput")
    wv4 = nc.dram_tensor("wv4", [P, KT, MG], BF16, kind="ExternalInput")
    wo4 = nc.dram_tensor("wo4", [P, MSUB, D_MODEL], BF16, kind="ExternalInput")
    tm2 = nc.dram_tensor("tm2", [P, 2, P], BF16, kind="ExternalInput")
    ident = nc.dram_tensor("ident", [P, P], BF16, kind="ExternalInput")
    # output y^T tiled [qt, p, nt, 512]
    yT4 = nc.dram_tensor("yT4", [QT, P, NT, 512], BF16, kind="ExternalOutput")

    with tile.TileContext(nc) as tc:
        with (
            tc.tile_pool(name="wpool", bufs=1) as wpool,
            tc.tile_pool(name="qkv", bufs=1) as qkv,
            tc.tile_pool(name="xs", bufs=2) as xs,
            tc.tile_pool(name="oh", bufs=3) as ohp,
            tc.tile_pool(name="ys", bufs=4) as ysp,
            tc.tile_pool(name="attn", bufs=4) as attn,
            tc.tile_pool(name="attnc", bufs=1) as attnc,
            tc.tile_pool(name="ohq", bufs=6) as ohqp,
            tc.tile_pool(name="zr", bufs=6) as zrp,
            # PSUM pools — creation order fixes bank packing:
            # ss 3 banks, pv 2, chains 2, z+tp slivers in bank 8
            tc.tile_pool(name="ps_s", bufs=2, space="PSUM") as ps_s,
            tc.tile_pool(name="ps_v", bufs=2, space="PSUM") as ps_v,
            tc.tile_pool(name="ps_c", bufs=2, space="PSUM") as ps_c,
        ):
            # ---- persistent sbuf ----
            w_sb = {}
            for name in ("q", "k", "v"):
                w_sb[name] = wpool.tile(
                    [P, KT, MG], BF16, tag=f"w{name}", name=f"w{name}"
                )
            wo_sb = wpool.tile([P, MSUB, D_MODEL], BF16, tag="wo")
            qT_sb = qkv.tile([P, MSUB, S], BF16, tag="qT")
            kT_sb = qkv.tile([P, MSUB, S], BF16, tag="kT")
            v_sb = qkv.tile([P, JT, HG, DK + 1], BF16, tag="v")

            tm_sb = attnc.tile([P, 2, P], BF16, tag="tm")
            id_sb = attnc.tile([P, P], BF16, tag="id")

            # ---- input DMA ----
            # issue cost is ~565-667ns per dma_start, serial per engine —
            # split the startup DMAs across the two HWDGE engines (SP +
            # Activation; ACT is idle until the first exp) so the first
            # projection's dependencies land by ~2.5us
            warm_src = attnc.tile([P, 256], BF16, tag="warm_src")
            nc.vector.memset(warm_src[:], 0.5)
            nc.gpsimd.memset(v_sb[:, :, :, DK : DK + 1], 1.0)
            x_tiles = [None] * QT

            def issue_x_dma(st):
                x_tiles[st] = xs.tile([P, KT, 512], BF16, tag="x", name=f"x{st}")
                for kp in range(4):
                    nc.sync.dma_start(
                        x_tiles[st][:, 2 * kp : 2 * kp + 2],
                        xt4[st, :, 2 * kp : 2 * kp + 2],
                    )

            # SP: q/k/v weight halves then wo; ACT: x0 + mask/identity
            nc.sync.dma_start(w_sb["q"][:, 0:4], wq4[:, 0:4])
            x_tiles[0] = xs.tile([P, KT, 512], BF16, tag="x", name="x0")
            nc.scalar.dma_start(x_tiles[0][:, 0:4], xt4[0, :, 0:4])
            nc.sync.dma_start(w_sb["q"][:, 4:8], wq4[:, 4:8])
            nc.scalar.dma_start(x_tiles[0][:, 4:8], xt4[0, :, 4:8])
            for name, wsrc in (("k", wk4), ("v", wv4)):
                nc.sync.dma_start(w_sb[name][:, 0:4], wsrc[:, 0:4])
                nc.sync.dma_start(w_sb[name][:, 4:8], wsrc[:, 4:8])
            nc.scalar.dma_start(tm_sb[:], tm2[:])
            nc.scalar.dma_start(id_sb[:], ident[:])
            nc.sync.dma_start(wo_sb[:], wo4[:])

            # warm the PE while the x/w DMAs land
            warm = ps_c.tile([P, 512], F32, tag="pp", name="warm")
            for _ in range(14):
                nc.tensor.matmul(
                    warm[:, 0:256], warm_src[:, 0:P], warm_src[:],
                    start=True, stop=True, skip_group_check=True,
                )

            # ---- filler-step factories (each step = ~4 matmuls on PE) ----
            def proj_qk_steps(name, dst, st):
                ssl = slice(st * 512, (st + 1) * 512)
                w = w_sb[name]
                x_t = x_tiles[st]
                steps = []
                for mt in range(MSUB):
                    msl = slice(mt * P, (mt + 1) * P)
                    holder = {}

                    def sk(k0, k1, mt=mt, msl=msl, holder=holder):
                        if k0 == 0:
                            holder["pt"] = ps_c.tile(
                                [P, 512], F32, tag="pp", name="prq"
                            )
                        pt = holder["pt"]
                        for kt in range(k0, k1):
                            nc.tensor.matmul(
                                pt[:], w[:, kt, msl], x_t[:, kt],
                                start=(kt == 0), stop=(kt == KT - 1),
                            )
                        if k1 == KT:
                            nc.vector.tensor_copy(dst[:, mt, ssl], pt[:])

                    # 2-matmul sub-steps: finer quota placement absorbs
                    # sub-500ns PE stalls
                    for k0 in range(0, KT, 2):
                        steps.append(
                            lambda k0=k0, sk=sk: sk(k0, k0 + 2)
                        )
                return steps

            def proj_v_steps(st):
                x_t = x_tiles[st]
                steps = []
                for ssub in range(4):
                    jt = st * 4 + ssub
                    s0 = ssub * P
                    holder = {}

                    def sk(k0, k1, jt=jt, s0=s0, holder=holder):
                        if k0 == 0:
                            holder["pt"] = ps_c.tile(
                                [P, 512], F32, tag="pp", name="prv"
                            )
                        pt = holder["pt"]
                        for kt in range(k0, k1):
                            nc.tensor.matmul(
                                pt[:], x_t[:, kt, s0 : s0 + P], w_sb["v"][:, kt],
                                start=(kt == 0), stop=(kt == KT - 1),
                            )
                        if k1 == KT:
                            nc.vector.tensor_copy(
                                v_sb[:, jt, :, 0:DK],
                                pt.rearrange("p (h d) -> p h d", h=HG),
                            )

                    for k0 in range(0, KT, 2):
                        steps.append(
                            lambda k0=k0, sk=sk: sk(k0, k0 + 2)
                        )
                return steps

            def outproj_steps(ohT_prev, qt_prev):
                steps = []
                holder = {}
                for nt in range(NT):
                    def sm(m0, m1, nt=nt):
                        if m0 == 0:
                            holder["py"] = ps_c.tile(
                                [P, 512], F32, tag="pp", name="py"
                            )
                        py = holder["py"]
                        for mt in range(m0, m1):
                            nc.tensor.matmul(
                                py[:],
                                wo_sb[:, mt, nt * P : (nt + 1) * P],
                                ohT_prev[:, mt, :],
                                start=(mt == 0), stop=(mt == MSUB - 1),
                            )
                        if m1 < MSUB:
                            return
                        if nt % 4 == 0:
                            holder["y4"] = ysp.tile(
                                [P, 4, 512], BF16, tag="y", name="y4"
                            )
                        nc.vector.tensor_copy(holder["y4"][:, nt % 4, :], py[:])
                        if nt % 4 == 3:
                            nc.gpsimd.dma_start(
                                yT4[qt_prev, :, nt - 3 : nt + 1], holder["y4"][:]
                            )

                    steps.append(lambda sm=sm: sm(0, 2))
                    steps.append(lambda sm=sm: sm(2, 4))
                return steps

            # ---- attention primitives ----
            def emit_scores(qt, hp, jt, ctx):
                """Two K=64 matmuls: ss[keys, h, q] for the pair's heads."""
                jsl = slice(jt * P, (jt + 1) * P)
                di = jt - qt * 4
                delta = P * di if di >= 0 else 0
                qsl_d = slice(qt * 512 + delta, (qt + 1) * 512)
                ss = ps_s.tile([P, 2, 512], F32, tag="ss")
                for h in range(2):
                    hd = slice(h * DK, (h + 1) * DK)
                    nc.tensor.matmul(
                        ss[:, h, delta:],
                        kT_sb[hd, hp, jsl],
                        qT_sb[hd, hp, qsl_d],
                        start=True, stop=True, skip_group_check=True,
                    )
                ctx["ss"] = ss
                ctx["delta"] = delta
                ctx["di"] = di

            def emit_exp(ctx):
                ss, delta, di = ctx["ss"], ctx["delta"], ctx["di"]
                pp = attn.tile([P, 2, 512], BF16, tag="pp")
                nc.scalar.activation(
                    pp[:, :, delta:], ss[:, :, delta:], EXP, scale=0.125
                )
                if di >= 0:
                    # zero the upper-triangle of the diagonal 128-block
                    nc.vector.tensor_tensor(
                        pp[:, :, delta : delta + P],
                        pp[:, :, delta : delta + P],
                        tm_sb[:],
                        MULT,
                    )
                ctx["pp"] = pp

            # ---- main interleaved schedule ----
            issue_x_dma(1)
            st0_steps = (
                proj_qk_steps("q", qT_sb, 0)
                + proj_qk_steps("k", kT_sb, 0)
                + proj_v_steps(0)
            )
            # weave chain pairs: both chains' kt0-3 chunks first (their
            # DMA half lands first), then kt4-7 — two chains open max
            # (psum bufs=2), and the kt4+ work starts after the second
            # DMA half has landed
            order = []
            nch = len(st0_steps) // 4
            for c0 in range(0, nch, 2):
                a, b = 4 * c0, 4 * (c0 + 1)
                order += [st0_steps[a], st0_steps[a + 1]]
                if c0 + 1 < nch:
                    order += [st0_steps[b], st0_steps[b + 1]]
                order += [st0_steps[a + 2], st0_steps[a + 3]]
                if c0 + 1 < nch:
                    order += [st0_steps[b + 2], st0_steps[b + 3]]
            for step in order:
                step()

            # ---- flat block stream across all q-tiles ----
            # one continuous pend-ahead pipeline (scores 2 ahead, exp 1
            # ahead) so nothing resets at q-tile boundaries
            all_units = []
            for qt in range(QT):
                for hp in range(MSUB):
                    for jt in range(4 * (qt + 1)):
                        all_units.append((qt, hp, jt))
            NU = len(all_units)
            qt_base = {}
            for wi, (qt, hp, jt) in enumerate(all_units):
                if qt not in qt_base:
                    qt_base[qt] = wi

            ohTs = []
            sweep_pv = {}
            pend_tp = []  # (ohq_t, ohT, hp, h, qs, wi_pushed)
            pend_cp = []  # (tpt, ohT, hp, h, qs, wi_emitted)
            pend_pv = []  # sweep-start pv emissions deferred one block
            ctxs = {}
            filler_state = {"fillers": [], "fi": 0, "nf": 0, "nfront": 0}

            # oh transpose via regular matmul against the identity:
            # out[d, j] = sum_q ohq[q, d] * I[q, j] = ohq.T.  Each
            # transpose gets its own chain-pool generation — PSUM
            # start=True zeroes lazily at bank granularity, so an
            # accumulator bank must never host two live groups.
            def do_transpose(ent, wi):
                ohq_t, ohT_e, ehp, eh, qs = ent
                tpt = ps_c.tile([P, 512], F32, tag="pp", name="tp")
                nc.tensor.matmul(
                    tpt[eh * DK : (eh + 1) * DK, 0:P],
                    ohq_t[:],
                    id_sb[:],
                    start=True, stop=True,
                    skip_group_check=True,
                )
                pend_cp.append((tpt, ohT_e, ehp, eh, qs, wi))

            def do_copy(ent):
                tpt, ohT_e, ehp, eh, qs, _ = ent
                nc.vector.tensor_copy(
                    ohT_e[eh * DK : (eh + 1) * DK, ehp, qs * P : (qs + 1) * P],
                    tpt[eh * DK : (eh + 1) * DK, 0:P],
                )

            def qt_fillers(qt):
                """Filler steps to interleave into q-tile qt's blocks."""
                fillers = []
                nfront = 0
                if qt + 1 < QT:
                    st = qt + 1
                    fillers += proj_qk_steps("q", qT_sb, st)
                    if st < QT - 1:
                        fillers += proj_qk_steps("k", kT_sb, st)
                        fillers += proj_v_steps(st)
                if qt == QT - 1:
                    # k/v(st3) deferred into qt3 (exp-bound): feeds jt>=12,
                    # so front-load it within the first blocks
                    kv3 = proj_qk_steps("k", kT_sb, 3) + proj_v_steps(3)
                    fillers = kv3 + fillers
                    nfront = len(kv3)
                # outproj runs 2 q-tiles late: the late q-tiles are
                # exp-bound and need the extra PE filler
                if qt == 2:
                    fillers += outproj_steps(ohTs[0], 0)
                elif qt == 3:
                    fillers += outproj_steps(ohTs[1], 1)
                    fillers += outproj_steps(ohTs[2], 2)
                return fillers, nfront

            # prologue for block 0/1 of qt0
            ctxs[0] = {}
            emit_scores(all_units[0][0], *all_units[0][1:], ctxs[0])
            emit_exp(ctxs[0])
            ctxs[1] = {}
            emit_scores(all_units[1][0], *all_units[1][1:], ctxs[1])

            for wi, (qt, hp, jt) in enumerate(all_units):
                if wi == qt_base[qt]:
                    # q-tile entry: flush previous fillers, set up new ones
                    while filler_state["fi"] < filler_state["nf"]:
                        filler_state["fillers"][filler_state["fi"]]()
                        filler_state["fi"] += 1
                    fillers, nfront = qt_fillers(qt)
                    filler_state = {
                        "fillers": fillers,
                        "fi": 0,
                        "nf": len(fillers),
                        "nfront": nfront,
                    }
                    if qt + 2 < QT:
                        issue_x_dma(qt + 2)
                    ohT = ohp.tile([P, MSUB, 512], BF16, tag="ohT")
                    ohTs.append(ohT)
                    nu_qt = (
                        qt_base[qt + 1] - qt_base[qt]
                        if qt + 1 < QT
                        else NU - qt_base[qt]
                    )
                ctx = ctxs.pop(wi)
                # issue exp for the NEXT block early (ACT queue ahead)
                if wi + 1 < NU and "pp" not in ctxs.get(wi + 1, {"pp": 1}):
                    emit_exp(ctxs[wi + 1])
                # epilogue pipeline: transpose ~2 blocks after the DVE
                # scale was issued, ohT copy ~1 block after the transpose
                while pend_tp and pend_tp[0][5] <= wi - 2:
                    ent = pend_tp.pop(0)
                    do_transpose(ent[:5], wi)
                while pend_cp and pend_cp[0][5] <= wi - 1:
                    do_copy(pend_cp.pop(0))
                # filler quota (local block index within this q-tile)
                bi = wi - qt_base[qt]
                nf = filler_state["nf"]
                nfront = filler_state["nfront"]
                want = (bi + 1) * nf // nu_qt
                if nfront and bi < 12:
                    want = max(want, min(nfront, (bi + 1) * nfront // 11))
                # at sweep starts the pv-tile WAR wait (previous sweep's
                # epilogue reads on DVE) stalls the PE — run a filler
                # first so the DVE drains behind real PE work
                sc_emitted = False
                if jt <= 1 and filler_state["fi"] < want:
                    filler_state["fillers"][filler_state["fi"]]()
                    filler_state["fi"] += 1
                # PV for current block (both heads)
                def emit_pv(qt, hp, jt, pp, di, ohT_e, wi):
                    q0 = di if di > 0 else 0
                    for h in range(2):
                        pv_t = sweep_pv[hp, h]
                        hh = hp * 2 + h
                        for qs in range(q0, 4):
                            last = jt == qt * 4 + qs
                            # start only on the bank's first group touch:
                            # PSUM start zeroes the whole bank lazily, so
                            # sibling slices rely on that single mark
                            nc.tensor.matmul(
                                pv_t[:, qs, :],
                                pp[:, h, qs * P : (qs + 1) * P],
                                v_sb[:, jt, hh, :],
                                start=(jt == 0 and qs == q0), stop=last,
                                skip_group_check=True,
                            )
                            if last:
                                # epilogue DVE: 1/z then scale into sbuf
                                zr = zrp.tile([P, 1], F32, tag="zr")
                                nc.vector.reciprocal(
                                    zr[:], pv_t[:, qs, DK : DK + 1]
                                )
                                ohq_t = ohqp.tile([P, DK], BF16, tag="ohq")
                                nc.vector.tensor_scalar_mul(
                                    ohq_t[:], pv_t[:, qs, 0:DK], zr[:]
                                )
                                pend_tp.append((ohq_t, ohT_e, hp, h, qs, wi))

                pp = ctx["pp"]
                di = ctx["di"]
                if jt == 0:
                    # defer the new sweep's first pv matmuls one block:
                    # the old generation's epilogue reads (DVE) get a full
                    # block to clear the pv-slot WAR before the start
                    for h in range(2):
                        sweep_pv[hp, h] = ps_v.tile(
                            [P, 4, DK + 1], F32, tag="pv", name="pv"
                        )
                    pend_pv.append((qt, hp, jt, pp, di, ohT, wi))
                else:
                    while pend_pv:
                        emit_pv(*pend_pv.pop(0))
                    emit_pv(qt, hp, jt, pp, di, ohT, wi)
                # emit scores for block wi+2 (pend-ahead depth 2)
                if wi + 2 < NU and not sc_emitted:
                    ctxs[wi + 2] = {}
                    u = all_units[wi + 2]
                    emit_scores(u[0], *u[1:], ctxs[wi + 2])
                # filler quota
                while filler_state["fi"] < want:
                    filler_state["fillers"][filler_state["fi"]]()
                    filler_state["fi"] += 1

            # pre-open the first two final outproj chains (mt 0..2 only
            # — those ohT pieces are long done) so the drain's DVE/PE
            # latency hides behind them
            ohT3 = ohTs[3]
            fin_pys = []
            for nt in range(2):
                py = ps_v.tile([P, 512], F32, tag="pv", name="pyf0")
                for mt in range(MSUB - 1):
                    nc.tensor.matmul(
                        py[:],
                        wo_sb[:, mt, nt * P : (nt + 1) * P],
                        ohT3[:, mt, :],
                        start=(mt == 0), stop=False,
                    )
                fin_pys.append(py)

            # drain the tail pipeline, remaining fillers interleaved
            nrem = len(pend_tp)
            for ri in range(nrem):
                ent = pend_tp.pop(0)
                if filler_state["fi"] < filler_state["nf"]:
                    filler_state["fillers"][filler_state["fi"]]()
                    filler_state["fi"] += 1
                do_transpose(ent[:5], NU + ri)
                while pend_cp and pend_cp[0][5] <= NU + ri - 1:
                    do_copy(pend_cp.pop(0))
            while filler_state["fi"] < filler_state["nf"]:
                filler_state["fillers"][filler_state["fi"]]()
                filler_state["fi"] += 1
            while pend_cp:
                do_copy(pend_cp.pop(0))

            # final output projection for qt=3: per-nt eviction + DMA on
            # alternating engines so the tail after the last matmul is
            # one small copy + one 1KB-row DMA.  Chains come from the
            # (now idle) pv pool so they don't contend with the drain's
            # transpose generations in the chain pool.
            qt3 = 3
            for nt in range(NT):
                y1f = ysp.tile([P, 512], BF16, tag="y2", name="y1f")
                if nt < 2:
                    py = fin_pys[nt]
                    nc.tensor.matmul(
                        py[:],
                        wo_sb[:, MSUB - 1, nt * P : (nt + 1) * P],
                        ohT3[:, MSUB - 1, :],
                        start=False, stop=True,
                    )
                elif nt < NT - 1:
                    py = ps_v.tile([P, 512], F32, tag="pv", name="pyf")
                    for mt in range(MSUB):
                        nc.tensor.matmul(
                            py[:],
                            wo_sb[:, mt, nt * P : (nt + 1) * P],
                            ohT3[:, mt, :],
                            start=(mt == 0), stop=(mt == MSUB - 1),
                        )
                else:
                    # last nt: two half-N chains; first half's eviction and
                    # DMA overlap the second half's matmuls
                    py = ps_v.tile([P, 512], F32, tag="pv", name="pyf")
                    for half in range(2):
                        hs = slice(half * 256, (half + 1) * 256)
                        for mt in range(MSUB):
                            nc.tensor.matmul(
                                py[:, hs],
                                wo_sb[:, mt, nt * P : (nt + 1) * P],
                                ohT3[:, mt, half * 256 : (half + 1) * 256],
                                start=(mt == 0 and half == 0),
                                stop=(mt == MSUB - 1 and half == 1),
                                skip_group_check=True,
                            )
                        if half == 0:
                            nc.vector.tensor_copy(y1f[:, hs], py[:, hs])
                            nc.sync.dma_start(yT4[qt3, :, nt, hs], y1f[:, hs])
                        else:
                            nc.scalar.copy(y1f[:, hs], py[:, hs])
                            nc.scalar.dma_start(yT4[qt3, :, nt, hs], y1f[:, hs])
                    continue
                if nt % 2 == 0:
                    nc.vector.tensor_copy(y1f[:], py[:])
                    nc.sync.dma_start(yT4[qt3, :, nt, :], y1f[:])
                else:
                    nc.scalar.copy(y1f[:], py[:])
                    nc.scalar.dma_start(yT4[qt3, :, nt, :], y1f[:])

    nc.finalize()
    return nc


_CACHED_NC = None


def _get_nc() -> bass.Bass:
    global _CACHED_NC
    if _CACHED_NC is None:
        _CACHED_NC = build_nc()
    return _CACHED_NC


def _make_tm2() -> np.ndarray:
    import ml_dtypes

    k = np.arange(P)[:, None]
    j = np.arange(P)[None, :]
    tm = np.where(j >= k, 1.0, 0.0).astype(np.float32)
    return np.stack([tm, tm], axis=1).astype(ml_dtypes.bfloat16)


def _make_ident() -> np.ndarray:
    import ml_dtypes

    return np.eye(P, dtype=np.float32).astype(ml_dtypes.bfloat16)


def make_in_maps(inputs):
    import ml_dtypes

    bf = ml_dtypes.bfloat16
    x = np.asarray(inputs["x"], np.float32)
    q_heads = np.asarray(inputs["q_heads"], np.float32)
    k_heads = np.asarray(inputs["k_heads"], np.float32)
    v_heads = np.asarray(inputs["v_heads"], np.float32)
    output_proj = np.asarray(inputs["output_proj"], np.float32)

    tm = _make_tm2()
    idn = _make_ident()

    def tile_w(wT):  # [1024, 512] -> [p, kt, m]
        return np.ascontiguousarray(
            wT.reshape(KT, P, MG).transpose(1, 0, 2)
        ).astype(bf)

    in_maps = []
    for core in range(N_CORES):
        b, g = divmod(core, 2)
        gsl = slice(g * MG, (g + 1) * MG)
        xT = x[b].T  # [1024, 2048]
        xt4 = np.ascontiguousarray(
            xT.reshape(KT, P, QT, 512).transpose(2, 1, 0, 3)
        ).astype(bf)  # [st, p, kt, 512]
        wo = output_proj[:, gsl].T  # [512, 1024]
        wo4 = np.ascontiguousarray(
            wo.reshape(MSUB, P, D_MODEL).transpose(1, 0, 2)
        ).astype(bf)
        in_maps.append(
            {
                "xt4": xt4,
                "wq4": tile_w(q_heads[gsl].T),
                "wk4": tile_w(k_heads[gsl].T),
                "wv4": tile_w(v_heads[gsl].T),
                "wo4": wo4,
                "tm2": tm,
                "ident": idn,
            }
        )
    return in_maps


def kernel(x, q_heads, k_heads, v_heads, output_proj):
    inputs = {
        "x": x,
        "q_heads": q_heads,
        "k_heads": k_heads,
        "v_heads": v_heads,
        "output_proj": output_proj,
    }
    in_maps = make_in_maps(inputs)
    nc = _get_nc()
    res = run_bass_kernel_spmd(nc, in_maps, list(range(N_CORES)))
    y = np.empty((B, S, D_MODEL), np.float32)
    for b in range(B):
        acc = res.results[2 * b]["yT4"].astype(np.float32) + res.results[
            2 * b + 1
        ]["yT4"].astype(np.float32)
        yT = acc.transpose(2, 1, 0, 3).reshape(D_MODEL, S)
        y[b] = yT.T
    return y


# revision 44
# speedup vs baseline: 12076.9438x; 1.0038x over previous
"""Causal MHA on 8 trn2 cores — v3: transposed-PV schedule.

Sharding: 8 cores = 4 batches x 2 head-groups (8 heads each).

v3 changes vs v2:
- PV computed transposed: out[q, dk] = pp[keys, q].T @ v[keys, dk] —
  stationary = probs tile, moving = v (N=64) — halves PV matmul time
  (cost scales with moving free size; old orientation paid N=512 for
  M=65 useful rows).
- Z (softmax denom) via N=1 ones-column matmuls chained like PV.
- causal mask as a post-exp DVE 0/1 multiply (off the PE).
- per-(head,qsub) epilogue: reciprocal + per-partition tensor_scalar
  scale, then a PE transpose places oh back in [dk, q] for the output
  projection (tile_position lands head 1 in partitions 64:128).
- per-head scores/exp (ss [P,512] f32, 3 PSUM banks) with 2-deep
  pend-ahead so ACT stays fed through the exp-bound late q-tiles.

PSUM budget (8 banks): ss 3 + pv 2 + chain 2 + (z, tp slivers) 1.
"""

import sys

if "/opt/trn_rl_repo" not in sys.path:
    sys.path.insert(0, "/opt/trn_rl_repo")

import numpy as np

import concourse.bass as bass
import concourse.mybir as mybir
from concourse import bacc, tile
from concourse.bass_utils import run_bass_kernel_spmd

P = 128
D_MODEL = 1024
NUM_HEADS = 16
DK = 64
B, S = 4, 2048
HG = NUM_HEADS // 2
MG = HG * DK
N_CORES = 8

QT = S // 512
JT = S // P
KT = D_MODEL // P
MSUB = MG // P
NT = D_MODEL // P

F32 = mybir.dt.float32
BF16 = mybir.dt.bfloat16
EXP = mybir.ActivationFunctionType.Exp
MULT = mybir.AluOpType.mult


def build_nc() -> bass.Bass:
    nc = bacc.Bacc("TRN2", target_bir_lowering=False, debug=False)

    # inputs pre-tiled host-side to partition-major layout so every DMA
    # partition-row is 8KB contiguous
    xt4 = nc.dram_tensor("xt4", [QT, P, KT, 512], BF16, kind="ExternalInput")
    wq4 = nc.dram_tensor("wq4", [P, KT, MG], BF16, kind="ExternalInput")
    wk4 = nc.dram_tensor("wk4", [P, KT, MG], BF16, kind="ExternalInput")
    wv4 = nc.dram_tensor("wv4", [P, KT, MG], BF16, kind="ExternalInput")
    wo4 = nc.dram_tensor("wo4", [P, MSUB, D_MODEL], BF16, kind="ExternalInput")
    tm2 = nc.dram_tensor("tm2", [P, 2, P], BF16, kind="ExternalInput")
    ident = nc.dram_tensor("ident", [P, P], BF16, kind="ExternalInput")
    # output y^T tiled [qt, p, nt, 512]
    yT4 = nc.dram_tensor("yT4", [QT, P, NT, 512], BF16, kind="ExternalOutput")

    with tile.TileContext(nc) as tc:
        with (
            tc.tile_pool(name="wpool", bufs=1) as wpool,
            tc.tile_pool(name="qkv", bufs=1) as qkv,
            tc.tile_pool(name="xs", bufs=2) as xs,
            tc.tile_pool(name="oh", bufs=3) as ohp,
            tc.tile_pool(name="ys", bufs=4) as ysp,
            tc.tile_pool(name="attn", bufs=4) as attn,
            tc.tile_pool(name="attnc", bufs=1) as attnc,
            tc.tile_pool(name="ohq", bufs=6) as ohqp,
            tc.tile_pool(name="zr", bufs=6) as zrp,
            # PSUM pools — creation order fixes bank packing:
            # ss 3 banks, pv 2, chains 2, z+tp slivers in bank 8
            tc.tile_pool(name="ps_s", bufs=2, space="PSUM") as ps_s,
            tc.tile_pool(name="ps_v", bufs=2, space="PSUM") as ps_v,
            tc.tile_pool(name="ps_c", bufs=2, space="PSUM") as ps_c,
        ):
            # ---- persistent sbuf ----
            w_sb = {}
            for name in ("q", "k", "v"):
                w_sb[name] = wpool.tile(
                    [P, KT, MG], BF16, tag=f"w{name}", name=f"w{name}"
                )
            wo_sb = wpool.tile([P, MSUB, D_MODEL], BF16, tag="wo")
            qT_sb = qkv.tile([P, MSUB, S], BF16, tag="qT")
            kT_sb = qkv.tile([P, MSUB, S], BF16, tag="kT")
            v_sb = qkv.tile([P, JT, HG, DK + 1], BF16, tag="v")

            tm_sb = attnc.tile([P, 2, P], BF16, tag="tm")
            id_sb = attnc.tile([P, P], BF16, tag="id")

            # ---- input DMA ----
            # issue cost is ~565-667ns per dma_start, serial per engine —
            # split the startup DMAs across the two HWDGE engines (SP +
            # Activation; ACT is idle until the first exp) so the first
            # projection's dependencies land by ~2.5us
            warm_src = attnc.tile([P, 256], BF16, tag="warm_src")
            nc.vector.memset(warm_src[:], 0.5)
            nc.gpsimd.memset(v_sb[:, :, :, DK : DK + 1], 1.0)
            x_tiles = [None] * QT

            def issue_x_dma(st):
                x_tiles[st] = xs.tile([P, KT, 512], BF16, tag="x", name=f"x{st}")
                for kp in range(4):
                    nc.sync.dma_start(
                        x_tiles[st][:, 2 * kp : 2 * kp + 2],
                        xt4[st, :, 2 * kp : 2 * kp + 2],
                    )

            # SP: q/k/v weight halves then wo; ACT: x0 + mask/identity
            nc.sync.dma_start(w_sb["q"][:, 0:4], wq4[:, 0:4])
            x_tiles[0] = xs.tile([P, KT, 512], BF16, tag="x", name="x0")
            nc.scalar.dma_start(x_tiles[0][:, 0:4], xt4[0, :, 0:4])
            nc.sync.dma_start(w_sb["q"][:, 4:8], wq4[:, 4:8])
            nc.scalar.dma_start(x_tiles[0][:, 4:8], xt4[0, :, 4:8])
            for name, wsrc in (("k", wk4), ("v", wv4)):
                nc.sync.dma_start(w_sb[name][:, 0:4], wsrc[:, 0:4])
                nc.sync.dma_start(w_sb[name][:, 4:8], wsrc[:, 4:8])
            nc.scalar.dma_start(tm_sb[:], tm2[:])
            nc.scalar.dma_start(id_sb[:], ident[:])
            nc.sync.dma_start(wo_sb[:], wo4[:])

            # warm the PE while the x/w DMAs land
            warm = ps_c.tile([P, 512], F32, tag="pp", name="warm")
            for _ in range(20):
                nc.tensor.matmul(
                    warm[:, 0:256], warm_src[:, 0:P], warm_src[:],
                    start=True, stop=True, skip_group_check=True,
                )

            # ---- filler-step factories (each step = ~4 matmuls on PE) ----
            def proj_qk_steps(name, dst, st):
                ssl = slice(st * 512, (st + 1) * 512)
                w = w_sb[name]
                x_t = x_tiles[st]
                steps = []
                for mt in range(MSUB):
                    msl = slice(mt * P, (mt + 1) * P)
                    holder = {}

                    def sk(k0, k1, mt=mt, msl=msl, holder=holder):
                        if k0 == 0:
                            holder["pt"] = ps_c.tile(
                                [P, 512], F32, tag="pp", name="prq"
                            )
                        pt = holder["pt"]
                        for kt in range(k0, k1):
                            nc.tensor.matmul(
                                pt[:], w[:, kt, msl], x_t[:, kt],
                                start=(kt == 0), stop=(kt == KT - 1),
                            )
                        if k1 == KT:
                            nc.vector.tensor_copy(dst[:, mt, ssl], pt[:])

                    # 2-matmul sub-steps: finer quota placement absorbs
                    # sub-500ns PE stalls
                    for k0 in range(0, KT, 2):
                        steps.append(
                            lambda k0=k0, sk=sk: sk(k0, k0 + 2)
                        )
                return steps

            def proj_v_steps(st):
                x_t = x_tiles[st]
                steps = []
                for ssub in range(4):
                    jt = st * 4 + ssub
                    s0 = ssub * P
                    holder = {}

                    def sk(k0, k1, jt=jt, s0=s0, holder=holder):
                        if k0 == 0:
                            holder["pt"] = ps_c.tile(
                                [P, 512], F32, tag="pp", name="prv"
                            )
                        pt = holder["pt"]
                        for kt in range(k0, k1):
                            nc.tensor.matmul(
                                pt[:], x_t[:, kt, s0 : s0 + P], w_sb["v"][:, kt],
                                start=(kt == 0), stop=(kt == KT - 1),
                            )
                        if k1 == KT:
                            nc.vector.tensor_copy(
                                v_sb[:, jt, :, 0:DK],
                                pt.rearrange("p (h d) -> p h d", h=HG),
                            )

                    for k0 in range(0, KT, 2):
                        steps.append(
                            lambda k0=k0, sk=sk: sk(k0, k0 + 2)
                        )
                return steps

            def outproj_steps(ohT_prev, qt_prev):
                steps = []
                holder = {}
                for nt in range(NT):
                    def sm(m0, m1, nt=nt):
                        if m0 == 0:
                            holder["py"] = ps_c.tile(
                                [P, 512], F32, tag="pp", name="py"
                            )
                        py = holder["py"]
                        for mt in range(m0, m1):
                            nc.tensor.matmul(
                                py[:],
                                wo_sb[:, mt, nt * P : (nt + 1) * P],
                                ohT_prev[:, mt, :],
                                start=(mt == 0), stop=(mt == MSUB - 1),
                            )
                        if m1 < MSUB:
                            return
                        if nt % 4 == 0:
                            holder["y4"] = ysp.tile(
                                [P, 4, 512], BF16, tag="y", name="y4"
                            )
                        nc.vector.tensor_copy(holder["y4"][:, nt % 4, :], py[:])
                        if nt % 4 == 3:
                            nc.gpsimd.dma_start(
                                yT4[qt_prev, :, nt - 3 : nt + 1], holder["y4"][:]
                            )

                    steps.append(lambda sm=sm: sm(0, 2))
                    steps.append(lambda sm=sm: sm(2, 4))
                return steps

            # ---- attention primitives ----
            def emit_scores(qt, hp, jt, ctx):
                """Two K=64 matmuls: ss[keys, h, q] for the pair's heads."""
                jsl = slice(jt * P, (jt + 1) * P)
                di = jt - qt * 4
                delta = P * di if di >= 0 else 0
                qsl_d = slice(qt * 512 + delta, (qt + 1) * 512)
                ss = ps_s.tile([P, 2, 512], F32, tag="ss")
                for h in range(2):
                    hd = slice(h * DK, (h + 1) * DK)
                    nc.tensor.matmul(
                        ss[:, h, delta:],
                        kT_sb[hd, hp, jsl],
                        qT_sb[hd, hp, qsl_d],
                        start=True, stop=True, skip_group_check=True,
                    )
                ctx["ss"] = ss
                ctx["delta"] = delta
                ctx["di"] = di

            def emit_exp(ctx):
                ss, delta, di = ctx["ss"], ctx["delta"], ctx["di"]
                pp = attn.tile([P, 2, 512], BF16, tag="pp")
                nc.scalar.activation(
                    pp[:, :, delta:], ss[:, :, delta:], EXP, scale=0.125
                )
                if di >= 0:
                    # zero the upper-triangle of the diagonal 128-block
                    nc.vector.tensor_tensor(
                        pp[:, :, delta : delta + P],
                        pp[:, :, delta : delta + P],
                        tm_sb[:],
                        MULT,
                    )
                ctx["pp"] = pp

            # ---- main interleaved schedule ----
            issue_x_dma(1)
            st0_steps = (
                proj_qk_steps("q", qT_sb, 0)
                + proj_qk_steps("k", kT_sb, 0)
                + proj_v_steps(0)
            )
            # weave chain pairs: both chains' kt0-3 chunks first (their
            # DMA half lands first), then kt4-7 — two chains open max
            # (psum bufs=2), and the kt4+ work starts after the second
            # DMA half has landed
            order = []
            nch = len(st0_steps) // 4
            for c0 in range(0, nch, 2):
                a, b = 4 * c0, 4 * (c0 + 1)
                order += [st0_steps[a], st0_steps[a + 1]]
                if c0 + 1 < nch:
                    order += [st0_steps[b], st0_steps[b + 1]]
                order += [st0_steps[a + 2], st0_steps[a + 3]]
                if c0 + 1 < nch:
                    order += [st0_steps[b + 2], st0_steps[b + 3]]
            for step in order:
                step()

            # ---- flat block stream across all q-tiles ----
            # one continuous pend-ahead pipeline (scores 2 ahead, exp 1
            # ahead) so nothing resets at q-tile boundaries
            all_units = []
            for qt in range(QT):
                for hp in range(MSUB):
                    for jt in range(4 * (qt + 1)):
                        all_units.append((qt, hp, jt))
            NU = len(all_units)
            qt_base = {}
            for wi, (qt, hp, jt) in enumerate(all_units):
                if qt not in qt_base:
                    qt_base[qt] = wi

            ohTs = []
            sweep_pv = {}
            pend_tp = []  # (ohq_t, ohT, hp, h, qs, wi_pushed)
            pend_cp = []  # (tpt, ohT, hp, h, qs, wi_emitted)
            pend_pv = []  # sweep-start pv emissions deferred one block
            ctxs = {}
            filler_state = {"fillers": [], "fi": 0, "nf": 0, "nfront": 0}

            # oh transpose via regular matmul against the identity:
            # out[d, j] = sum_q ohq[q, d] * I[q, j] = ohq.T.  Each
            # transpose gets its own chain-pool generation — PSUM
            # start=True zeroes lazily at bank granularity, so an
            # accumulator bank must never host two live groups.
            def do_transpose(ent, wi):
                ohq_t, ohT_e, ehp, eh, qs = ent
                tpt = ps_c.tile([P, 512], F32, tag="pp", name="tp")
                nc.tensor.matmul(
                    tpt[eh * DK : (eh + 1) * DK, 0:P],
                    ohq_t[:],
                    id_sb[:],
                    start=True, stop=True,
                    skip_group_check=True,
                )
                pend_cp.append((tpt, ohT_e, ehp, eh, qs, wi))

            def do_copy(ent):
                tpt, ohT_e, ehp, eh, qs, _ = ent
                nc.vector.tensor_copy(
                    ohT_e[eh * DK : (eh + 1) * DK, ehp, qs * P : (qs + 1) * P],
                    tpt[eh * DK : (eh + 1) * DK, 0:P],
                )

            def qt_fillers(qt):
                """Filler steps to interleave into q-tile qt's blocks."""
                fillers = []
                nfront = 0
                if qt + 1 < QT:
                    st = qt + 1
                    fillers += proj_qk_steps("q", qT_sb, st)
                    if st < QT - 1:
                        fillers += proj_qk_steps("k", kT_sb, st)
                        fillers += proj_v_steps(st)
                if qt == QT - 1:
                    # k/v(st3) deferred into qt3 (exp-bound): feeds jt>=12,
                    # so front-load it within the first blocks
                    kv3 = proj_qk_steps("k", kT_sb, 3) + proj_v_steps(3)
                    fillers = kv3 + fillers
                    nfront = len(kv3)
                # outproj runs 2 q-tiles late: the late q-tiles are
                # exp-bound and need the extra PE filler
                if qt == 2:
                    fillers += outproj_steps(ohTs[0], 0)
                elif qt == 3:
                    fillers += outproj_steps(ohTs[1], 1)
                    fillers += outproj_steps(ohTs[2], 2)
                return fillers, nfront

            # prologue for block 0/1 of qt0
            ctxs[0] = {}
            emit_scores(all_units[0][0], *all_units[0][1:], ctxs[0])
            emit_exp(ctxs[0])
            ctxs[1] = {}
            emit_scores(all_units[1][0], *all_units[1][1:], ctxs[1])
            emit_exp(ctxs[1])

            for wi, (qt, hp, jt) in enumerate(all_units):
                if wi == qt_base[qt]:
                    # q-tile entry: flush previous fillers, set up new ones
                    while filler_state["fi"] < filler_state["nf"]:
                        filler_state["fillers"][filler_state["fi"]]()
                        filler_state["fi"] += 1
                    fillers, nfront = qt_fillers(qt)
                    filler_state = {
                        "fillers": fillers,
                        "fi": 0,
                        "nf": len(fillers),
                        "nfront": nfront,
                    }
                    if qt + 2 < QT:
                        issue_x_dma(qt + 2)
                    ohT = ohp.tile([P, MSUB, 512], BF16, tag="ohT")
                    ohTs.append(ohT)
                    nu_qt = (
                        qt_base[qt + 1] - qt_base[qt]
                        if qt + 1 < QT
                        else NU - qt_base[qt]
                    )
                ctx = ctxs.pop(wi)
                # epilogue pipeline: transpose ~2 blocks after the DVE
                # scale was issued, ohT copy ~1 block after the transpose
                while pend_tp and pend_tp[0][5] <= wi - 2:
                    ent = pend_tp.pop(0)
                    do_transpose(ent[:5], wi)
                while pend_cp and pend_cp[0][5] <= wi - 1:
                    do_copy(pend_cp.pop(0))
                # filler quota (local block index within this q-tile)
                bi = wi - qt_base[qt]
                nf = filler_state["nf"]
                nfront = filler_state["nfront"]
                want = (bi + 1) * nf // nu_qt
                if nfront and bi < 12:
                    want = max(want, min(nfront, (bi + 1) * nfront // 11))
                # at sweep starts the pv-tile WAR wait (previous sweep's
                # epilogue reads on DVE) stalls the PE — run a filler
                # first so the DVE drains behind real PE work
                sc_emitted = False
                if jt <= 1 and filler_state["fi"] < want:
                    filler_state["fillers"][filler_state["fi"]]()
                    filler_state["fi"] += 1
                # PV for current block (both heads)
                def emit_pv(qt, hp, jt, pp, di, ohT_e, wi):
                    q0 = di if di > 0 else 0
                    for h in range(2):
                        pv_t = sweep_pv[hp, h]
                        hh = hp * 2 + h
                        for qs in range(q0, 4):
                            last = jt == qt * 4 + qs
                            # start only on the bank's first group touch:
                            # PSUM start zeroes the whole bank lazily, so
                            # sibling slices rely on that single mark
                            nc.tensor.matmul(
                                pv_t[:, qs, :],
                                pp[:, h, qs * P : (qs + 1) * P],
                                v_sb[:, jt, hh, :],
                                start=(jt == 0 and qs == q0), stop=last,
                                skip_group_check=True,
                            )
                            if last:
                                # epilogue DVE: 1/z then scale into sbuf
                                zr = zrp.tile([P, 1], F32, tag="zr")
                                nc.vector.reciprocal(
                                    zr[:], pv_t[:, qs, DK : DK + 1]
                                )
                                ohq_t = ohqp.tile([P, DK], BF16, tag="ohq")
                                nc.vector.tensor_scalar_mul(
                                    ohq_t[:], pv_t[:, qs, 0:DK], zr[:]
                                )
                                pend_tp.append((ohq_t, ohT_e, hp, h, qs, wi))

                pp = ctx["pp"]
                di = ctx["di"]
                if jt == 0:
                    # defer the new sweep's first pv matmuls one block:
                    # the old generation's epilogue reads (DVE) get a full
                    # block to clear the pv-slot WAR before the start
                    for h in range(2):
                        sweep_pv[hp, h] = ps_v.tile(
                            [P, 4, DK + 1], F32, tag="pv", name="pv"
                        )
                    pend_pv.append((qt, hp, jt, pp, di, ohT, wi))
                else:
                    while pend_pv:
                        emit_pv(*pend_pv.pop(0))
                    emit_pv(qt, hp, jt, pp, di, ohT, wi)
                # emit scores for block wi+2 (pend-ahead depth 2), then
                # its exp straight into the ACT queue
                if wi + 2 < NU and not sc_emitted:
                    ctxs[wi + 2] = {}
                    u = all_units[wi + 2]
                    emit_scores(u[0], *u[1:], ctxs[wi + 2])
                if wi + 2 < NU and "pp" not in ctxs[wi + 2]:
                    emit_exp(ctxs[wi + 2])
                # filler quota
                while filler_state["fi"] < want:
                    filler_state["fillers"][filler_state["fi"]]()
                    filler_state["fi"] += 1

            # pre-open the first two final outproj chains (mt 0..2 only
            # — those ohT pieces are long done); their matmuls are fed
            # one-at-a-time between the drain's transposes so the PE has
            # cover while the last epilogues clear the DVE
            ohT3 = ohTs[3]
            fin_pys = []
            fin_steps = []
            for nt in range(2):
                py = ps_v.tile([P, 512], F32, tag="pv", name="pyf0")
                for mt in range(MSUB - 1):
                    def fs(py=py, mt=mt, nt=nt):
                        nc.tensor.matmul(
                            py[:],
                            wo_sb[:, mt, nt * P : (nt + 1) * P],
                            ohT3[:, mt, :],
                            start=(mt == 0), stop=False,
                        )
                    fin_steps.append(fs)
                fin_pys.append(py)

            # drain the tail pipeline, fin/filler work interleaved
            nrem = len(pend_tp)
            fsi = 0
            for ri in range(nrem):
                ent = pend_tp.pop(0)
                if filler_state["fi"] < filler_state["nf"]:
                    filler_state["fillers"][filler_state["fi"]]()
                    filler_state["fi"] += 1
                elif fsi < len(fin_steps):
                    fin_steps[fsi]()
                    fsi += 1
                do_transpose(ent[:5], NU + ri)
                while pend_cp and pend_cp[0][5] <= NU + ri - 1:
                    do_copy(pend_cp.pop(0))
            while filler_state["fi"] < filler_state["nf"]:
                filler_state["fillers"][filler_state["fi"]]()
                filler_state["fi"] += 1
            while fsi < len(fin_steps):
                fin_steps[fsi]()
                fsi += 1
            while pend_cp:
                do_copy(pend_cp.pop(0))

            # final output projection for qt=3: per-nt eviction + DMA on
            # alternating engines so the tail after the last matmul is
            # one small copy + one 1KB-row DMA.  Chains come from the
            # (now idle) pv pool so they don't contend with the drain's
            # transpose generations in the chain pool.
            qt3 = 3
            for nt in range(NT):
                y1f = ysp.tile([P, 512], BF16, tag="y2", name="y1f")
                if nt < 2:
                    py = fin_pys[nt]
                    nc.tensor.matmul(
                        py[:],
                        wo_sb[:, MSUB - 1, nt * P : (nt + 1) * P],
                        ohT3[:, MSUB - 1, :],
                        start=False, stop=True,
                    )
                elif nt < NT - 1:
                    py = ps_v.tile([P, 512], F32, tag="pv", name="pyf")
                    for mt in range(MSUB):
                        nc.tensor.matmul(
                            py[:],
                            wo_sb[:, mt, nt * P : (nt + 1) * P],
                            ohT3[:, mt, :],
                            start=(mt == 0), stop=(mt == MSUB - 1),
                        )
                else:
                    # last nt: two half-N chains; first half's eviction and
                    # DMA overlap the second half's matmuls
                    py = ps_v.tile([P, 512], F32, tag="pv", name="pyf")
                    for half in range(2):
                        hs = slice(half * 256, (half + 1) * 256)
                        for mt in range(MSUB):
                            nc.tensor.matmul(
                                py[:, hs],
                                wo_sb[:, mt, nt * P : (nt + 1) * P],
                                ohT3[:, mt, half * 256 : (half + 1) * 256],
                                start=(mt == 0 and half == 0),
                                stop=(mt == MSUB - 1 and half == 1),
                                skip_group_check=True,
                            )
                        if half == 0:
                            nc.vector.tensor_copy(y1f[:, hs], py[:, hs])
                            nc.sync.dma_start(yT4[qt3, :, nt, hs], y1f[:, hs])
                        else:
                            nc.scalar.copy(y1f[:, hs], py[:, hs])
                            nc.scalar.dma_start(yT4[qt3, :, nt, hs], y1f[:, hs])
                    continue
                if nt % 2 == 0:
                    nc.vector.tensor_copy(y1f[:], py[:])
                    nc.sync.dma_start(yT4[qt3, :, nt, :], y1f[:])
                else:
                    nc.scalar.copy(y1f[:], py[:])
                    nc.scalar.dma_start(yT4[qt3, :, nt, :], y1f[:])

    nc.finalize()
    return nc


_CACHED_NC = None


def _get_nc() -> bass.Bass:
    global _CACHED_NC
    if _CACHED_NC is None:
        _CACHED_NC = build_nc()
    return _CACHED_NC


def _make_tm2() -> np.ndarray:
    import ml_dtypes

    k = np.arange(P)[:, None]
    j = np.arange(P)[None, :]
    tm = np.where(j >= k, 1.0, 0.0).astype(np.float32)
    return np.stack([tm, tm], axis=1).astype(ml_dtypes.bfloat16)


def _make_ident() -> np.ndarray:
    import ml_dtypes

    return np.eye(P, dtype=np.float32).astype(ml_dtypes.bfloat16)


def make_in_maps(inputs):
    import ml_dtypes

    bf = ml_dtypes.bfloat16
    x = np.asarray(inputs["x"], np.float32)
    q_heads = np.asarray(inputs["q_heads"], np.float32)
    k_heads = np.asarray(inputs["k_heads"], np.float32)
    v_heads = np.asarray(inputs["v_heads"], np.float32)
    output_proj = np.asarray(inputs["output_proj"], np.float32)

    tm = _make_tm2()
    idn = _make_ident()

    def tile_w(wT):  # [1024, 512] -> [p, kt, m]
        return np.ascontiguousarray(
            wT.reshape(KT, P, MG).transpose(1, 0, 2)
        ).astype(bf)

    in_maps = []
    for core in range(N_CORES):
        b, g = divmod(core, 2)
        gsl = slice(g * MG, (g + 1) * MG)
        xT = x[b].T  # [1024, 2048]
        xt4 = np.ascontiguousarray(
            xT.reshape(KT, P, QT, 512).transpose(2, 1, 0, 3)
        ).astype(bf)  # [st, p, kt, 512]
        wo = output_proj[:, gsl].T  # [512, 1024]
        wo4 = np.ascontiguousarray(
            wo.reshape(MSUB, P, D_MODEL).transpose(1, 0, 2)
        ).astype(bf)
        in_maps.append(
            {
                "xt4": xt4,
                "wq4": tile_w(q_heads[gsl].T),
                "wk4": tile_w(k_heads[gsl].T),
                "wv4": tile_w(v_heads[gsl].T),
                "wo4": wo4,
                "tm2": tm,
                "ident": idn,
            }
        )
    return in_maps


def kernel(x, q_heads, k_heads, v_heads, output_proj):
    inputs = {
        "x": x,
        "q_heads": q_heads,
        "k_heads": k_heads,
        "v_heads": v_heads,
        "output_proj": output_proj,
    }
    in_maps = make_in_maps(inputs)
    nc = _get_nc()
    res = run_bass_kernel_spmd(nc, in_maps, list(range(N_CORES)))
    y = np.empty((B, S, D_MODEL), np.float32)
    for b in range(B):
        acc = res.results[2 * b]["yT4"].astype(np.float32) + res.results[
            2 * b + 1
        ]["yT4"].astype(np.float32)
        yT = acc.transpose(2, 1, 0, 3).reshape(D_MODEL, S)
        y[b] = yT.T
    return y


# revision 48
# speedup vs baseline: 12399.2216x; 1.0267x over previous
"""Causal MHA on 8 trn2 cores — v3: transposed-PV schedule.

Sharding: 8 cores = 4 batches x 2 head-groups (8 heads each).

v3 changes vs v2:
- PV computed transposed: out[q, dk] = pp[keys, q].T @ v[keys, dk] —
  stationary = probs tile, moving = v (N=64) — halves PV matmul time
  (cost scales with moving free size; old orientation paid N=512 for
  M=65 useful rows).
- Z (softmax denom) via N=1 ones-column matmuls chained like PV.
- causal mask as a post-exp DVE 0/1 multiply (off the PE).
- per-(head,qsub) epilogue: reciprocal + per-partition tensor_scalar
  scale, then a PE transpose places oh back in [dk, q] for the output
  projection (tile_position lands head 1 in partitions 64:128).
- per-head scores/exp (ss [P,512] f32, 3 PSUM banks) with 2-deep
  pend-ahead so ACT stays fed through the exp-bound late q-tiles.

PSUM budget (8 banks): ss 3 + pv 2 + chain 2 + (z, tp slivers) 1.
"""

import sys

if "/opt/trn_rl_repo" not in sys.path:
    sys.path.insert(0, "/opt/trn_rl_repo")

import numpy as np

import concourse.bass as bass
import concourse.mybir as mybir
from concourse import bacc, tile
from concourse.bass_utils import run_bass_kernel_spmd

P = 128
D_MODEL = 1024
NUM_HEADS = 16
DK = 64
B, S = 4, 2048
HG = NUM_HEADS // 2
MG = HG * DK
N_CORES = 8

QT = S // 512
JT = S // P
KT = D_MODEL // P
MSUB = MG // P
NT = D_MODEL // P

F32 = mybir.dt.float32
BF16 = mybir.dt.bfloat16
EXP = mybir.ActivationFunctionType.Exp
MULT = mybir.AluOpType.mult


def build_nc() -> bass.Bass:
    nc = bacc.Bacc("TRN2", target_bir_lowering=False, debug=False)

    # inputs pre-tiled host-side to partition-major layout so every DMA
    # partition-row is 8KB contiguous
    xt4 = nc.dram_tensor("xt4", [QT, P, KT, 512], BF16, kind="ExternalInput")
    wq4 = nc.dram_tensor("wq4", [P, KT, MG], BF16, kind="ExternalInput")
    wk4 = nc.dram_tensor("wk4", [P, KT, MG], BF16, kind="ExternalInput")
    wv4 = nc.dram_tensor("wv4", [P, KT, MG], BF16, kind="ExternalInput")
    wo4 = nc.dram_tensor("wo4", [P, MSUB, D_MODEL], BF16, kind="ExternalInput")
    tm2 = nc.dram_tensor("tm2", [P, 2, P], BF16, kind="ExternalInput")
    ident = nc.dram_tensor("ident", [P, P], BF16, kind="ExternalInput")
    # output y^T tiled [qt, p, nt, 512]
    yT4 = nc.dram_tensor("yT4", [QT, P, NT, 512], BF16, kind="ExternalOutput")

    with tile.TileContext(nc) as tc:
        with (
            tc.tile_pool(name="wpool", bufs=1) as wpool,
            tc.tile_pool(name="qkv", bufs=1) as qkv,
            tc.tile_pool(name="xs", bufs=2) as xs,
            tc.tile_pool(name="oh", bufs=3) as ohp,
            tc.tile_pool(name="ys", bufs=4) as ysp,
            tc.tile_pool(name="attn", bufs=5) as attn,
            tc.tile_pool(name="attnc", bufs=1) as attnc,
            tc.tile_pool(name="ohq", bufs=8) as ohqp,
            tc.tile_pool(name="zr", bufs=6) as zrp,
            # PSUM pools — creation order fixes bank packing:
            # ss 3 banks, pv 2, chains 2, z+tp slivers in bank 8
            tc.tile_pool(name="ps_s", bufs=2, space="PSUM") as ps_s,
            tc.tile_pool(name="ps_v", bufs=2, space="PSUM") as ps_v,
            tc.tile_pool(name="ps_c", bufs=2, space="PSUM") as ps_c,
        ):
            # ---- persistent sbuf ----
            w_sb = {}
            for name in ("q", "k", "v"):
                w_sb[name] = wpool.tile(
                    [P, KT, MG], BF16, tag=f"w{name}", name=f"w{name}"
                )
            wo_sb = wpool.tile([P, MSUB, D_MODEL], BF16, tag="wo")
            qT_sb = qkv.tile([P, MSUB, S], BF16, tag="qT")
            kT_sb = qkv.tile([P, MSUB, S], BF16, tag="kT")
            v_sb = qkv.tile([P, JT, HG, DK + 1], BF16, tag="v")

            tm_sb = attnc.tile([P, 2, P], BF16, tag="tm")
            id_sb = attnc.tile([P, P], BF16, tag="id")

            # ---- input DMA ----
            # issue cost is ~565-667ns per dma_start, serial per engine —
            # split the startup DMAs across the two HWDGE engines (SP +
            # Activation; ACT is idle until the first exp) so the first
            # projection's dependencies land by ~2.5us
            warm_src = attnc.tile([P, 256], BF16, tag="warm_src")
            nc.vector.memset(warm_src[:], 0.5)
            nc.gpsimd.memset(v_sb[:, :, :, DK : DK + 1], 1.0)
            x_tiles = [None] * QT

            def issue_x_dma(st):
                x_tiles[st] = xs.tile([P, KT, 512], BF16, tag="x", name=f"x{st}")
                for kp in range(4):
                    nc.sync.dma_start(
                        x_tiles[st][:, 2 * kp : 2 * kp + 2],
                        xt4[st, :, 2 * kp : 2 * kp + 2],
                    )

            # SP: q/k/v weight halves then wo; ACT: x0 + mask/identity
            nc.sync.dma_start(w_sb["q"][:, 0:4], wq4[:, 0:4])
            x_tiles[0] = xs.tile([P, KT, 512], BF16, tag="x", name="x0")
            nc.scalar.dma_start(x_tiles[0][:, 0:4], xt4[0, :, 0:4])
            nc.sync.dma_start(w_sb["q"][:, 4:8], wq4[:, 4:8])
            nc.scalar.dma_start(x_tiles[0][:, 4:8], xt4[0, :, 4:8])
            for name, wsrc in (("k", wk4), ("v", wv4)):
                nc.sync.dma_start(w_sb[name][:, 0:4], wsrc[:, 0:4])
                nc.sync.dma_start(w_sb[name][:, 4:8], wsrc[:, 4:8])
            nc.scalar.dma_start(tm_sb[:], tm2[:])
            nc.scalar.dma_start(id_sb[:], ident[:])
            nc.sync.dma_start(wo_sb[:], wo4[:])

            # warm the PE while the x/w DMAs land
            warm = ps_c.tile([P, 512], F32, tag="pp", name="warm")
            for _ in range(20):
                nc.tensor.matmul(
                    warm[:, 0:256], warm_src[:, 0:P], warm_src[:],
                    start=True, stop=True, skip_group_check=True,
                )

            # ---- filler-step factories (each step = ~4 matmuls on PE) ----
            def proj_qk_steps(name, dst, st):
                ssl = slice(st * 512, (st + 1) * 512)
                w = w_sb[name]
                x_t = x_tiles[st]
                steps = []
                for mt in range(MSUB):
                    msl = slice(mt * P, (mt + 1) * P)
                    holder = {}

                    def sk(k0, k1, mt=mt, msl=msl, holder=holder):
                        if k0 == 0:
                            holder["pt"] = ps_c.tile(
                                [P, 512], F32, tag="pp", name="prq"
                            )
                        pt = holder["pt"]
                        for kt in range(k0, k1):
                            nc.tensor.matmul(
                                pt[:], w[:, kt, msl], x_t[:, kt],
                                start=(kt == 0), stop=(kt == KT - 1),
                            )
                        if k1 == KT:
                            nc.vector.tensor_copy(dst[:, mt, ssl], pt[:])

                    # 2-matmul sub-steps: finer quota placement absorbs
                    # sub-500ns PE stalls
                    for k0 in range(0, KT, 2):
                        steps.append(
                            lambda k0=k0, sk=sk: sk(k0, k0 + 2)
                        )
                return steps

            def proj_v_steps(st):
                x_t = x_tiles[st]
                steps = []
                for ssub in range(4):
                    jt = st * 4 + ssub
                    s0 = ssub * P
                    holder = {}

                    def sk(k0, k1, jt=jt, s0=s0, holder=holder):
                        if k0 == 0:
                            holder["pt"] = ps_c.tile(
                                [P, 512], F32, tag="pp", name="prv"
                            )
                        pt = holder["pt"]
                        for kt in range(k0, k1):
                            nc.tensor.matmul(
                                pt[:], x_t[:, kt, s0 : s0 + P], w_sb["v"][:, kt],
                                start=(kt == 0), stop=(kt == KT - 1),
                            )
                        if k1 == KT:
                            nc.vector.tensor_copy(
                                v_sb[:, jt, :, 0:DK],
                                pt.rearrange("p (h d) -> p h d", h=HG),
                            )

                    for k0 in range(0, KT, 2):
                        steps.append(
                            lambda k0=k0, sk=sk: sk(k0, k0 + 2)
                        )
                return steps

            def outproj_steps(ohT_prev, qt_prev):
                steps = []
                holder = {}
                for nt in range(NT):
                    def sm(m0, m1, nt=nt):
                        if m0 == 0:
                            holder["py"] = ps_c.tile(
                                [P, 512], F32, tag="pp", name="py"
                            )
                        py = holder["py"]
                        for mt in range(m0, m1):
                            nc.tensor.matmul(
                                py[:],
                                wo_sb[:, mt, nt * P : (nt + 1) * P],
                                ohT_prev[:, mt, :],
                                start=(mt == 0), stop=(mt == MSUB - 1),
                            )
                        if m1 < MSUB:
                            return
                        if nt % 4 == 0:
                            holder["y4"] = ysp.tile(
                                [P, 4, 512], BF16, tag="y", name="y4"
                            )
                        nc.vector.tensor_copy(holder["y4"][:, nt % 4, :], py[:])
                        if nt % 4 == 3:
                            nc.gpsimd.dma_start(
                                yT4[qt_prev, :, nt - 3 : nt + 1], holder["y4"][:]
                            )

                    steps.append(lambda sm=sm: sm(0, 2))
                    steps.append(lambda sm=sm: sm(2, 4))
                return steps

            # ---- attention primitives ----
            def emit_scores(qt, hp, jt, ctx):
                """Two K=64 matmuls: ss[keys, h, q] for the pair's heads."""
                jsl = slice(jt * P, (jt + 1) * P)
                di = jt - qt * 4
                delta = P * di if di >= 0 else 0
                qsl_d = slice(qt * 512 + delta, (qt + 1) * 512)
                ss = ps_s.tile([P, 2, 512], F32, tag="ss")
                for h in range(2):
                    hd = slice(h * DK, (h + 1) * DK)
                    nc.tensor.matmul(
                        ss[:, h, delta:],
                        kT_sb[hd, hp, jsl],
                        qT_sb[hd, hp, qsl_d],
                        start=True, stop=True, skip_group_check=True,
                    )
                ctx["ss"] = ss
                ctx["delta"] = delta
                ctx["di"] = di

            def emit_exp(ctx):
                ss, delta, di = ctx["ss"], ctx["delta"], ctx["di"]
                pp = attn.tile([P, 2, 512], BF16, tag="pp")
                nc.scalar.activation(
                    pp[:, :, delta:], ss[:, :, delta:], EXP, scale=0.125
                )
                if di >= 0:
                    # zero the upper-triangle of the diagonal 128-block
                    nc.vector.tensor_tensor(
                        pp[:, :, delta : delta + P],
                        pp[:, :, delta : delta + P],
                        tm_sb[:],
                        MULT,
                    )
                ctx["pp"] = pp

            # ---- main interleaved schedule ----
            issue_x_dma(1)
            st0_steps = (
                proj_qk_steps("q", qT_sb, 0)
                + proj_qk_steps("k", kT_sb, 0)
                + proj_v_steps(0)
            )
            # weave chain pairs: both chains' kt0-3 chunks first (their
            # DMA half lands first), then kt4-7 — two chains open max
            # (psum bufs=2), and the kt4+ work starts after the second
            # DMA half has landed
            order = []
            nch = len(st0_steps) // 4
            for c0 in range(0, nch, 2):
                a, b = 4 * c0, 4 * (c0 + 1)
                order += [st0_steps[a], st0_steps[a + 1]]
                if c0 + 1 < nch:
                    order += [st0_steps[b], st0_steps[b + 1]]
                order += [st0_steps[a + 2], st0_steps[a + 3]]
                if c0 + 1 < nch:
                    order += [st0_steps[b + 2], st0_steps[b + 3]]
            for step in order:
                step()

            # ---- flat block stream across all q-tiles ----
            # one continuous pend-ahead pipeline (scores 2 ahead, exp 1
            # ahead) so nothing resets at q-tile boundaries
            all_units = []
            for qt in range(QT):
                for hp in range(MSUB):
                    for jt in range(4 * (qt + 1)):
                        all_units.append((qt, hp, jt))
            NU = len(all_units)
            qt_base = {}
            for wi, (qt, hp, jt) in enumerate(all_units):
                if qt not in qt_base:
                    qt_base[qt] = wi

            ohTs = []
            sweep_pv = {}
            pend_tp = []  # (ohq_t, ohT, hp, h, qs, wi_pushed)
            pend_cp = []  # (tpt, ohT, hp, h, qs, wi_emitted)
            pend_pv = []  # sweep-start pv emissions deferred one block
            ohq2 = {}  # (hp, qs) -> paired ohq tile awaiting both heads
            ctxs = {}
            filler_state = {"fillers": [], "fi": 0, "nf": 0, "nfront": 0}

            # oh transpose via regular matmul against the identity:
            # out[d, j] = sum_q ohq[q, d] * I[q, j] = ohq.T.  Each
            # transpose gets its own chain-pool generation — PSUM
            # start=True zeroes lazily at bank granularity, so an
            # accumulator bank must never host two live groups.
            def do_transpose(ent, wi):
                # both heads' ohq halves sit in one [P, 2, DK] tile: a
                # single N=128 matmul against the identity transposes the
                # pair straight into the [dk-pair, q] layout ohT wants
                ohq_t, ohT_e, ehp, qs = ent
                tpt = ps_c.tile([P, 512], F32, tag="pp", name="tp")
                nc.tensor.matmul(
                    tpt[:, 0:P],
                    ohq_t[:],
                    id_sb[:],
                    start=True, stop=True,
                    skip_group_check=True,
                )
                pend_cp.append((tpt, ohT_e, ehp, qs, wi))

            def do_copy(ent):
                tpt, ohT_e, ehp, qs, _ = ent
                nc.vector.tensor_copy(
                    ohT_e[:, ehp, qs * P : (qs + 1) * P],
                    tpt[:, 0:P],
                )

            def qt_fillers(qt):
                """Filler steps to interleave into q-tile qt's blocks."""
                fillers = []
                nfront = 0
                if qt + 1 < QT:
                    st = qt + 1
                    fillers += proj_qk_steps("q", qT_sb, st)
                    if st < QT - 1:
                        fillers += proj_qk_steps("k", kT_sb, st)
                        fillers += proj_v_steps(st)
                if qt == QT - 1:
                    # k/v(st3) deferred into qt3 (exp-bound): feeds jt>=12,
                    # so front-load it within the first blocks
                    kv3 = proj_qk_steps("k", kT_sb, 3) + proj_v_steps(3)
                    fillers = kv3 + fillers
                    nfront = len(kv3)
                # outproj runs 2 q-tiles late: the late q-tiles are
                # exp-bound and need the extra PE filler
                if qt == 2:
                    fillers += outproj_steps(ohTs[0], 0)
                elif qt == 3:
                    fillers += outproj_steps(ohTs[1], 1)
                    fillers += outproj_steps(ohTs[2], 2)
                return fillers, nfront

            # prologue for block 0/1 of qt0
            ctxs[0] = {}
            emit_scores(all_units[0][0], *all_units[0][1:], ctxs[0])
            emit_exp(ctxs[0])
            ctxs[1] = {}
            emit_scores(all_units[1][0], *all_units[1][1:], ctxs[1])
            emit_exp(ctxs[1])

            for wi, (qt, hp, jt) in enumerate(all_units):
                if wi == qt_base[qt]:
                    # q-tile entry: flush previous fillers, set up new ones
                    while filler_state["fi"] < filler_state["nf"]:
                        filler_state["fillers"][filler_state["fi"]]()
                        filler_state["fi"] += 1
                    fillers, nfront = qt_fillers(qt)
                    filler_state = {
                        "fillers": fillers,
                        "fi": 0,
                        "nf": len(fillers),
                        "nfront": nfront,
                    }
                    if qt + 2 < QT:
                        issue_x_dma(qt + 2)
                    ohT = ohp.tile([P, MSUB, 512], BF16, tag="ohT")
                    ohTs.append(ohT)
                    nu_qt = (
                        qt_base[qt + 1] - qt_base[qt]
                        if qt + 1 < QT
                        else NU - qt_base[qt]
                    )
                ctx = ctxs.pop(wi)
                # epilogue pipeline: transpose ~2 blocks after the DVE
                # scale was issued, ohT copy ~1 block after the transpose
                while pend_tp and pend_tp[0][4] <= wi - 2:
                    ent = pend_tp.pop(0)
                    do_transpose(ent[:4], wi)
                while pend_cp and pend_cp[0][4] <= wi - 1:
                    do_copy(pend_cp.pop(0))
                # filler quota (local block index within this q-tile)
                bi = wi - qt_base[qt]
                nf = filler_state["nf"]
                nfront = filler_state["nfront"]
                want = (bi + 1) * nf // nu_qt
                if nfront and bi < 12:
                    want = max(want, min(nfront, (bi + 1) * nfront // 11))
                # at sweep starts the pv-tile WAR wait (previous sweep's
                # epilogue reads on DVE) stalls the PE — run a filler
                # first so the DVE drains behind real PE work
                sc_emitted = False
                if jt <= 1 and filler_state["fi"] < want:
                    filler_state["fillers"][filler_state["fi"]]()
                    filler_state["fi"] += 1
                # PV for current block (both heads)
                def emit_pv(qt, hp, jt, pp, di, ohT_e, wi):
                    q0 = di if di > 0 else 0
                    for h in range(2):
                        pv_t = sweep_pv[hp, h]
                        hh = hp * 2 + h
                        for qs in range(q0, 4):
                            last = jt == qt * 4 + qs
                            # start only on the bank's first group touch:
                            # PSUM start zeroes the whole bank lazily, so
                            # sibling slices rely on that single mark
                            nc.tensor.matmul(
                                pv_t[:, qs, :],
                                pp[:, h, qs * P : (qs + 1) * P],
                                v_sb[:, jt, hh, :],
                                start=(jt == 0 and qs == q0), stop=last,
                                skip_group_check=True,
                            )
                            if last:
                                # epilogue DVE: 1/z then scale into sbuf;
                                # the heads share one [P, 2, DK] tile so
                                # one matmul transposes the pair
                                zr = zrp.tile([P, 1], F32, tag="zr")
                                nc.vector.reciprocal(
                                    zr[:], pv_t[:, qs, DK : DK + 1]
                                )
                                if h == 0:
                                    ohq2[hp, qs] = ohqp.tile(
                                        [P, 2, DK], BF16, tag="ohq",
                                        name="ohq2",
                                    )
                                nc.vector.tensor_scalar_mul(
                                    ohq2[hp, qs][:, h, :],
                                    pv_t[:, qs, 0:DK],
                                    zr[:],
                                )
                                if h == 1:
                                    pend_tp.append(
                                        (ohq2.pop((hp, qs)), ohT_e, hp, qs, wi)
                                    )

                pp = ctx["pp"]
                di = ctx["di"]
                if jt == 0:
                    # defer the new sweep's first pv matmuls one block:
                    # the old generation's epilogue reads (DVE) get a full
                    # block to clear the pv-slot WAR before the start
                    for h in range(2):
                        sweep_pv[hp, h] = ps_v.tile(
                            [P, 4, DK + 1], F32, tag="pv", name="pv"
                        )
                    pend_pv.append((qt, hp, jt, pp, di, ohT, wi))
                else:
                    while pend_pv:
                        emit_pv(*pend_pv.pop(0))
                    emit_pv(qt, hp, jt, pp, di, ohT, wi)
                # emit scores for block wi+2 (pend-ahead depth 2), then
                # its exp straight into the ACT queue
                if wi + 2 < NU and not sc_emitted:
                    ctxs[wi + 2] = {}
                    u = all_units[wi + 2]
                    emit_scores(u[0], *u[1:], ctxs[wi + 2])
                if wi + 2 < NU and "pp" not in ctxs[wi + 2]:
                    emit_exp(ctxs[wi + 2])
                # filler quota
                while filler_state["fi"] < want:
                    filler_state["fillers"][filler_state["fi"]]()
                    filler_state["fi"] += 1

            # pre-open the first two final outproj chains (mt 0..2 only
            # — those ohT pieces are long done); their matmuls are fed
            # one-at-a-time between the drain's transposes so the PE has
            # cover while the last epilogues clear the DVE
            ohT3 = ohTs[3]
            fin_pys = []
            fin_steps = []
            for nt in range(2):
                py = ps_v.tile([P, 512], F32, tag="pv", name="pyf0")
                for mt in range(MSUB - 1):
                    def fs(py=py, mt=mt, nt=nt):
                        nc.tensor.matmul(
                            py[:],
                            wo_sb[:, mt, nt * P : (nt + 1) * P],
                            ohT3[:, mt, :],
                            start=(mt == 0), stop=False,
                        )
                    fin_steps.append(fs)
                fin_pys.append(py)

            # drain the tail pipeline, fin/filler work interleaved
            nrem = len(pend_tp)
            fsi = 0
            for ri in range(nrem):
                ent = pend_tp.pop(0)
                if filler_state["fi"] < filler_state["nf"]:
                    filler_state["fillers"][filler_state["fi"]]()
                    filler_state["fi"] += 1
                elif fsi < len(fin_steps):
                    fin_steps[fsi]()
                    fsi += 1
                do_transpose(ent[:4], NU + ri)
                while pend_cp and pend_cp[0][4] <= NU + ri - 1:
                    do_copy(pend_cp.pop(0))
            while filler_state["fi"] < filler_state["nf"]:
                filler_state["fillers"][filler_state["fi"]]()
                filler_state["fi"] += 1
            while fsi < len(fin_steps):
                fin_steps[fsi]()
                fsi += 1
            while pend_cp:
                do_copy(pend_cp.pop(0))

            # final output projection for qt=3: per-nt eviction + DMA on
            # alternating engines so the tail after the last matmul is
            # one small copy + one 1KB-row DMA.  Chains come from the
            # (now idle) pv pool so they don't contend with the drain's
            # transpose generations in the chain pool.
            qt3 = 3
            for nt in range(NT):
                y1f = ysp.tile([P, 512], BF16, tag="y2", name="y1f")
                if nt < 2:
                    py = fin_pys[nt]
                    nc.tensor.matmul(
                        py[:],
                        wo_sb[:, MSUB - 1, nt * P : (nt + 1) * P],
                        ohT3[:, MSUB - 1, :],
                        start=False, stop=True,
                    )
                elif nt < NT - 1:
                    py = ps_v.tile([P, 512], F32, tag="pv", name="pyf")
                    for mt in range(MSUB):
                        nc.tensor.matmul(
                            py[:],
                            wo_sb[:, mt, nt * P : (nt + 1) * P],
                            ohT3[:, mt, :],
                            start=(mt == 0), stop=(mt == MSUB - 1),
                        )
                else:
                    # last nt: two half-N chains; first half's eviction and
                    # DMA overlap the second half's matmuls
                    py = ps_v.tile([P, 512], F32, tag="pv", name="pyf")
                    for half in range(2):
                        hs = slice(half * 256, (half + 1) * 256)
                        for mt in range(MSUB):
                            nc.tensor.matmul(
                                py[:, hs],
                                wo_sb[:, mt, nt * P : (nt + 1) * P],
                                ohT3[:, mt, half * 256 : (half + 1) * 256],
                                start=(mt == 0 and half == 0),
                                stop=(mt == MSUB - 1 and half == 1),
                                skip_group_check=True,
                            )
                        if half == 0:
                            nc.vector.tensor_copy(y1f[:, hs], py[:, hs])
                            nc.sync.dma_start(yT4[qt3, :, nt, hs], y1f[:, hs])
                        else:
                            nc.scalar.copy(y1f[:, hs], py[:, hs])
                            nc.scalar.dma_start(yT4[qt3, :, nt, hs], y1f[:, hs])
                    continue
                if nt % 2 == 0:
                    nc.vector.tensor_copy(y1f[:], py[:])
                    nc.sync.dma_start(yT4[qt3, :, nt, :], y1f[:])
                else:
                    nc.scalar.copy(y1f[:], py[:])
                    nc.scalar.dma_start(yT4[qt3, :, nt, :], y1f[:])

    nc.finalize()
    return nc


_CACHED_NC = None


def _get_nc() -> bass.Bass:
    global _CACHED_NC
    if _CACHED_NC is None:
        _CACHED_NC = build_nc()
    return _CACHED_NC


def _make_tm2() -> np.ndarray:
    import ml_dtypes

    k = np.arange(P)[:, None]
    j = np.arange(P)[None, :]
    tm = np.where(j >= k, 1.0, 0.0).astype(np.float32)
    return np.stack([tm, tm], axis=1).astype(ml_dtypes.bfloat16)


def _make_ident() -> np.ndarray:
    import ml_dtypes

    return np.eye(P, dtype=np.float32).astype(ml_dtypes.bfloat16)


def make_in_maps(inputs):
    import ml_dtypes

    bf = ml_dtypes.bfloat16
    x = np.asarray(inputs["x"], np.float32)
    q_heads = np.asarray(inputs["q_heads"], np.float32)
    k_heads = np.asarray(inputs["k_heads"], np.float32)
    v_heads = np.asarray(inputs["v_heads"], np.float32)
    output_proj = np.asarray(inputs["output_proj"], np.float32)

    tm = _make_tm2()
    idn = _make_ident()

    def tile_w(wT):  # [1024, 512] -> [p, kt, m]
        return np.ascontiguousarray(
            wT.reshape(KT, P, MG).transpose(1, 0, 2)
        ).astype(bf)

    in_maps = []
    for core in range(N_CORES):
        b, g = divmod(core, 2)
        gsl = slice(g * MG, (g + 1) * MG)
        xT = x[b].T  # [1024, 2048]
        xt4 = np.ascontiguousarray(
            xT.reshape(KT, P, QT, 512).transpose(2, 1, 0, 3)
        ).astype(bf)  # [st, p, kt, 512]
        wo = output_proj[:, gsl].T  # [512, 1024]
        wo4 = np.ascontiguousarray(
            wo.reshape(MSUB, P, D_MODEL).transpose(1, 0, 2)
        ).astype(bf)
        in_maps.append(
            {
                "xt4": xt4,
                "wq4": tile_w(q_heads[gsl].T),
                "wk4": tile_w(k_heads[gsl].T),
                "wv4": tile_w(v_heads[gsl].T),
                "wo4": wo4,
                "tm2": tm,
                "ident": idn,
            }
        )
    return in_maps


def kernel(x, q_heads, k_heads, v_heads, output_proj):
    inputs = {
        "x": x,
        "q_heads": q_heads,
        "k_heads": k_heads,
        "v_heads": v_heads,
        "output_proj": output_proj,
    }
    in_maps = make_in_maps(inputs)
    nc = _get_nc()
    res = run_bass_kernel_spmd(nc, in_maps, list(range(N_CORES)))
    y = np.empty((B, S, D_MODEL), np.float32)
    for b in range(B):
        acc = res.results[2 * b]["yT4"].astype(np.float32) + res.results[
            2 * b + 1
        ]["yT4"].astype(np.float32)
        yT = acc.transpose(2, 1, 0, 3).reshape(D_MODEL, S)
        y[b] = yT.T
    return y


# revision 55
# speedup vs baseline: 12579.2136x; 1.0145x over previous
"""Causal MHA on 8 trn2 cores — v3: transposed-PV schedule.

Sharding: 8 cores = 4 batches x 2 head-groups (8 heads each).

v3 changes vs v2:
- PV computed transposed: out[q, dk] = pp[keys, q].T @ v[keys, dk] —
  stationary = probs tile, moving = v (N=64) — halves PV matmul time
  (cost scales with moving free size; old orientation paid N=512 for
  M=65 useful rows).
- Z (softmax denom) via N=1 ones-column matmuls chained like PV.
- causal mask as a post-exp DVE 0/1 multiply (off the PE).
- per-(head,qsub) epilogue: reciprocal + per-partition tensor_scalar
  scale, then a PE transpose places oh back in [dk, q] for the output
  projection (tile_position lands head 1 in partitions 64:128).
- per-head scores/exp (ss [P,512] f32, 3 PSUM banks) with 2-deep
  pend-ahead so ACT stays fed through the exp-bound late q-tiles.

PSUM budget (8 banks): ss 3 + pv 2 + chain 2 + (z, tp slivers) 1.
"""

import sys

if "/opt/trn_rl_repo" not in sys.path:
    sys.path.insert(0, "/opt/trn_rl_repo")

import numpy as np

import concourse.bass as bass
import concourse.mybir as mybir
from concourse import bacc, tile
from concourse.bass_utils import run_bass_kernel_spmd

P = 128
D_MODEL = 1024
NUM_HEADS = 16
DK = 64
B, S = 4, 2048
HG = NUM_HEADS // 2
MG = HG * DK
N_CORES = 8

QT = S // 512
JT = S // P
KT = D_MODEL // P
MSUB = MG // P
NT = D_MODEL // P

F32 = mybir.dt.float32
BF16 = mybir.dt.bfloat16
EXP = mybir.ActivationFunctionType.Exp
MULT = mybir.AluOpType.mult


def build_nc() -> bass.Bass:
    nc = bacc.Bacc("TRN2", target_bir_lowering=False, debug=False)

    # inputs pre-tiled host-side to partition-major layout so every DMA
    # partition-row is 8KB contiguous
    xt4 = nc.dram_tensor("xt4", [QT, P, KT, 512], BF16, kind="ExternalInput")
    wq4 = nc.dram_tensor("wq4", [P, KT, MG], BF16, kind="ExternalInput")
    wk4 = nc.dram_tensor("wk4", [P, KT, MG], BF16, kind="ExternalInput")
    wv4 = nc.dram_tensor("wv4", [P, KT, MG], BF16, kind="ExternalInput")
    wo4 = nc.dram_tensor("wo4", [P, MSUB, D_MODEL], BF16, kind="ExternalInput")
    tm2 = nc.dram_tensor("tm2", [P, 2, P], BF16, kind="ExternalInput")
    ident = nc.dram_tensor("ident", [P, P], BF16, kind="ExternalInput")
    # output y^T tiled [qt, p, nt, 512]
    yT4 = nc.dram_tensor("yT4", [QT, P, NT, 512], BF16, kind="ExternalOutput")

    with tile.TileContext(nc) as tc:
        with (
            tc.tile_pool(name="wpool", bufs=1) as wpool,
            tc.tile_pool(name="qkv", bufs=1) as qkv,
            tc.tile_pool(name="xs", bufs=2) as xs,
            tc.tile_pool(name="oh", bufs=3) as ohp,
            tc.tile_pool(name="ys", bufs=4) as ysp,
            tc.tile_pool(name="attn", bufs=5) as attn,
            tc.tile_pool(name="attnc", bufs=1) as attnc,
            tc.tile_pool(name="ohq", bufs=8) as ohqp,
            tc.tile_pool(name="zr", bufs=6) as zrp,
            # PSUM pools — creation order fixes bank packing:
            # ss 3 banks, pv 2, chains 2, z+tp slivers in bank 8
            tc.tile_pool(name="ps_s", bufs=2, space="PSUM") as ps_s,
            tc.tile_pool(name="ps_v", bufs=2, space="PSUM") as ps_v,
            tc.tile_pool(name="ps_c", bufs=2, space="PSUM") as ps_c,
        ):
            # ---- persistent sbuf ----
            w_sb = {}
            for name in ("q", "k", "v"):
                w_sb[name] = wpool.tile(
                    [P, KT, MG], BF16, tag=f"w{name}", name=f"w{name}"
                )
            wo_sb = wpool.tile([P, MSUB, D_MODEL], BF16, tag="wo")
            qT_sb = qkv.tile([P, MSUB, S], BF16, tag="qT")
            kT_sb = qkv.tile([P, MSUB, S], BF16, tag="kT")
            v_sb = qkv.tile([P, JT, HG, DK + 1], BF16, tag="v")

            tm_sb = attnc.tile([P, 2, P], BF16, tag="tm")
            id_sb = attnc.tile([P, P], BF16, tag="id")

            # ---- input DMA ----
            # issue cost is ~565-667ns per dma_start, serial per engine —
            # split the startup DMAs across the two HWDGE engines (SP +
            # Activation; ACT is idle until the first exp) so the first
            # projection's dependencies land by ~2.5us
            # warm_src feeds p-state-ramp matmuls whose output is never
            # read; memset it on the (otherwise idle) gpsimd engine so the
            # first Ldweights isn't gated on the DVE pipeline spin-up
            warm_src = attnc.tile([P, 256], BF16, tag="warm_src")
            nc.gpsimd.memset(warm_src[:], 0.5)
            nc.gpsimd.memset(v_sb[:, :, :, DK : DK + 1], 1.0)
            x_tiles = [None] * QT

            def issue_x_dma(st):
                x_tiles[st] = xs.tile([P, KT, 512], BF16, tag="x", name=f"x{st}")
                for kp in range(4):
                    nc.sync.dma_start(
                        x_tiles[st][:, 2 * kp : 2 * kp + 2],
                        xt4[st, :, 2 * kp : 2 * kp + 2],
                    )

            # SP: q/k/v weight halves then wo; ACT: x0 + mask/identity
            nc.sync.dma_start(w_sb["q"][:, 0:4], wq4[:, 0:4])
            x_tiles[0] = xs.tile([P, KT, 512], BF16, tag="x", name="x0")
            nc.scalar.dma_start(x_tiles[0][:, 0:4], xt4[0, :, 0:4])
            nc.sync.dma_start(w_sb["q"][:, 4:8], wq4[:, 4:8])
            nc.scalar.dma_start(x_tiles[0][:, 4:8], xt4[0, :, 4:8])
            for name, wsrc in (("k", wk4), ("v", wv4)):
                nc.sync.dma_start(w_sb[name][:, 0:4], wsrc[:, 0:4])
                nc.sync.dma_start(w_sb[name][:, 4:8], wsrc[:, 4:8])
            nc.scalar.dma_start(tm_sb[:], tm2[:])
            nc.scalar.dma_start(id_sb[:], ident[:])
            nc.sync.dma_start(wo_sb[:], wo4[:])

            # warm the PE while the x/w DMAs land
            warm = ps_c.tile([P, 512], F32, tag="pp", name="warm")
            for _ in range(20):
                nc.tensor.matmul(
                    warm[:, 0:256], warm_src[:, 0:P], warm_src[:],
                    start=True, stop=True, skip_group_check=True,
                )

            # ---- filler-step factories (each step = ~4 matmuls on PE) ----
            def proj_qk_steps(name, dst, st):
                ssl = slice(st * 512, (st + 1) * 512)
                w = w_sb[name]
                x_t = x_tiles[st]
                steps = []
                for mt in range(MSUB):
                    msl = slice(mt * P, (mt + 1) * P)
                    holder = {}

                    def sk(k0, k1, mt=mt, msl=msl, holder=holder):
                        if k0 == 0:
                            holder["pt"] = ps_c.tile(
                                [P, 512], F32, tag="pp", name="prq"
                            )
                        pt = holder["pt"]
                        for kt in range(k0, k1):
                            nc.tensor.matmul(
                                pt[:], w[:, kt, msl], x_t[:, kt],
                                start=(kt == 0), stop=(kt == KT - 1),
                            )
                        if k1 == KT:
                            nc.vector.tensor_copy(dst[:, mt, ssl], pt[:])

                    # 2-matmul sub-steps: finer quota placement absorbs
                    # sub-500ns PE stalls
                    for k0 in range(0, KT, 2):
                        steps.append(
                            lambda k0=k0, sk=sk: sk(k0, k0 + 2)
                        )
                return steps

            def proj_v_steps(st):
                x_t = x_tiles[st]
                steps = []
                for ssub in range(4):
                    jt = st * 4 + ssub
                    s0 = ssub * P
                    holder = {}

                    def sk(k0, k1, jt=jt, s0=s0, holder=holder):
                        if k0 == 0:
                            holder["pt"] = ps_c.tile(
                                [P, 512], F32, tag="pp", name="prv"
                            )
                        pt = holder["pt"]
                        for kt in range(k0, k1):
                            nc.tensor.matmul(
                                pt[:], x_t[:, kt, s0 : s0 + P], w_sb["v"][:, kt],
                                start=(kt == 0), stop=(kt == KT - 1),
                            )
                        if k1 == KT:
                            nc.vector.tensor_copy(
                                v_sb[:, jt, :, 0:DK],
                                pt.rearrange("p (h d) -> p h d", h=HG),
                            )

                    for k0 in range(0, KT, 2):
                        steps.append(
                            lambda k0=k0, sk=sk: sk(k0, k0 + 2)
                        )
                return steps

            def outproj_steps(ohT_prev, qt_prev):
                steps = []
                holder = {}
                for nt in range(NT):
                    def sm(m0, m1, nt=nt):
                        if m0 == 0:
                            holder["py"] = ps_c.tile(
                                [P, 512], F32, tag="pp", name="py"
                            )
                        py = holder["py"]
                        for mt in range(m0, m1):
                            nc.tensor.matmul(
                                py[:],
                                wo_sb[:, mt, nt * P : (nt + 1) * P],
                                ohT_prev[:, mt, :],
                                start=(mt == 0), stop=(mt == MSUB - 1),
                            )
                        if m1 < MSUB:
                            return
                        if nt % 4 == 0:
                            holder["y4"] = ysp.tile(
                                [P, 4, 512], BF16, tag="y", name="y4"
                            )
                        nc.vector.tensor_copy(holder["y4"][:, nt % 4, :], py[:])
                        if nt % 4 == 3:
                            nc.gpsimd.dma_start(
                                yT4[qt_prev, :, nt - 3 : nt + 1], holder["y4"][:]
                            )

                    steps.append(lambda sm=sm: sm(0, 2))
                    steps.append(lambda sm=sm: sm(2, 4))
                return steps

            # ---- attention primitives ----
            def emit_scores(qt, hp, jt, ctx):
                """Two K=64 matmuls: ss[keys, h, q] for the pair's heads."""
                jsl = slice(jt * P, (jt + 1) * P)
                di = jt - qt * 4
                delta = P * di if di >= 0 else 0
                qsl_d = slice(qt * 512 + delta, (qt + 1) * 512)
                ss = ps_s.tile([P, 2, 512], F32, tag="ss")
                for h in range(2):
                    hd = slice(h * DK, (h + 1) * DK)
                    nc.tensor.matmul(
                        ss[:, h, delta:],
                        kT_sb[hd, hp, jsl],
                        qT_sb[hd, hp, qsl_d],
                        start=True, stop=True, skip_group_check=True,
                    )
                ctx["ss"] = ss
                ctx["delta"] = delta
                ctx["di"] = di

            def emit_exp(ctx):
                ss, delta, di = ctx["ss"], ctx["delta"], ctx["di"]
                pp = attn.tile([P, 2, 512], BF16, tag="pp")
                nc.scalar.activation(
                    pp[:, :, delta:], ss[:, :, delta:], EXP, scale=0.125
                )
                if di >= 0:
                    # zero the upper-triangle of the diagonal 128-block
                    nc.vector.tensor_tensor(
                        pp[:, :, delta : delta + P],
                        pp[:, :, delta : delta + P],
                        tm_sb[:],
                        MULT,
                    )
                ctx["pp"] = pp

            # ---- main interleaved schedule ----
            issue_x_dma(1)
            st0_steps = (
                proj_qk_steps("q", qT_sb, 0)
                + proj_qk_steps("k", kT_sb, 0)
                + proj_v_steps(0)
            )
            # weave chain pairs: both chains' kt0-3 chunks first (their
            # DMA half lands first), then kt4-7 — two chains open max
            # (psum bufs=2), and the kt4+ work starts after the second
            # DMA half has landed
            order = []
            nch = len(st0_steps) // 4
            for c0 in range(0, nch, 2):
                a, b = 4 * c0, 4 * (c0 + 1)
                order += [st0_steps[a], st0_steps[a + 1]]
                if c0 + 1 < nch:
                    order += [st0_steps[b], st0_steps[b + 1]]
                order += [st0_steps[a + 2], st0_steps[a + 3]]
                if c0 + 1 < nch:
                    order += [st0_steps[b + 2], st0_steps[b + 3]]
            for step in order:
                step()

            # ---- flat block stream across all q-tiles ----
            # one continuous pend-ahead pipeline (scores 2 ahead, exp 1
            # ahead) so nothing resets at q-tile boundaries
            all_units = []
            for qt in range(QT):
                for hp in range(MSUB):
                    for jt in range(4 * (qt + 1)):
                        all_units.append((qt, hp, jt))
            NU = len(all_units)
            qt_base = {}
            for wi, (qt, hp, jt) in enumerate(all_units):
                if qt not in qt_base:
                    qt_base[qt] = wi

            ohTs = []
            sweep_pv = {}
            pend_tp = []  # (ohq_t, ohT, hp, h, qs, wi_pushed)
            pend_cp = []  # (tpt, ohT, hp, h, qs, wi_emitted)
            pend_pv = []  # sweep-start pv emissions deferred one block
            ohq2 = {}  # (hp, qs) -> paired ohq tile awaiting both heads
            ctxs = {}
            filler_state = {"fillers": [], "fi": 0, "nf": 0, "nfront": 0}

            # oh transpose via regular matmul against the identity:
            # out[d, j] = sum_q ohq[q, d] * I[q, j] = ohq.T.  Each
            # transpose gets its own chain-pool generation — PSUM
            # start=True zeroes lazily at bank granularity, so an
            # accumulator bank must never host two live groups.
            def do_transpose(ent, wi):
                # both heads' ohq halves sit in one [P, 2, DK] tile: a
                # single N=128 matmul against the identity transposes the
                # pair straight into the [dk-pair, q] layout ohT wants
                ohq_t, ohT_e, ehp, qs = ent
                tpt = ps_c.tile([P, 512], F32, tag="pp", name="tp")
                nc.tensor.matmul(
                    tpt[:, 0:P],
                    ohq_t[:],
                    id_sb[:],
                    start=True, stop=True,
                    skip_group_check=True,
                )
                pend_cp.append((tpt, ohT_e, ehp, qs, wi))

            def do_copy(ent):
                tpt, ohT_e, ehp, qs, _ = ent
                nc.vector.tensor_copy(
                    ohT_e[:, ehp, qs * P : (qs + 1) * P],
                    tpt[:, 0:P],
                )

            def qt_fillers(qt):
                """Filler steps to interleave into q-tile qt's blocks."""
                fillers = []
                nfront = 0
                if qt + 1 < QT:
                    st = qt + 1
                    fillers += proj_qk_steps("q", qT_sb, st)
                    if st < QT - 1:
                        fillers += proj_qk_steps("k", kT_sb, st)
                        fillers += proj_v_steps(st)
                if qt == QT - 1:
                    # k/v(st3) deferred into qt3 (exp-bound): feeds jt>=12,
                    # so front-load it within the first blocks
                    kv3 = proj_qk_steps("k", kT_sb, 3) + proj_v_steps(3)
                    fillers = kv3 + fillers
                    nfront = len(kv3)
                # outproj runs 2 q-tiles late: the late q-tiles are
                # exp-bound and need the extra PE filler
                if qt == 2:
                    fillers += outproj_steps(ohTs[0], 0)
                elif qt == 3:
                    fillers += outproj_steps(ohTs[1], 1)
                    fillers += outproj_steps(ohTs[2], 2)
                return fillers, nfront

            # prologue for block 0/1 of qt0
            ctxs[0] = {}
            emit_scores(all_units[0][0], *all_units[0][1:], ctxs[0])
            emit_exp(ctxs[0])
            ctxs[1] = {}
            emit_scores(all_units[1][0], *all_units[1][1:], ctxs[1])
            emit_exp(ctxs[1])

            for wi, (qt, hp, jt) in enumerate(all_units):
                if wi == qt_base[qt]:
                    # q-tile entry: flush previous fillers, set up new ones
                    while filler_state["fi"] < filler_state["nf"]:
                        filler_state["fillers"][filler_state["fi"]]()
                        filler_state["fi"] += 1
                    fillers, nfront = qt_fillers(qt)
                    filler_state = {
                        "fillers": fillers,
                        "fi": 0,
                        "nf": len(fillers),
                        "nfront": nfront,
                    }
                    if qt + 2 < QT:
                        issue_x_dma(qt + 2)
                    ohT = ohp.tile([P, MSUB, 512], BF16, tag="ohT")
                    ohTs.append(ohT)
                    nu_qt = (
                        qt_base[qt + 1] - qt_base[qt]
                        if qt + 1 < QT
                        else NU - qt_base[qt]
                    )
                ctx = ctxs.pop(wi)
                # epilogue pipeline: transpose ~2 blocks after the DVE
                # scale was issued, ohT copy ~1 block after the transpose
                while pend_tp and pend_tp[0][4] <= wi - 3:
                    ent = pend_tp.pop(0)
                    do_transpose(ent[:4], wi)
                while pend_cp and pend_cp[0][4] <= wi - 2:
                    do_copy(pend_cp.pop(0))
                # filler quota (local block index within this q-tile)
                bi = wi - qt_base[qt]
                nf = filler_state["nf"]
                nfront = filler_state["nfront"]
                want = (bi + 1) * nf // nu_qt
                if nfront and bi < 12:
                    want = max(want, min(nfront, (bi + 1) * nfront // 11))
                # at sweep starts the pv-tile WAR wait (previous sweep's
                # epilogue reads on DVE) stalls the PE — run a filler
                # first so the DVE drains behind real PE work
                sc_emitted = False
                if jt <= 1 and filler_state["fi"] < want:
                    filler_state["fillers"][filler_state["fi"]]()
                    filler_state["fi"] += 1
                # PV for current block (both heads)
                def emit_pv(qt, hp, jt, pp, di, ohT_e, wi):
                    q0 = di if di > 0 else 0
                    for h in range(2):
                        pv_t = sweep_pv[hp, h]
                        hh = hp * 2 + h
                        for qs in range(q0, 4):
                            last = jt == qt * 4 + qs
                            # start only on the bank's first group touch:
                            # PSUM start zeroes the whole bank lazily, so
                            # sibling slices rely on that single mark
                            nc.tensor.matmul(
                                pv_t[:, qs, :],
                                pp[:, h, qs * P : (qs + 1) * P],
                                v_sb[:, jt, hh, :],
                                start=(jt == 0 and qs == q0), stop=last,
                                skip_group_check=True,
                            )
                            if last:
                                # epilogue DVE: 1/z then scale into sbuf;
                                # the heads share one [P, 2, DK] tile so
                                # one matmul transposes the pair
                                zr = zrp.tile([P, 1], F32, tag="zr")
                                nc.vector.reciprocal(
                                    zr[:], pv_t[:, qs, DK : DK + 1]
                                )
                                if h == 0:
                                    ohq2[hp, qs] = ohqp.tile(
                                        [P, 2, DK], BF16, tag="ohq",
                                        name="ohq2",
                                    )
                                nc.vector.tensor_scalar_mul(
                                    ohq2[hp, qs][:, h, :],
                                    pv_t[:, qs, 0:DK],
                                    zr[:],
                                )
                                if h == 1:
                                    pend_tp.append(
                                        (ohq2.pop((hp, qs)), ohT_e, hp, qs, wi)
                                    )

                pp = ctx["pp"]
                di = ctx["di"]
                if jt == 0:
                    # defer the new sweep's first pv matmuls one block:
                    # the old generation's epilogue reads (DVE) get a full
                    # block to clear the pv-slot WAR before the start
                    for h in range(2):
                        sweep_pv[hp, h] = ps_v.tile(
                            [P, 4, DK + 1], F32, tag="pv", name="pv"
                        )
                    pend_pv.append((qt, hp, jt, pp, di, ohT, wi))
                else:
                    while pend_pv:
                        emit_pv(*pend_pv.pop(0))
                    emit_pv(qt, hp, jt, pp, di, ohT, wi)
                # emit scores for block wi+2 (pend-ahead depth 2), then
                # its exp straight into the ACT queue
                if wi + 2 < NU and not sc_emitted:
                    ctxs[wi + 2] = {}
                    u = all_units[wi + 2]
                    emit_scores(u[0], *u[1:], ctxs[wi + 2])
                if wi + 2 < NU and "pp" not in ctxs[wi + 2]:
                    emit_exp(ctxs[wi + 2])
                # filler quota
                while filler_state["fi"] < want:
                    filler_state["fillers"][filler_state["fi"]]()
                    filler_state["fi"] += 1

            # pre-open the first two final outproj chains (mt 0..2 only
            # — those ohT pieces are long done); their matmuls are fed
            # one-at-a-time between the drain's transposes so the PE has
            # cover while the last epilogues clear the DVE
            ohT3 = ohTs[3]
            fin_pys = []
            fin_steps = []
            for nt in range(2):
                py = ps_v.tile([P, 512], F32, tag="pv", name="pyf0")
                for mt in range(MSUB - 1):
                    def fs(py=py, mt=mt, nt=nt):
                        nc.tensor.matmul(
                            py[:],
                            wo_sb[:, mt, nt * P : (nt + 1) * P],
                            ohT3[:, mt, :],
                            start=(mt == 0), stop=False,
                        )
                    fin_steps.append(fs)
                fin_pys.append(py)

            # drain the tail pipeline, fin/filler work interleaved
            nrem = len(pend_tp)
            fsi = 0
            for ri in range(nrem):
                ent = pend_tp.pop(0)
                if filler_state["fi"] < filler_state["nf"]:
                    filler_state["fillers"][filler_state["fi"]]()
                    filler_state["fi"] += 1
                elif fsi < len(fin_steps):
                    fin_steps[fsi]()
                    fsi += 1
                do_transpose(ent[:4], NU + ri)
                while pend_cp and pend_cp[0][4] <= NU + ri - 1:
                    do_copy(pend_cp.pop(0))
            while filler_state["fi"] < filler_state["nf"]:
                filler_state["fillers"][filler_state["fi"]]()
                filler_state["fi"] += 1
            while fsi < len(fin_steps):
                fin_steps[fsi]()
                fsi += 1
            while pend_cp:
                do_copy(pend_cp.pop(0))

            # final output projection for qt=3: per-nt eviction + DMA on
            # alternating engines so the tail after the last matmul is
            # one small copy + one 1KB-row DMA.  Chains come from the
            # (now idle) pv pool so they don't contend with the drain's
            # transpose generations in the chain pool.
            qt3 = 3
            for nt in range(NT):
                y1f = ysp.tile([P, 512], BF16, tag="y2", name="y1f")
                if nt < 2:
                    py = fin_pys[nt]
                    nc.tensor.matmul(
                        py[:],
                        wo_sb[:, MSUB - 1, nt * P : (nt + 1) * P],
                        ohT3[:, MSUB - 1, :],
                        start=False, stop=True,
                    )
                elif nt < NT - 1:
                    py = ps_v.tile([P, 512], F32, tag="pv", name="pyf")
                    for mt in range(MSUB):
                        nc.tensor.matmul(
                            py[:],
                            wo_sb[:, mt, nt * P : (nt + 1) * P],
                            ohT3[:, mt, :],
                            start=(mt == 0), stop=(mt == MSUB - 1),
                        )
                else:
                    # last nt: two half-N chains; first half's eviction and
                    # DMA overlap the second half's matmuls
                    py = ps_v.tile([P, 512], F32, tag="pv", name="pyf")
                    for half in range(2):
                        hs = slice(half * 256, (half + 1) * 256)
                        for mt in range(MSUB):
                            nc.tensor.matmul(
                                py[:, hs],
                                wo_sb[:, mt, nt * P : (nt + 1) * P],
                                ohT3[:, mt, half * 256 : (half + 1) * 256],
                                start=(mt == 0 and half == 0),
                                stop=(mt == MSUB - 1 and half == 1),
                                skip_group_check=True,
                            )
                        if half == 0:
                            nc.vector.tensor_copy(y1f[:, hs], py[:, hs])
                            nc.sync.dma_start(yT4[qt3, :, nt, hs], y1f[:, hs])
                        else:
                            nc.scalar.copy(y1f[:, hs], py[:, hs])
                            nc.scalar.dma_start(yT4[qt3, :, nt, hs], y1f[:, hs])
                    continue
                if nt % 2 == 0:
                    nc.vector.tensor_copy(y1f[:], py[:])
                    nc.sync.dma_start(yT4[qt3, :, nt, :], y1f[:])
                else:
                    nc.scalar.copy(y1f[:], py[:])
                    nc.scalar.dma_start(yT4[qt3, :, nt, :], y1f[:])

    nc.finalize()
    return nc


_CACHED_NC = None


def _get_nc() -> bass.Bass:
    global _CACHED_NC
    if _CACHED_NC is None:
        _CACHED_NC = build_nc()
    return _CACHED_NC


def _make_tm2() -> np.ndarray:
    import ml_dtypes

    k = np.arange(P)[:, None]
    j = np.arange(P)[None, :]
    tm = np.where(j >= k, 1.0, 0.0).astype(np.float32)
    return np.stack([tm, tm], axis=1).astype(ml_dtypes.bfloat16)


def _make_ident() -> np.ndarray:
    import ml_dtypes

    return np.eye(P, dtype=np.float32).astype(ml_dtypes.bfloat16)


def make_in_maps(inputs):
    import ml_dtypes

    bf = ml_dtypes.bfloat16
    x = np.asarray(inputs["x"], np.float32)
    q_heads = np.asarray(inputs["q_heads"], np.float32)
    k_heads = np.asarray(inputs["k_heads"], np.float32)
    v_heads = np.asarray(inputs["v_heads"], np.float32)
    output_proj = np.asarray(inputs["output_proj"], np.float32)

    tm = _make_tm2()
    idn = _make_ident()

    def tile_w(wT):  # [1024, 512] -> [p, kt, m]
        return np.ascontiguousarray(
            wT.reshape(KT, P, MG).transpose(1, 0, 2)
        ).astype(bf)

    in_maps = []
    for core in range(N_CORES):
        b, g = divmod(core, 2)
        gsl = slice(g * MG, (g + 1) * MG)
        xT = x[b].T  # [1024, 2048]
        xt4 = np.ascontiguousarray(
            xT.reshape(KT, P, QT, 512).transpose(2, 1, 0, 3)
        ).astype(bf)  # [st, p, kt, 512]
        wo = output_proj[:, gsl].T  # [512, 1024]
        wo4 = np.ascontiguousarray(
            wo.reshape(MSUB, P, D_MODEL).transpose(1, 0, 2)
        ).astype(bf)
        in_maps.append(
            {
                "xt4": xt4,
                "wq4": tile_w(q_heads[gsl].T),
                "wk4": tile_w(k_heads[gsl].T),
                "wv4": tile_w(v_heads[gsl].T),
                "wo4": wo4,
                "tm2": tm,
                "ident": idn,
            }
        )
    return in_maps


def kernel(x, q_heads, k_heads, v_heads, output_proj):
    inputs = {
        "x": x,
        "q_heads": q_heads,
        "k_heads": k_heads,
        "v_heads": v_heads,
        "output_proj": output_proj,
    }
    in_maps = make_in_maps(inputs)
    nc = _get_nc()
    res = run_bass_kernel_spmd(nc, in_maps, list(range(N_CORES)))
    y = np.empty((B, S, D_MODEL), np.float32)
    for b in range(B):
        acc = res.results[2 * b]["yT4"].astype(np.float32) + res.results[
            2 * b + 1
        ]["yT4"].astype(np.float32)
        yT = acc.transpose(2, 1, 0, 3).reshape(D_MODEL, S)
        y[b] = yT.T
    return y


# revision 60
# speedup vs baseline: 12609.2202x; 1.0024x over previous
"""Causal MHA on 8 trn2 cores — v3: transposed-PV flat-stream schedule.

Sharding: 8 cores = 4 batches x 2 head-groups (8 heads each); each core
computes its batch's q/k/v projections for its 8 heads, the causal
attention, and its half of the output projection; host adds the two
half-results per batch.

Design (vs the v2 baseline):
- PV computed transposed: pv[q, dk+1] = pp[keys, q].T @ [v | 1] —
  stationary = probs tile, moving = v with a ones column (N=65 vs 512;
  matmul cost scales with the moving free size, and the old orientation
  paid N=512 for 65 useful output rows).  The ones column yields the
  softmax denominator Z in place.
- causal mask as a post-exp DVE 0/1 multiply (off the PE); scores for
  diagonal blocks are delta-trimmed.
- per-(pair,qsub) epilogue: 1/Z reciprocal + per-partition scale on
  DVE into a shared [P, 2, DK] tile, then ONE regular matmul against
  the identity transposes both heads into the [dk-pair, q] layout the
  output projection consumes (PE is_transpose works too, but the
  identity matmul needs no bf16-PSUM bitcast tricks).
- one flat block stream across all q-tiles: scores emitted 2 blocks
  ahead, exp issued 1 block ahead, sweep-start PV deferred one block
  (clears the pv-slot WAR on the previous sweep's epilogue reads),
  transposes deferred 3 blocks / ohT copies 2 (absorb DVE latency).
- q/k/v and output projections run as 2-matmul filler sub-steps
  interleaved by quota; k/v(st=3) are deferred into the exp-bound
  qt=3, output projections run two q-tiles late for the same reason.
- startup DMAs split across the SP + ACT hardware DGE queues; y DMAs
  ride the software (gpsimd) queue; the final q-tile's projection
  pre-opens two chains and splits the last one in half-N pieces so the
  tail is one small eviction + one small DMA.

PSUM banks (8, each 2KB, one live accumulation group per bank — a
matmul start zeroes its whole bank lazily): ss 2x[P,2,512] (4 banks),
pv 2x[P,4,65] (2, one start per generation), chains/transposes
2x[P,512] (2, shared tag).
"""

import sys

if "/opt/trn_rl_repo" not in sys.path:
    sys.path.insert(0, "/opt/trn_rl_repo")

import numpy as np

import concourse.bass as bass
import concourse.mybir as mybir
from concourse import bacc, tile
from concourse.bass_utils import run_bass_kernel_spmd

P = 128
D_MODEL = 1024
NUM_HEADS = 16
DK = 64
B, S = 4, 2048
HG = NUM_HEADS // 2
MG = HG * DK
N_CORES = 8

QT = S // 512
JT = S // P
KT = D_MODEL // P
MSUB = MG // P
NT = D_MODEL // P

F32 = mybir.dt.float32
BF16 = mybir.dt.bfloat16
EXP = mybir.ActivationFunctionType.Exp
MULT = mybir.AluOpType.mult


def build_nc() -> bass.Bass:
    nc = bacc.Bacc("TRN2", target_bir_lowering=False, debug=False)

    # inputs pre-tiled host-side to partition-major layout so every DMA
    # partition-row is 8KB contiguous
    xt4 = nc.dram_tensor("xt4", [QT, P, KT, 512], BF16, kind="ExternalInput")
    wq4 = nc.dram_tensor("wq4", [P, KT, MG], BF16, kind="ExternalInput")
    wk4 = nc.dram_tensor("wk4", [P, KT, MG], BF16, kind="ExternalInput")
    wv4 = nc.dram_tensor("wv4", [P, KT, MG], BF16, kind="ExternalInput")
    wo4 = nc.dram_tensor("wo4", [P, MSUB, D_MODEL], BF16, kind="ExternalInput")
    tm2 = nc.dram_tensor("tm2", [P, 2, P], BF16, kind="ExternalInput")
    ident = nc.dram_tensor("ident", [P, P], BF16, kind="ExternalInput")
    # output y^T tiled [qt, p, nt, 512]
    yT4 = nc.dram_tensor("yT4", [QT, P, NT, 512], BF16, kind="ExternalOutput")

    with tile.TileContext(nc) as tc:
        with (
            tc.tile_pool(name="wpool", bufs=1) as wpool,
            tc.tile_pool(name="qkv", bufs=1) as qkv,
            tc.tile_pool(name="xs", bufs=2) as xs,
            tc.tile_pool(name="oh", bufs=3) as ohp,
            tc.tile_pool(name="ys", bufs=4) as ysp,
            tc.tile_pool(name="attn", bufs=5) as attn,
            tc.tile_pool(name="attnc", bufs=1) as attnc,
            tc.tile_pool(name="ohq", bufs=8) as ohqp,
            tc.tile_pool(name="zr", bufs=6) as zrp,
            # PSUM pools — creation order fixes bank packing:
            # ss 3 banks, pv 2, chains 2, z+tp slivers in bank 8
            tc.tile_pool(name="ps_s", bufs=2, space="PSUM") as ps_s,
            tc.tile_pool(name="ps_v", bufs=2, space="PSUM") as ps_v,
            tc.tile_pool(name="ps_c", bufs=2, space="PSUM") as ps_c,
        ):
            # ---- persistent sbuf ----
            w_sb = {}
            for name in ("q", "k", "v"):
                w_sb[name] = wpool.tile(
                    [P, KT, MG], BF16, tag=f"w{name}", name=f"w{name}"
                )
            wo_sb = wpool.tile([P, MSUB, D_MODEL], BF16, tag="wo")
            qT_sb = qkv.tile([P, MSUB, S], BF16, tag="qT")
            kT_sb = qkv.tile([P, MSUB, S], BF16, tag="kT")
            v_sb = qkv.tile([P, JT, HG, DK + 1], BF16, tag="v")

            tm_sb = attnc.tile([P, 2, P], BF16, tag="tm")
            id_sb = attnc.tile([P, P], BF16, tag="id")

            # ---- input DMA ----
            # issue cost is ~565-667ns per dma_start, serial per engine —
            # split the startup DMAs across the two HWDGE engines (SP +
            # Activation; ACT is idle until the first exp) so the first
            # projection's dependencies land by ~2.5us
            # warm_src feeds p-state-ramp matmuls whose output is never
            # read; memset it on the (otherwise idle) gpsimd engine so the
            # first Ldweights isn't gated on the DVE pipeline spin-up
            warm_src = attnc.tile([P, 256], BF16, tag="warm_src")
            nc.gpsimd.memset(warm_src[:], 0.5)
            nc.gpsimd.memset(v_sb[:, :, :, DK : DK + 1], 1.0)
            x_tiles = [None] * QT

            def issue_x_dma(st):
                x_tiles[st] = xs.tile([P, KT, 512], BF16, tag="x", name=f"x{st}")
                for kp in range(4):
                    nc.sync.dma_start(
                        x_tiles[st][:, 2 * kp : 2 * kp + 2],
                        xt4[st, :, 2 * kp : 2 * kp + 2],
                    )

            # SP: q/k/v weight halves then wo; ACT: x0 + mask/identity
            nc.sync.dma_start(w_sb["q"][:, 0:4], wq4[:, 0:4])
            x_tiles[0] = xs.tile([P, KT, 512], BF16, tag="x", name="x0")
            nc.scalar.dma_start(x_tiles[0][:, 0:4], xt4[0, :, 0:4])
            nc.sync.dma_start(w_sb["q"][:, 4:8], wq4[:, 4:8])
            nc.scalar.dma_start(x_tiles[0][:, 4:8], xt4[0, :, 4:8])
            for name, wsrc in (("k", wk4), ("v", wv4)):
                nc.sync.dma_start(w_sb[name][:, 0:4], wsrc[:, 0:4])
                nc.sync.dma_start(w_sb[name][:, 4:8], wsrc[:, 4:8])
            nc.scalar.dma_start(tm_sb[:], tm2[:])
            nc.scalar.dma_start(id_sb[:], ident[:])
            nc.sync.dma_start(wo_sb[:], wo4[:])

            # warm the PE while the x/w DMAs land
            warm = ps_c.tile([P, 512], F32, tag="pp", name="warm")
            for _ in range(24):
                nc.tensor.matmul(
                    warm[:, 0:256], warm_src[:, 0:P], warm_src[:],
                    start=True, stop=True, skip_group_check=True,
                )

            # ---- filler-step factories (each step = ~4 matmuls on PE) ----
            def proj_qk_steps(name, dst, st):
                ssl = slice(st * 512, (st + 1) * 512)
                w = w_sb[name]
                x_t = x_tiles[st]
                steps = []
                for mt in range(MSUB):
                    msl = slice(mt * P, (mt + 1) * P)
                    holder = {}

                    def sk(k0, k1, mt=mt, msl=msl, holder=holder):
                        if k0 == 0:
                            holder["pt"] = ps_c.tile(
                                [P, 512], F32, tag="pp", name="prq"
                            )
                        pt = holder["pt"]
                        for kt in range(k0, k1):
                            nc.tensor.matmul(
                                pt[:], w[:, kt, msl], x_t[:, kt],
                                start=(kt == 0), stop=(kt == KT - 1),
                            )
                        if k1 == KT:
                            nc.vector.tensor_copy(dst[:, mt, ssl], pt[:])

                    # 2-matmul sub-steps: finer quota placement absorbs
                    # sub-500ns PE stalls
                    for k0 in range(0, KT, 2):
                        steps.append(
                            lambda k0=k0, sk=sk: sk(k0, k0 + 2)
                        )
                return steps

            def proj_v_steps(st):
                x_t = x_tiles[st]
                steps = []
                for ssub in range(4):
                    jt = st * 4 + ssub
                    s0 = ssub * P
                    holder = {}

                    def sk(k0, k1, jt=jt, s0=s0, holder=holder):
                        if k0 == 0:
                            holder["pt"] = ps_c.tile(
                                [P, 512], F32, tag="pp", name="prv"
                            )
                        pt = holder["pt"]
                        for kt in range(k0, k1):
                            nc.tensor.matmul(
                                pt[:], x_t[:, kt, s0 : s0 + P], w_sb["v"][:, kt],
                                start=(kt == 0), stop=(kt == KT - 1),
                            )
                        if k1 == KT:
                            nc.vector.tensor_copy(
                                v_sb[:, jt, :, 0:DK],
                                pt.rearrange("p (h d) -> p h d", h=HG),
                            )

                    for k0 in range(0, KT, 2):
                        steps.append(
                            lambda k0=k0, sk=sk: sk(k0, k0 + 2)
                        )
                return steps

            def outproj_steps(ohT_prev, qt_prev):
                steps = []
                holder = {}
                for nt in range(NT):
                    def sm(m0, m1, nt=nt):
                        if m0 == 0:
                            holder["py"] = ps_c.tile(
                                [P, 512], F32, tag="pp", name="py"
                            )
                        py = holder["py"]
                        for mt in range(m0, m1):
                            nc.tensor.matmul(
                                py[:],
                                wo_sb[:, mt, nt * P : (nt + 1) * P],
                                ohT_prev[:, mt, :],
                                start=(mt == 0), stop=(mt == MSUB - 1),
                            )
                        if m1 < MSUB:
                            return
                        if nt % 4 == 0:
                            holder["y4"] = ysp.tile(
                                [P, 4, 512], BF16, tag="y", name="y4"
                            )
                        nc.vector.tensor_copy(holder["y4"][:, nt % 4, :], py[:])
                        if nt % 4 == 3:
                            nc.gpsimd.dma_start(
                                yT4[qt_prev, :, nt - 3 : nt + 1], holder["y4"][:]
                            )

                    steps.append(lambda sm=sm: sm(0, 2))
                    steps.append(lambda sm=sm: sm(2, 4))
                return steps

            # ---- attention primitives ----
            def emit_scores(qt, hp, jt, ctx):
                """Two K=64 matmuls: ss[keys, h, q] for the pair's heads."""
                jsl = slice(jt * P, (jt + 1) * P)
                di = jt - qt * 4
                delta = P * di if di >= 0 else 0
                qsl_d = slice(qt * 512 + delta, (qt + 1) * 512)
                ss = ps_s.tile([P, 2, 512], F32, tag="ss")
                for h in range(2):
                    hd = slice(h * DK, (h + 1) * DK)
                    nc.tensor.matmul(
                        ss[:, h, delta:],
                        kT_sb[hd, hp, jsl],
                        qT_sb[hd, hp, qsl_d],
                        start=True, stop=True, skip_group_check=True,
                    )
                ctx["ss"] = ss
                ctx["delta"] = delta
                ctx["di"] = di

            def emit_exp(ctx):
                ss, delta, di = ctx["ss"], ctx["delta"], ctx["di"]
                pp = attn.tile([P, 2, 512], BF16, tag="pp")
                nc.scalar.activation(
                    pp[:, :, delta:], ss[:, :, delta:], EXP, scale=0.125
                )
                if di >= 0:
                    # zero the upper-triangle of the diagonal 128-block
                    nc.vector.tensor_tensor(
                        pp[:, :, delta : delta + P],
                        pp[:, :, delta : delta + P],
                        tm_sb[:],
                        MULT,
                    )
                ctx["pp"] = pp

            # ---- main interleaved schedule ----
            issue_x_dma(1)
            st0_steps = (
                proj_qk_steps("q", qT_sb, 0)
                + proj_qk_steps("k", kT_sb, 0)
                + proj_v_steps(0)
            )
            # weave chain pairs: both chains' kt0-3 chunks first (their
            # DMA half lands first), then kt4-7 — two chains open max
            # (psum bufs=2), and the kt4+ work starts after the second
            # DMA half has landed
            order = []
            nch = len(st0_steps) // 4
            for c0 in range(0, nch, 2):
                a, b = 4 * c0, 4 * (c0 + 1)
                order += [st0_steps[a], st0_steps[a + 1]]
                if c0 + 1 < nch:
                    order += [st0_steps[b], st0_steps[b + 1]]
                order += [st0_steps[a + 2], st0_steps[a + 3]]
                if c0 + 1 < nch:
                    order += [st0_steps[b + 2], st0_steps[b + 3]]
            for step in order:
                step()

            # ---- flat block stream across all q-tiles ----
            # one continuous pend-ahead pipeline (scores 2 ahead, exp 1
            # ahead) so nothing resets at q-tile boundaries
            all_units = []
            for qt in range(QT):
                for hp in range(MSUB):
                    for jt in range(4 * (qt + 1)):
                        all_units.append((qt, hp, jt))
            NU = len(all_units)
            qt_base = {}
            for wi, (qt, hp, jt) in enumerate(all_units):
                if qt not in qt_base:
                    qt_base[qt] = wi

            ohTs = []
            sweep_pv = {}
            pend_tp = []  # (ohq_t, ohT, hp, h, qs, wi_pushed)
            pend_cp = []  # (tpt, ohT, hp, h, qs, wi_emitted)
            pend_pv = []  # sweep-start pv emissions deferred one block
            ohq2 = {}  # (hp, qs) -> paired ohq tile awaiting both heads
            ctxs = {}
            filler_state = {"fillers": [], "fi": 0, "nf": 0, "nfront": 0}

            # oh transpose via regular matmul against the identity:
            # out[d, j] = sum_q ohq[q, d] * I[q, j] = ohq.T.  Each
            # transpose gets its own chain-pool generation — PSUM
            # start=True zeroes lazily at bank granularity, so an
            # accumulator bank must never host two live groups.
            def do_transpose(ent, wi):
                # both heads' ohq halves sit in one [P, 2, DK] tile: a
                # single N=128 matmul against the identity transposes the
                # pair straight into the [dk-pair, q] layout ohT wants
                ohq_t, ohT_e, ehp, qs = ent
                tpt = ps_c.tile([P, 512], F32, tag="pp", name="tp")
                nc.tensor.matmul(
                    tpt[:, 0:P],
                    ohq_t[:],
                    id_sb[:],
                    start=True, stop=True,
                    skip_group_check=True,
                )
                pend_cp.append((tpt, ohT_e, ehp, qs, wi))

            def do_copy(ent):
                tpt, ohT_e, ehp, qs, _ = ent
                nc.vector.tensor_copy(
                    ohT_e[:, ehp, qs * P : (qs + 1) * P],
                    tpt[:, 0:P],
                )

            def qt_fillers(qt):
                """Filler steps to interleave into q-tile qt's blocks."""
                fillers = []
                nfront = 0
                if qt + 1 < QT:
                    st = qt + 1
                    fillers += proj_qk_steps("q", qT_sb, st)
                    if st < QT - 1:
                        fillers += proj_qk_steps("k", kT_sb, st)
                        fillers += proj_v_steps(st)
                if qt == QT - 1:
                    # k/v(st3) deferred into qt3 (exp-bound): feeds jt>=12,
                    # so front-load it within the first blocks
                    kv3 = proj_qk_steps("k", kT_sb, 3) + proj_v_steps(3)
                    fillers = kv3 + fillers
                    nfront = len(kv3)
                # outproj runs 2 q-tiles late: the late q-tiles are
                # exp-bound and need the extra PE filler
                if qt == 2:
                    fillers += outproj_steps(ohTs[0], 0)
                elif qt == 3:
                    fillers += outproj_steps(ohTs[1], 1)
                    fillers += outproj_steps(ohTs[2], 2)
                return fillers, nfront

            # prologue for block 0/1 of qt0
            ctxs[0] = {}
            emit_scores(all_units[0][0], *all_units[0][1:], ctxs[0])
            emit_exp(ctxs[0])
            ctxs[1] = {}
            emit_scores(all_units[1][0], *all_units[1][1:], ctxs[1])
            emit_exp(ctxs[1])

            for wi, (qt, hp, jt) in enumerate(all_units):
                if wi == qt_base[qt]:
                    # q-tile entry: flush previous fillers, set up new ones
                    while filler_state["fi"] < filler_state["nf"]:
                        filler_state["fillers"][filler_state["fi"]]()
                        filler_state["fi"] += 1
                    fillers, nfront = qt_fillers(qt)
                    filler_state = {
                        "fillers": fillers,
                        "fi": 0,
                        "nf": len(fillers),
                        "nfront": nfront,
                    }
                    if qt + 2 < QT:
                        issue_x_dma(qt + 2)
                    ohT = ohp.tile([P, MSUB, 512], BF16, tag="ohT")
                    ohTs.append(ohT)
                    nu_qt = (
                        qt_base[qt + 1] - qt_base[qt]
                        if qt + 1 < QT
                        else NU - qt_base[qt]
                    )
                ctx = ctxs.pop(wi)
                # epilogue pipeline: transpose ~2 blocks after the DVE
                # scale was issued, ohT copy ~1 block after the transpose
                while pend_tp and pend_tp[0][4] <= wi - 3:
                    ent = pend_tp.pop(0)
                    do_transpose(ent[:4], wi)
                while pend_cp and pend_cp[0][4] <= wi - 2:
                    do_copy(pend_cp.pop(0))
                # filler quota (local block index within this q-tile)
                bi = wi - qt_base[qt]
                nf = filler_state["nf"]
                nfront = filler_state["nfront"]
                want = (bi + 1) * nf // nu_qt
                if nfront and bi < 12:
                    want = max(want, min(nfront, (bi + 1) * nfront // 11))
                # at sweep starts the pv-tile WAR wait (previous sweep's
                # epilogue reads on DVE) stalls the PE — run a filler
                # first so the DVE drains behind real PE work
                sc_emitted = False
                if jt <= 1 and filler_state["fi"] < want:
                    filler_state["fillers"][filler_state["fi"]]()
                    filler_state["fi"] += 1
                # PV for current block (both heads)
                def emit_pv(qt, hp, jt, pp, di, ohT_e, wi):
                    q0 = di if di > 0 else 0
                    for h in range(2):
                        pv_t = sweep_pv[hp, h]
                        hh = hp * 2 + h
                        for qs in range(q0, 4):
                            last = jt == qt * 4 + qs
                            # start only on the bank's first group touch:
                            # PSUM start zeroes the whole bank lazily, so
                            # sibling slices rely on that single mark
                            nc.tensor.matmul(
                                pv_t[:, qs, :],
                                pp[:, h, qs * P : (qs + 1) * P],
                                v_sb[:, jt, hh, :],
                                start=(jt == 0 and qs == q0), stop=last,
                                skip_group_check=True,
                            )
                            if last:
                                # epilogue DVE: 1/z then scale into sbuf;
                                # the heads share one [P, 2, DK] tile so
                                # one matmul transposes the pair
                                zr = zrp.tile([P, 1], F32, tag="zr")
                                nc.vector.reciprocal(
                                    zr[:], pv_t[:, qs, DK : DK + 1]
                                )
                                if h == 0:
                                    ohq2[hp, qs] = ohqp.tile(
                                        [P, 2, DK], BF16, tag="ohq",
                                        name="ohq2",
                                    )
                                nc.vector.tensor_scalar_mul(
                                    ohq2[hp, qs][:, h, :],
                                    pv_t[:, qs, 0:DK],
                                    zr[:],
                                )
                                if h == 1:
                                    pend_tp.append(
                                        (ohq2.pop((hp, qs)), ohT_e, hp, qs, wi)
                                    )

                pp = ctx["pp"]
                di = ctx["di"]
                if jt == 0:
                    # defer the new sweep's first pv matmuls one block:
                    # the old generation's epilogue reads (DVE) get a full
                    # block to clear the pv-slot WAR before the start
                    for h in range(2):
                        sweep_pv[hp, h] = ps_v.tile(
                            [P, 4, DK + 1], F32, tag="pv", name="pv"
                        )
                    pend_pv.append((qt, hp, jt, pp, di, ohT, wi))
                else:
                    while pend_pv:
                        emit_pv(*pend_pv.pop(0))
                    emit_pv(qt, hp, jt, pp, di, ohT, wi)
                # emit scores for block wi+2 (pend-ahead depth 2), then
                # its exp straight into the ACT queue
                if wi + 2 < NU and not sc_emitted:
                    ctxs[wi + 2] = {}
                    u = all_units[wi + 2]
                    emit_scores(u[0], *u[1:], ctxs[wi + 2])
                if wi + 2 < NU and "pp" not in ctxs[wi + 2]:
                    emit_exp(ctxs[wi + 2])
                # filler quota
                while filler_state["fi"] < want:
                    filler_state["fillers"][filler_state["fi"]]()
                    filler_state["fi"] += 1

            # pre-open the first two final outproj chains (mt 0..2 only
            # — those ohT pieces are long done); their matmuls are fed
            # one-at-a-time between the drain's transposes so the PE has
            # cover while the last epilogues clear the DVE
            ohT3 = ohTs[3]
            fin_pys = []
            fin_steps = []
            for nt in range(2):
                py = ps_v.tile([P, 512], F32, tag="pv", name="pyf0")
                for mt in range(MSUB - 1):
                    def fs(py=py, mt=mt, nt=nt):
                        nc.tensor.matmul(
                            py[:],
                            wo_sb[:, mt, nt * P : (nt + 1) * P],
                            ohT3[:, mt, :],
                            start=(mt == 0), stop=False,
                        )
                    fin_steps.append(fs)
                fin_pys.append(py)

            # drain the tail pipeline, fin/filler work interleaved
            nrem = len(pend_tp)
            fsi = 0
            for ri in range(nrem):
                ent = pend_tp.pop(0)
                if filler_state["fi"] < filler_state["nf"]:
                    filler_state["fillers"][filler_state["fi"]]()
                    filler_state["fi"] += 1
                elif fsi < len(fin_steps):
                    fin_steps[fsi]()
                    fsi += 1
                do_transpose(ent[:4], NU + ri)
                while pend_cp and pend_cp[0][4] <= NU + ri - 1:
                    do_copy(pend_cp.pop(0))
            while filler_state["fi"] < filler_state["nf"]:
                filler_state["fillers"][filler_state["fi"]]()
                filler_state["fi"] += 1
            while fsi < len(fin_steps):
                fin_steps[fsi]()
                fsi += 1
            while pend_cp:
                do_copy(pend_cp.pop(0))

            # final output projection for qt=3: per-nt eviction + DMA on
            # alternating engines so the tail after the last matmul is
            # one small copy + one 1KB-row DMA.  Chains come from the
            # (now idle) pv pool so they don't contend with the drain's
            # transpose generations in the chain pool.
            qt3 = 3
            for nt in range(NT):
                y1f = ysp.tile([P, 512], BF16, tag="y2", name="y1f")
                if nt < 2:
                    py = fin_pys[nt]
                    nc.tensor.matmul(
                        py[:],
                        wo_sb[:, MSUB - 1, nt * P : (nt + 1) * P],
                        ohT3[:, MSUB - 1, :],
                        start=False, stop=True,
                    )
                elif nt < NT - 1:
                    py = ps_v.tile([P, 512], F32, tag="pv", name="pyf")
                    for mt in range(MSUB):
                        nc.tensor.matmul(
                            py[:],
                            wo_sb[:, mt, nt * P : (nt + 1) * P],
                            ohT3[:, mt, :],
                            start=(mt == 0), stop=(mt == MSUB - 1),
                        )
                else:
                    # last nt: two half-N chains in separate psum
                    # generations so the first half's eviction + DMA
                    # overlap the second half's matmuls
                    for half in range(2):
                        hs = slice(half * 256, (half + 1) * 256)
                        pyh = ps_v.tile([P, 256], F32, tag="pv", name="pyh")
                        for mt in range(MSUB):
                            nc.tensor.matmul(
                                pyh[:],
                                wo_sb[:, mt, nt * P : (nt + 1) * P],
                                ohT3[:, mt, half * 256 : (half + 1) * 256],
                                start=(mt == 0), stop=(mt == MSUB - 1),
                                skip_group_check=True,
                            )
                        if half == 0:
                            nc.vector.tensor_copy(y1f[:, hs], pyh[:])
                            nc.sync.dma_start(yT4[qt3, :, nt, hs], y1f[:, hs])
                        else:
                            nc.scalar.copy(y1f[:, hs], pyh[:])
                            nc.scalar.dma_start(yT4[qt3, :, nt, hs], y1f[:, hs])
                    continue
                if nt % 2 == 0:
                    nc.vector.tensor_copy(y1f[:], py[:])
                    nc.sync.dma_start(yT4[qt3, :, nt, :], y1f[:])
                else:
                    nc.scalar.copy(y1f[:], py[:])
                    nc.scalar.dma_start(yT4[qt3, :, nt, :], y1f[:])

    nc.finalize()
    return nc


_CACHED_NC = None


def _get_nc() -> bass.Bass:
    global _CACHED_NC
    if _CACHED_NC is None:
        _CACHED_NC = build_nc()
    return _CACHED_NC


def _make_tm2() -> np.ndarray:
    import ml_dtypes

    k = np.arange(P)[:, None]
    j = np.arange(P)[None, :]
    tm = np.where(j >= k, 1.0, 0.0).astype(np.float32)
    return np.stack([tm, tm], axis=1).astype(ml_dtypes.bfloat16)


def _make_ident() -> np.ndarray:
    import ml_dtypes

    return np.eye(P, dtype=np.float32).astype(ml_dtypes.bfloat16)


def make_in_maps(inputs):
    import ml_dtypes

    bf = ml_dtypes.bfloat16
    x = np.asarray(inputs["x"], np.float32)
    q_heads = np.asarray(inputs["q_heads"], np.float32)
    k_heads = np.asarray(inputs["k_heads"], np.float32)
    v_heads = np.asarray(inputs["v_heads"], np.float32)
    output_proj = np.asarray(inputs["output_proj"], np.float32)

    tm = _make_tm2()
    idn = _make_ident()

    def tile_w(wT):  # [1024, 512] -> [p, kt, m]
        return np.ascontiguousarray(
            wT.reshape(KT, P, MG).transpose(1, 0, 2)
        ).astype(bf)

    in_maps = []
    for core in range(N_CORES):
        b, g = divmod(core, 2)
        gsl = slice(g * MG, (g + 1) * MG)
        xT = x[b].T  # [1024, 2048]
        xt4 = np.ascontiguousarray(
            xT.reshape(KT, P, QT, 512).transpose(2, 1, 0, 3)
        ).astype(bf)  # [st, p, kt, 512]
        wo = output_proj[:, gsl].T  # [512, 1024]
        wo4 = np.ascontiguousarray(
            wo.reshape(MSUB, P, D_MODEL).transpose(1, 0, 2)
        ).astype(bf)
        in_maps.append(
            {
                "xt4": xt4,
                "wq4": tile_w(q_heads[gsl].T),
                "wk4": tile_w(k_heads[gsl].T),
                "wv4": tile_w(v_heads[gsl].T),
                "wo4": wo4,
                "tm2": tm,
                "ident": idn,
            }
        )
    return in_maps


def kernel(x, q_heads, k_heads, v_heads, output_proj):
    inputs = {
        "x": x,
        "q_heads": q_heads,
        "k_heads": k_heads,
        "v_heads": v_heads,
        "output_proj": output_proj,
    }
    in_maps = make_in_maps(inputs)
    nc = _get_nc()
    res = run_bass_kernel_spmd(nc, in_maps, list(range(N_CORES)))
    y = np.empty((B, S, D_MODEL), np.float32)
    for b in range(B):
        acc = res.results[2 * b]["yT4"].astype(np.float32) + res.results[
            2 * b + 1
        ]["yT4"].astype(np.float32)
        yT = acc.transpose(2, 1, 0, 3).reshape(D_MODEL, S)
        y[b] = yT.T
    return y


# revision 70
# speedup vs baseline: 12630.3651x; 1.0017x over previous
"""Causal MHA on 8 trn2 cores — transposed-PV flat-stream schedule.

Sharding: 8 cores = 4 batches x 2 head-groups (8 heads each); each core
computes its batch's q/k/v projections for its 8 heads, the causal
attention, and its half of the output projection; host adds the two
half-results per batch.

Key design points:
- PV computed transposed: pv[q, dk+1] = pp[keys, q].T @ [v | 1] —
  stationary = probs tile, moving = v with a ones column (N=65 vs the
  old orientation's N=512 for 65 useful output rows; matmul time
  scales with the moving free size).  The ones column yields the
  softmax denominator Z in place.
- causal mask as a post-exp DVE 0/1 multiply (off the PE); diagonal
  score blocks are delta-trimmed.
- per-(pair,qsub) epilogue: 1/Z + per-partition scale on DVE into a
  shared [P, 2, DK] tile; ONE regular matmul against the identity
  transposes both heads into the [dk-pair, q] layout for the output
  projection.
- one flat block stream across all q-tiles: scores emitted 2 blocks
  ahead, exp 1 ahead, sweep-start PV deferred a block (pv-slot WAR),
  transposes deferred 3 blocks / ohT copies 2 (DVE latency).
- projections run as 2-matmul filler sub-steps spread by quota;
  k/v(st=3) deferred into the exp-bound qt=3; output projections run
  two q-tiles late for the same reason.
- startup DMAs split across the SP + ACT hardware DGE queues; y DMAs
  on the software (gpsimd) queue; endgame pre-opens two chains and
  splits the last one into half-N pieces.

PSUM banks (8 x 2KB; a matmul start lazily zeroes its whole bank, so
one live accumulation group per bank, single start per generation):
ss 2x[P,2,512] (4), pv 2x[P,4,65] (2), chains/transposes 2x[P,512] (2).
"""

import sys

if "/opt/trn_rl_repo" not in sys.path:
    sys.path.insert(0, "/opt/trn_rl_repo")

import numpy as np

import concourse.bass as bass
import concourse.mybir as mybir
from concourse import bacc, tile
from concourse.bass_utils import run_bass_kernel_spmd

P = 128
D_MODEL = 1024
NUM_HEADS = 16
DK = 64
B, S = 4, 2048
HG = NUM_HEADS // 2
MG = HG * DK
N_CORES = 8

QT = S // 512
JT = S // P
KT = D_MODEL // P
MSUB = MG // P
NT = D_MODEL // P

F32 = mybir.dt.float32
BF16 = mybir.dt.bfloat16
EXP = mybir.ActivationFunctionType.Exp
MULT = mybir.AluOpType.mult


def build_nc() -> bass.Bass:
    nc = bacc.Bacc("TRN2", target_bir_lowering=False, debug=False)

    # inputs pre-tiled host-side to partition-major layout so every DMA
    # partition-row is 8KB contiguous
    xt4 = nc.dram_tensor("xt4", [QT, P, KT, 512], BF16, kind="ExternalInput")
    wq4 = nc.dram_tensor("wq4", [P, KT, MG], BF16, kind="ExternalInput")
    wk4 = nc.dram_tensor("wk4", [P, KT, MG], BF16, kind="ExternalInput")
    wv4 = nc.dram_tensor("wv4", [P, KT, MG], BF16, kind="ExternalInput")
    wo4 = nc.dram_tensor("wo4", [P, MSUB, D_MODEL], BF16, kind="ExternalInput")
    tm2 = nc.dram_tensor("tm2", [P, 2, P], BF16, kind="ExternalInput")
    ident = nc.dram_tensor("ident", [P, P], BF16, kind="ExternalInput")
    # output y^T tiled [qt, p, nt, 512]
    yT4 = nc.dram_tensor("yT4", [QT, P, NT, 512], BF16, kind="ExternalOutput")

    with tile.TileContext(nc) as tc:
        with (
            tc.tile_pool(name="wpool", bufs=1) as wpool,
            tc.tile_pool(name="qkv", bufs=1) as qkv,
            tc.tile_pool(name="xs", bufs=2) as xs,
            tc.tile_pool(name="oh", bufs=3) as ohp,
            tc.tile_pool(name="ys", bufs=4) as ysp,
            tc.tile_pool(name="attn", bufs=5) as attn,
            tc.tile_pool(name="attnc", bufs=1) as attnc,
            tc.tile_pool(name="ohq", bufs=8) as ohqp,
            tc.tile_pool(name="zr", bufs=6) as zrp,
            # PSUM pools — creation order fixes bank packing:
            # ss 3 banks, pv 2, chains 2, z+tp slivers in bank 8
            tc.tile_pool(name="ps_s", bufs=2, space="PSUM") as ps_s,
            tc.tile_pool(name="ps_v", bufs=2, space="PSUM") as ps_v,
            tc.tile_pool(name="ps_c", bufs=2, space="PSUM") as ps_c,
        ):
            # ---- persistent sbuf ----
            w_sb = {}
            for name in ("q", "k", "v"):
                w_sb[name] = wpool.tile(
                    [P, KT, MG], BF16, tag=f"w{name}", name=f"w{name}"
                )
            wo_sb = wpool.tile([P, MSUB, D_MODEL], BF16, tag="wo")
            qT_sb = qkv.tile([P, MSUB, S], BF16, tag="qT")
            kT_sb = qkv.tile([P, MSUB, S], BF16, tag="kT")
            v_sb = qkv.tile([P, JT, HG, DK + 1], BF16, tag="v")

            tm_sb = attnc.tile([P, 2, P], BF16, tag="tm")
            id_sb = attnc.tile([P, P], BF16, tag="id")

            # ---- input DMA ----
            # issue cost is ~565-667ns per dma_start, serial per engine —
            # split the startup DMAs across the two HWDGE engines (SP +
            # Activation; ACT is idle until the first exp) so the first
            # projection's dependencies land by ~2.5us
            # warm_src feeds p-state-ramp matmuls whose output is never
            # read; memset it on the (otherwise idle) gpsimd engine so the
            # first Ldweights isn't gated on the DVE pipeline spin-up
            warm_src = attnc.tile([P, 256], BF16, tag="warm_src")
            nc.gpsimd.memset(warm_src[:], 0.5)
            nc.gpsimd.memset(v_sb[:, :, :, DK : DK + 1], 1.0)
            x_tiles = [None] * QT

            def issue_x_dma(st):
                x_tiles[st] = xs.tile([P, KT, 512], BF16, tag="x", name=f"x{st}")
                for kp in range(4):
                    nc.sync.dma_start(
                        x_tiles[st][:, 2 * kp : 2 * kp + 2],
                        xt4[st, :, 2 * kp : 2 * kp + 2],
                    )

            # SP: q/k/v weight halves then wo; ACT: x0 + mask/identity
            nc.sync.dma_start(w_sb["q"][:, 0:4], wq4[:, 0:4])
            x_tiles[0] = xs.tile([P, KT, 512], BF16, tag="x", name="x0")
            nc.scalar.dma_start(x_tiles[0][:, 0:4], xt4[0, :, 0:4])
            nc.sync.dma_start(w_sb["q"][:, 4:8], wq4[:, 4:8])
            nc.scalar.dma_start(x_tiles[0][:, 4:8], xt4[0, :, 4:8])
            for name, wsrc in (("k", wk4), ("v", wv4)):
                nc.sync.dma_start(w_sb[name][:, 0:4], wsrc[:, 0:4])
                nc.sync.dma_start(w_sb[name][:, 4:8], wsrc[:, 4:8])
            nc.scalar.dma_start(tm_sb[:], tm2[:])
            nc.scalar.dma_start(id_sb[:], ident[:])
            nc.sync.dma_start(wo_sb[:], wo4[:])

            # warm the PE while the x/w DMAs land
            warm = ps_c.tile([P, 512], F32, tag="pp", name="warm")
            for _ in range(24):
                nc.tensor.matmul(
                    warm[:, 0:256], warm_src[:, 0:P], warm_src[:],
                    start=True, stop=True, skip_group_check=True,
                )

            # ---- filler-step factories (each step = ~4 matmuls on PE) ----
            def proj_qk_steps(name, dst, st):
                ssl = slice(st * 512, (st + 1) * 512)
                w = w_sb[name]
                x_t = x_tiles[st]
                steps = []
                for mt in range(MSUB):
                    msl = slice(mt * P, (mt + 1) * P)
                    holder = {}

                    def sk(k0, k1, mt=mt, msl=msl, holder=holder):
                        if k0 == 0:
                            holder["pt"] = ps_c.tile(
                                [P, 512], F32, tag="pp", name="prq"
                            )
                        pt = holder["pt"]
                        for kt in range(k0, k1):
                            nc.tensor.matmul(
                                pt[:], w[:, kt, msl], x_t[:, kt],
                                start=(kt == 0), stop=(kt == KT - 1),
                            )
                        if k1 == KT:
                            nc.vector.tensor_copy(dst[:, mt, ssl], pt[:])

                    # 2-matmul sub-steps: finer quota placement absorbs
                    # sub-500ns PE stalls
                    for k0 in range(0, KT, 2):
                        steps.append(
                            lambda k0=k0, sk=sk: sk(k0, k0 + 2)
                        )
                return steps

            def proj_v_steps(st):
                x_t = x_tiles[st]
                steps = []
                for ssub in range(4):
                    jt = st * 4 + ssub
                    s0 = ssub * P
                    holder = {}

                    def sk(k0, k1, jt=jt, s0=s0, holder=holder):
                        if k0 == 0:
                            holder["pt"] = ps_c.tile(
                                [P, 512], F32, tag="pp", name="prv"
                            )
                        pt = holder["pt"]
                        for kt in range(k0, k1):
                            nc.tensor.matmul(
                                pt[:], x_t[:, kt, s0 : s0 + P], w_sb["v"][:, kt],
                                start=(kt == 0), stop=(kt == KT - 1),
                            )
                        if k1 == KT:
                            nc.vector.tensor_copy(
                                v_sb[:, jt, :, 0:DK],
                                pt.rearrange("p (h d) -> p h d", h=HG),
                            )

                    for k0 in range(0, KT, 2):
                        steps.append(
                            lambda k0=k0, sk=sk: sk(k0, k0 + 2)
                        )
                return steps

            def outproj_steps(ohT_prev, qt_prev):
                steps = []
                holder = {}
                for nt in range(NT):
                    def sm(m0, m1, nt=nt):
                        if m0 == 0:
                            holder["py"] = ps_c.tile(
                                [P, 512], F32, tag="pp", name="py"
                            )
                        py = holder["py"]
                        for mt in range(m0, m1):
                            nc.tensor.matmul(
                                py[:],
                                wo_sb[:, mt, nt * P : (nt + 1) * P],
                                ohT_prev[:, mt, :],
                                start=(mt == 0), stop=(mt == MSUB - 1),
                            )
                        if m1 < MSUB:
                            return
                        if nt % 4 == 0:
                            holder["y4"] = ysp.tile(
                                [P, 4, 512], BF16, tag="y", name="y4"
                            )
                        nc.vector.tensor_copy(holder["y4"][:, nt % 4, :], py[:])
                        if nt % 4 == 3:
                            nc.gpsimd.dma_start(
                                yT4[qt_prev, :, nt - 3 : nt + 1], holder["y4"][:]
                            )

                    steps.append(lambda sm=sm: sm(0, 2))
                    steps.append(lambda sm=sm: sm(2, 4))
                return steps

            # ---- attention primitives ----
            def emit_scores(qt, hp, jt, ctx):
                """Two K=64 matmuls: ss[keys, h, q] for the pair's heads."""
                jsl = slice(jt * P, (jt + 1) * P)
                di = jt - qt * 4
                delta = P * di if di >= 0 else 0
                qsl_d = slice(qt * 512 + delta, (qt + 1) * 512)
                ss = ps_s.tile([P, 2, 512], F32, tag="ss")
                for h in range(2):
                    hd = slice(h * DK, (h + 1) * DK)
                    nc.tensor.matmul(
                        ss[:, h, delta:],
                        kT_sb[hd, hp, jsl],
                        qT_sb[hd, hp, qsl_d],
                        start=True, stop=True, skip_group_check=True,
                    )
                ctx["ss"] = ss
                ctx["delta"] = delta
                ctx["di"] = di

            def emit_exp(ctx):
                ss, delta, di = ctx["ss"], ctx["delta"], ctx["di"]
                pp = attn.tile([P, 2, 512], BF16, tag="pp")
                nc.scalar.activation(
                    pp[:, :, delta:], ss[:, :, delta:], EXP, scale=0.125
                )
                if di >= 0:
                    # zero the upper-triangle of the diagonal 128-block
                    nc.vector.tensor_tensor(
                        pp[:, :, delta : delta + P],
                        pp[:, :, delta : delta + P],
                        tm_sb[:],
                        MULT,
                    )
                ctx["pp"] = pp

            # ---- main interleaved schedule ----
            issue_x_dma(1)
            st0_steps = (
                proj_qk_steps("q", qT_sb, 0)
                + proj_qk_steps("k", kT_sb, 0)
                + proj_v_steps(0)
            )
            # weave chain pairs: both chains' kt0-3 chunks first (their
            # DMA half lands first), then kt4-7 — two chains open max
            # (psum bufs=2), and the kt4+ work starts after the second
            # DMA half has landed
            order = []
            nch = len(st0_steps) // 4
            for c0 in range(0, nch, 2):
                a, b = 4 * c0, 4 * (c0 + 1)
                order += [st0_steps[a], st0_steps[a + 1]]
                if c0 + 1 < nch:
                    order += [st0_steps[b], st0_steps[b + 1]]
                order += [st0_steps[a + 2], st0_steps[a + 3]]
                if c0 + 1 < nch:
                    order += [st0_steps[b + 2], st0_steps[b + 3]]
            for step in order:
                step()

            # ---- flat block stream across all q-tiles ----
            # one continuous pend-ahead pipeline (scores 2 ahead, exp 1
            # ahead) so nothing resets at q-tile boundaries
            all_units = []
            for qt in range(QT):
                for hp in range(MSUB):
                    for jt in range(4 * (qt + 1)):
                        all_units.append((qt, hp, jt))
            NU = len(all_units)
            qt_base = {}
            for wi, (qt, hp, jt) in enumerate(all_units):
                if qt not in qt_base:
                    qt_base[qt] = wi

            ohTs = []
            sweep_pv = {}
            pend_tp = []  # (ohq_t, ohT, hp, h, qs, wi_pushed)
            pend_cp = []  # (tpt, ohT, hp, h, qs, wi_emitted)
            pend_pv = []  # sweep-start pv emissions deferred one block
            ohq2 = {}  # (hp, qs) -> paired ohq tile awaiting both heads
            ctxs = {}
            filler_state = {"fillers": [], "fi": 0, "nf": 0, "nfront": 0}

            # oh transpose via regular matmul against the identity:
            # out[d, j] = sum_q ohq[q, d] * I[q, j] = ohq.T.  Each
            # transpose gets its own chain-pool generation — PSUM
            # start=True zeroes lazily at bank granularity, so an
            # accumulator bank must never host two live groups.
            def do_transpose(ent, wi):
                # both heads' ohq halves sit in one [P, 2, DK] tile: a
                # single N=128 matmul against the identity transposes the
                # pair straight into the [dk-pair, q] layout ohT wants
                ohq_t, ohT_e, ehp, qs = ent
                tpt = ps_c.tile([P, 512], F32, tag="pp", name="tp")
                nc.tensor.matmul(
                    tpt[:, 0:P],
                    ohq_t[:],
                    id_sb[:],
                    start=True, stop=True,
                    skip_group_check=True,
                )
                pend_cp.append((tpt, ohT_e, ehp, qs, wi))

            def do_copy(ent):
                tpt, ohT_e, ehp, qs, _ = ent
                nc.vector.tensor_copy(
                    ohT_e[:, ehp, qs * P : (qs + 1) * P],
                    tpt[:, 0:P],
                )

            def qt_fillers(qt):
                """Filler steps to interleave into q-tile qt's blocks."""
                fillers = []
                nfront = 0
                if qt + 1 < QT:
                    st = qt + 1
                    fillers += proj_qk_steps("q", qT_sb, st)
                    if st < QT - 1:
                        fillers += proj_qk_steps("k", kT_sb, st)
                        fillers += proj_v_steps(st)
                if qt == QT - 1:
                    # k/v(st3) deferred into qt3 (exp-bound): feeds jt>=12,
                    # so front-load it within the first blocks
                    kv3 = proj_qk_steps("k", kT_sb, 3) + proj_v_steps(3)
                    fillers = kv3 + fillers
                    nfront = len(kv3)
                # outproj runs 2 q-tiles late: the late q-tiles are
                # exp-bound and need the extra PE filler
                if qt == 2:
                    fillers += outproj_steps(ohTs[0], 0)
                elif qt == 3:
                    fillers += outproj_steps(ohTs[1], 1)
                    fillers += outproj_steps(ohTs[2], 2)
                return fillers, nfront

            # prologue for block 0/1 of qt0
            ctxs[0] = {}
            emit_scores(all_units[0][0], *all_units[0][1:], ctxs[0])
            emit_exp(ctxs[0])
            ctxs[1] = {}
            emit_scores(all_units[1][0], *all_units[1][1:], ctxs[1])
            emit_exp(ctxs[1])

            for wi, (qt, hp, jt) in enumerate(all_units):
                if wi == qt_base[qt]:
                    # q-tile entry: flush previous fillers, set up new ones
                    while filler_state["fi"] < filler_state["nf"]:
                        filler_state["fillers"][filler_state["fi"]]()
                        filler_state["fi"] += 1
                    fillers, nfront = qt_fillers(qt)
                    filler_state = {
                        "fillers": fillers,
                        "fi": 0,
                        "nf": len(fillers),
                        "nfront": nfront,
                    }
                    if qt + 2 < QT:
                        issue_x_dma(qt + 2)
                    ohT = ohp.tile([P, MSUB, 512], BF16, tag="ohT")
                    ohTs.append(ohT)
                    nu_qt = (
                        qt_base[qt + 1] - qt_base[qt]
                        if qt + 1 < QT
                        else NU - qt_base[qt]
                    )
                ctx = ctxs.pop(wi)
                # epilogue pipeline: transpose ~2 blocks after the DVE
                # scale was issued, ohT copy ~1 block after the transpose
                while pend_tp and pend_tp[0][4] <= wi - 3:
                    ent = pend_tp.pop(0)
                    do_transpose(ent[:4], wi)
                while pend_cp and pend_cp[0][4] <= wi - 2:
                    do_copy(pend_cp.pop(0))
                # filler quota (local block index within this q-tile)
                bi = wi - qt_base[qt]
                nf = filler_state["nf"]
                nfront = filler_state["nfront"]
                want = (bi + 1) * nf // nu_qt
                if nfront and bi < 12:
                    want = max(want, min(nfront, (bi + 1) * nfront // 11))
                # at sweep starts the pv-tile WAR wait (previous sweep's
                # epilogue reads on DVE) stalls the PE — run a filler
                # first so the DVE drains behind real PE work
                sc_emitted = False
                if jt <= 1 and filler_state["fi"] < want:
                    filler_state["fillers"][filler_state["fi"]]()
                    filler_state["fi"] += 1
                # PV for current block (both heads)
                def emit_pv(qt, hp, jt, pp, di, ohT_e, wi):
                    q0 = di if di > 0 else 0
                    # masked qsub (the diagonal slice) last: its stationary
                    # waits the DVE mask, so the unmasked slices give the
                    # mask a few hundred ns of cover
                    qs_order = list(range(q0 + 1, 4)) + [q0]
                    for h in range(2):
                        pv_t = sweep_pv[hp, h]
                        hh = hp * 2 + h
                        for qs in qs_order:
                            last = jt == qt * 4 + qs
                            # start only on the bank's first-executed group
                            # touch: PSUM start zeroes the whole bank
                            # lazily, so sibling slices rely on that mark
                            nc.tensor.matmul(
                                pv_t[:, qs, :],
                                pp[:, h, qs * P : (qs + 1) * P],
                                v_sb[:, jt, hh, :],
                                start=(jt == 0 and qs == qs_order[0]),
                                stop=last,
                                skip_group_check=True,
                            )
                            if last:
                                # epilogue DVE: 1/z then scale into sbuf;
                                # the heads share one [P, 2, DK] tile so
                                # one matmul transposes the pair
                                zr = zrp.tile([P, 1], F32, tag="zr")
                                nc.vector.reciprocal(
                                    zr[:], pv_t[:, qs, DK : DK + 1]
                                )
                                if h == 0:
                                    ohq2[hp, qs] = ohqp.tile(
                                        [P, 2, DK], BF16, tag="ohq",
                                        name="ohq2",
                                    )
                                nc.vector.tensor_scalar_mul(
                                    ohq2[hp, qs][:, h, :],
                                    pv_t[:, qs, 0:DK],
                                    zr[:],
                                )
                                if h == 1:
                                    pend_tp.append(
                                        (ohq2.pop((hp, qs)), ohT_e, hp, qs, wi)
                                    )

                pp = ctx["pp"]
                di = ctx["di"]
                if jt == 0:
                    # defer the new sweep's first pv matmuls one block:
                    # the old generation's epilogue reads (DVE) get a full
                    # block to clear the pv-slot WAR before the start
                    for h in range(2):
                        sweep_pv[hp, h] = ps_v.tile(
                            [P, 4, DK + 1], F32, tag="pv", name="pv"
                        )
                    pend_pv.append((qt, hp, jt, pp, di, ohT, wi))
                else:
                    while pend_pv:
                        emit_pv(*pend_pv.pop(0))
                    emit_pv(qt, hp, jt, pp, di, ohT, wi)
                # emit scores for block wi+2 (pend-ahead depth 2), then
                # its exp straight into the ACT queue
                if wi + 2 < NU and not sc_emitted:
                    ctxs[wi + 2] = {}
                    u = all_units[wi + 2]
                    emit_scores(u[0], *u[1:], ctxs[wi + 2])
                if wi + 2 < NU and "pp" not in ctxs[wi + 2]:
                    emit_exp(ctxs[wi + 2])
                # filler quota
                while filler_state["fi"] < want:
                    filler_state["fillers"][filler_state["fi"]]()
                    filler_state["fi"] += 1

            # pre-open the first two final outproj chains (mt 0..2 only
            # — those ohT pieces are long done); their matmuls are fed
            # one-at-a-time between the drain's transposes so the PE has
            # cover while the last epilogues clear the DVE
            ohT3 = ohTs[3]
            fin_pys = []
            fin_steps = []
            for nt in range(2):
                py = ps_v.tile([P, 512], F32, tag="pv", name="pyf0")
                for mt in range(MSUB - 1):
                    def fs(py=py, mt=mt, nt=nt):
                        nc.tensor.matmul(
                            py[:],
                            wo_sb[:, mt, nt * P : (nt + 1) * P],
                            ohT3[:, mt, :],
                            start=(mt == 0), stop=False,
                        )
                    fin_steps.append(fs)
                fin_pys.append(py)

            # drain the tail pipeline, fin/filler work interleaved
            nrem = len(pend_tp)
            fsi = 0
            for ri in range(nrem):
                ent = pend_tp.pop(0)
                if filler_state["fi"] < filler_state["nf"]:
                    filler_state["fillers"][filler_state["fi"]]()
                    filler_state["fi"] += 1
                elif fsi < len(fin_steps):
                    fin_steps[fsi]()
                    fsi += 1
                do_transpose(ent[:4], NU + ri)
                while pend_cp and pend_cp[0][4] <= NU + ri - 1:
                    do_copy(pend_cp.pop(0))
            while filler_state["fi"] < filler_state["nf"]:
                filler_state["fillers"][filler_state["fi"]]()
                filler_state["fi"] += 1
            while fsi < len(fin_steps):
                fin_steps[fsi]()
                fsi += 1
            while pend_cp:
                do_copy(pend_cp.pop(0))

            # final output projection for qt=3: per-nt eviction + DMA on
            # alternating engines so the tail after the last matmul is
            # one small copy + one 1KB-row DMA.  Chains come from the
            # (now idle) pv pool so they don't contend with the drain's
            # transpose generations in the chain pool.
            qt3 = 3
            for nt in range(NT):
                y1f = ysp.tile([P, 512], BF16, tag="y2", name="y1f")
                if nt < 2:
                    py = fin_pys[nt]
                    nc.tensor.matmul(
                        py[:],
                        wo_sb[:, MSUB - 1, nt * P : (nt + 1) * P],
                        ohT3[:, MSUB - 1, :],
                        start=False, stop=True,
                    )
                elif nt < NT - 1:
                    py = ps_v.tile([P, 512], F32, tag="pv", name="pyf")
                    for mt in range(MSUB):
                        nc.tensor.matmul(
                            py[:],
                            wo_sb[:, mt, nt * P : (nt + 1) * P],
                            ohT3[:, mt, :],
                            start=(mt == 0), stop=(mt == MSUB - 1),
                        )
                else:
                    # last nt: two half-N chains in separate psum
                    # generations so the first half's eviction + DMA
                    # overlap the second half's matmuls
                    for half in range(2):
                        hs = slice(half * 256, (half + 1) * 256)
                        pyh = ps_v.tile([P, 256], F32, tag="pv", name="pyh")
                        for mt in range(MSUB):
                            nc.tensor.matmul(
                                pyh[:],
                                wo_sb[:, mt, nt * P : (nt + 1) * P],
                                ohT3[:, mt, half * 256 : (half + 1) * 256],
                                start=(mt == 0), stop=(mt == MSUB - 1),
                                skip_group_check=True,
                            )
                        if half == 0:
                            nc.vector.tensor_copy(y1f[:, hs], pyh[:])
                        else:
                            nc.scalar.copy(y1f[:, hs], pyh[:])
                        # both halves' DMAs issue on SP: the SP sequencer
                        # is idle here, so descriptor generation overlaps
                        # the eviction copy instead of queueing behind it
                        # on the ACT sequencer
                        nc.sync.dma_start(yT4[qt3, :, nt, hs], y1f[:, hs])
                    continue
                if nt % 2 == 0:
                    nc.vector.tensor_copy(y1f[:], py[:])
                    nc.sync.dma_start(yT4[qt3, :, nt, :], y1f[:])
                else:
                    nc.scalar.copy(y1f[:], py[:])
                    nc.scalar.dma_start(yT4[qt3, :, nt, :], y1f[:])

    nc.finalize()
    return nc


_CACHED_NC = None


def _get_nc() -> bass.Bass:
    global _CACHED_NC
    if _CACHED_NC is None:
        _CACHED_NC = build_nc()
    return _CACHED_NC


def _make_tm2() -> np.ndarray:
    import ml_dtypes

    k = np.arange(P)[:, None]
    j = np.arange(P)[None, :]
    tm = np.where(j >= k, 1.0, 0.0).astype(np.float32)
    return np.stack([tm, tm], axis=1).astype(ml_dtypes.bfloat16)


def _make_ident() -> np.ndarray:
    import ml_dtypes

    return np.eye(P, dtype=np.float32).astype(ml_dtypes.bfloat16)


def make_in_maps(inputs):
    import ml_dtypes

    bf = ml_dtypes.bfloat16
    x = np.asarray(inputs["x"], np.float32)
    q_heads = np.asarray(inputs["q_heads"], np.float32)
    k_heads = np.asarray(inputs["k_heads"], np.float32)
    v_heads = np.asarray(inputs["v_heads"], np.float32)
    output_proj = np.asarray(inputs["output_proj"], np.float32)

    tm = _make_tm2()
    idn = _make_ident()

    def tile_w(wT):  # [1024, 512] -> [p, kt, m]
        return np.ascontiguousarray(
            wT.reshape(KT, P, MG).transpose(1, 0, 2)
        ).astype(bf)

    in_maps = []
    for core in range(N_CORES):
        b, g = divmod(core, 2)
        gsl = slice(g * MG, (g + 1) * MG)
        xT = x[b].T  # [1024, 2048]
        xt4 = np.ascontiguousarray(
            xT.reshape(KT, P, QT, 512).transpose(2, 1, 0, 3)
        ).astype(bf)  # [st, p, kt, 512]
        wo = output_proj[:, gsl].T  # [512, 1024]
        wo4 = np.ascontiguousarray(
            wo.reshape(MSUB, P, D_MODEL).transpose(1, 0, 2)
        ).astype(bf)
        in_maps.append(
            {
                "xt4": xt4,
                "wq4": tile_w(q_heads[gsl].T),
                "wk4": tile_w(k_heads[gsl].T),
                "wv4": tile_w(v_heads[gsl].T),
                "wo4": wo4,
                "tm2": tm,
                "ident": idn,
            }
        )
    return in_maps


def kernel(x, q_heads, k_heads, v_heads, output_proj):
    inputs = {
        "x": x,
        "q_heads": q_heads,
        "k_heads": k_heads,
        "v_heads": v_heads,
        "output_proj": output_proj,
    }
    in_maps = make_in_maps(inputs)
    nc = _get_nc()
    res = run_bass_kernel_spmd(nc, in_maps, list(range(N_CORES)))
    y = np.empty((B, S, D_MODEL), np.float32)
    for b in range(B):
        acc = res.results[2 * b]["yT4"].astype(np.float32) + res.results[
            2 * b + 1
        ]["yT4"].astype(np.float32)
        yT = acc.transpose(2, 1, 0, 3).reshape(D_MODEL, S)
        y[b] = yT.T
    return y
